# revision 1
# baseline (speedup 1.0000x reference)
"""MoE (8 routed experts, top-2, + shared expert) on 8 NeuronCores.

Strategy: data-parallel over tokens (1024 tokens/core), gate + all expert
weights replicated. The shared expert (hidden 4096) is split into two
H=2048 halves so the kernel is a uniform loop over 10 "virtual experts".
Dense formulation: every expert processes every token, scaled by the
(zero for unrouted) renormalized top-2 combine weight, fused into the
PSUM eviction. Gate runs in fp32 so routing decisions match the
reference; expert matmuls run in bf16 with fp32 accumulation.
"""

import numpy as np
import ml_dtypes

import concourse.bacc as bacc
import concourse.bass as bass
import concourse.tile as tile
import concourse.mybir as mybir
from concourse.bass_utils import run_bass_kernel_spmd

BF16 = ml_dtypes.bfloat16
F32 = mybir.dt.float32
BF = mybir.dt.bfloat16
AF = mybir.ActivationFunctionType
OP = mybir.AluOpType

P = 128


class Cfg:
    def __init__(self, D=1024, H=2048, E=8, n_sh=2, T=1024, n_cores=8, capm=96):
        self.D, self.H, self.E, self.n_sh, self.T = D, H, E, n_sh, T
        self.NV = E + n_sh          # virtual experts
        self.HS = n_sh * H          # shared hidden
        self.KD = D // P            # K chunks over D
        self.HCN = H // P           # h chunks over H
        self.TT = T // P            # token 128-tiles
        self.DT = (D + 511) // 512  # output d 512-tiles
        self.FT = (T + 511) // 512  # layer-1 free 512-tiles
        self.n_cores = n_cores
        self.capm = capm            # per-(expert, tile-pair) dispatch capacity
        self.NP = self.TT // 2      # token-tile pairs
        self.CAPE = self.NP * capm  # slots per expert
        self.ST = (self.CAPE + P - 1) // P  # slot 128-tiles per expert


def build_nc(cfg: Cfg):
    D, H, E, NV, T = cfg.D, cfg.H, cfg.E, cfg.NV, cfg.T
    KD, HCN, TT, DT, FT = cfg.KD, cfg.HCN, cfg.TT, cfg.DT, cfg.FT

    nc = bacc.Bacc("TRN2", target_bir_lowering=False)

    xT = nc.dram_tensor("xT", [P, KD, T], F32, kind="ExternalInput")
    w1t = nc.dram_tensor("w1t", [NV, HCN, P, KD, P], BF, kind="ExternalInput")
    w3t = nc.dram_tensor("w3t", [NV, HCN, P, KD, P], BF, kind="ExternalInput")
    w2t = nc.dram_tensor("w2t", [NV, P, HCN, D], BF, kind="ExternalInput")
    b1a = nc.dram_tensor("b1a", [NV, P, HCN], F32, kind="ExternalInput")
    b3a = nc.dram_tensor("b3a", [NV, P, HCN], F32, kind="ExternalInput")
    b2r = nc.dram_tensor("b2r", [1, NV, D], BF, kind="ExternalInput")
    gwt = nc.dram_tensor("gwt", [P, KD, E], F32, kind="ExternalInput")
    gb = nc.dram_tensor("gb", [1, E], F32, kind="ExternalInput")
    ones1 = nc.dram_tensor("ones1", [1, P], BF, kind="ExternalInput")
    y = nc.dram_tensor("y", [P, TT, D], F32, kind="ExternalOutput")

    with tile.TileContext(nc) as tc:
        with (
            tc.tile_pool(name="const1", bufs=1) as const1,
            tc.tile_pool(name="gchunk", bufs=2) as gchunk,
            tc.tile_pool(name="gtmp", bufs=4) as gtmp,
            tc.tile_pool(name="w1s", bufs=3) as w1s,
            tc.tile_pool(name="b13", bufs=2) as b13,
            tc.tile_pool(name="w2s", bufs=2) as w2s,
            tc.tile_pool(name="hpool", bufs=1) as hpool,
            tc.tile_pool(name="s1p", bufs=3) as s1p,
            tc.tile_pool(name="ps_l1", bufs=2, space="PSUM") as ps_l1,
            tc.tile_pool(name="ps_y", bufs=2, space="PSUM") as ps_y,
            tc.tile_pool(name="ps_g", bufs=2, space="PSUM") as ps_g,
        ):
            # ---- resident constants ----
            xTb = const1.tile([P, KD, T], BF)
            cw = const1.tile([P, TT, NV], F32)
            yacc = const1.tile([P, TT, D], F32)
            b2r_sb = const1.tile([1, NV, D], BF)
            ones_sb = const1.tile([1, P], BF)
            gwt_sb = const1.tile([P, KD, E], F32)
            gb_sb = const1.tile([1, E], F32)
            zerob = const1.tile([P, 1], F32)
            onesf = const1.tile([1, P], F32)

            nc.sync.dma_start(out=b2r_sb[:], in_=b2r[:])
            nc.sync.dma_start(out=ones_sb[:], in_=ones1[:])
            nc.sync.dma_start(out=gwt_sb[:], in_=gwt[:])
            nc.sync.dma_start(out=gb_sb[:], in_=gb[:])
            nc.vector.memset(zerob[:], 0.0)
            nc.vector.memset(onesf[:], 1.0)

            # ---- gate + bf16 cast of activations, per 128-token tile ----
            for m in range(TT):
                xchunk = gchunk.tile([P, KD, P], F32)
                nc.sync.dma_start(out=xchunk[:], in_=xT[:, :, m * P:(m + 1) * P])
                nc.vector.tensor_copy(xTb[:, :, m * P:(m + 1) * P], xchunk[:])

                pg = ps_g.tile([P, E], F32, space="PSUM")
                for k in range(KD):
                    nc.tensor.matmul(out=pg[:], lhsT=xchunk[:, k, :],
                                     rhs=gwt_sb[:, k, :],
                                     start=(k == 0), stop=False)
                # + gate bias via K=1 matmul with a ones row
                nc.tensor.matmul(out=pg[:], lhsT=onesf[:], rhs=gb_sb[:],
                                 start=False, stop=True)

                lg = gtmp.tile([P, E], F32)
                nc.scalar.activation(lg[:], pg[:], AF.Copy)
                m8 = gtmp.tile([P, 8], F32)
                nc.vector.max(m8[:], lg[:])
                # exp(l - max)
                ex = gtmp.tile([P, E], F32)
                nc.vector.tensor_scalar(out=ex[:], in0=lg[:],
                                        scalar1=m8[:, 0:1], scalar2=None,
                                        op0=OP.subtract)
                nc.scalar.activation(ex[:], ex[:], AF.Exp, bias=zerob[:])
                # top-2 mask
                mask = gtmp.tile([P, E], F32)
                nc.vector.tensor_scalar(out=mask[:], in0=lg[:],
                                        scalar1=m8[:, 1:2], scalar2=None,
                                        op0=OP.is_ge)
                # denom = 1 + exp(second - max);  cw = mask * ex / denom
                e2 = gtmp.tile([P, 1], F32)
                nc.vector.tensor_tensor(out=e2[:], in0=m8[:, 1:2], in1=m8[:, 0:1],
                                        op=OP.subtract)
                nc.scalar.activation(e2[:], e2[:], AF.Exp, bias=zerob[:])
                den = gtmp.tile([P, 1], F32)
                nc.vector.tensor_scalar(out=den[:], in0=e2[:], scalar1=1.0,
                                        scalar2=None, op0=OP.add)
                rec = gtmp.tile([P, 1], F32)
                nc.vector.reciprocal(rec[:], den[:])
                cwm = gtmp.tile([P, E], F32)
                nc.vector.tensor_mul(cwm[:], ex[:], mask[:])
                nc.vector.tensor_scalar(out=cw[:, m, 0:E], in0=cwm[:],
                                        scalar1=rec[:, 0:1], scalar2=None,
                                        op0=OP.mult)
                if NV > E:
                    nc.vector.memset(cw[:, m, E:NV], 1.0)

            # ---- virtual experts ----
            for e in range(NV):
                w2sb = w2s.tile([P, HCN, D], BF)
                nc.sync.dma_start(out=w2sb[:], in_=w2t[e])
                b1sb = b13.tile([P, HCN], F32)
                nc.sync.dma_start(out=b1sb[:], in_=b1a[e])
                b3sb = b13.tile([P, HCN], F32)
                nc.sync.dma_start(out=b3sb[:], in_=b3a[e])

                hT = hpool.tile([P, HCN, T], BF)

                # phase A: hT[h, t] = silu(W1 x + b1) * (W3 x + b3), feature-major
                for hc in range(HCN):
                    w1c = w1s.tile([P, KD, P], BF)
                    nc.sync.dma_start(out=w1c[:], in_=w1t[e, hc])
                    w3c = w1s.tile([P, KD, P], BF)
                    nc.sync.dma_start(out=w3c[:], in_=w3t[e, hc])
                    for ft in range(FT):
                        fsl = slice(ft * 512, min((ft + 1) * 512, T))
                        fw = fsl.stop - fsl.start
                        o1 = ps_l1.tile([P, 512], F32, space="PSUM", name="o1")
                        for k in range(KD):
                            nc.tensor.matmul(out=o1[:, :fw], lhsT=w1c[:, k, :],
                                             rhs=xTb[:, k, fsl],
                                             start=(k == 0), stop=(k == KD - 1))
                        # silu(v) = v * sigmoid(v), v = o1 + b1
                        s1 = s1p.tile([P, 512], F32)
                        nc.scalar.activation(s1[:, :fw], o1[:, :fw], AF.Sigmoid,
                                             bias=b1sb[:, hc:hc + 1])
                        t1 = s1p.tile([P, 512], F32)
                        nc.vector.scalar_tensor_tensor(
                            out=t1[:, :fw], in0=o1[:, :fw],
                            scalar=b1sb[:, hc:hc + 1], in1=s1[:, :fw],
                            op0=OP.add, op1=OP.mult)
                        o3 = ps_l1.tile([P, 512], F32, space="PSUM", name="o3")
                        for k in range(KD):
                            nc.tensor.matmul(out=o3[:, :fw], lhsT=w3c[:, k, :],
                                             rhs=xTb[:, k, fsl],
                                             start=(k == 0), stop=(k == KD - 1))
                        # h = (o3 + b3) * silu_out
                        nc.vector.scalar_tensor_tensor(
                            out=hT[:, hc, fsl], in0=o3[:, :fw],
                            scalar=b3sb[:, hc:hc + 1], in1=t1[:, :fw],
                            op0=OP.add, op1=OP.mult)

                # phase B: yacc[t, d] (+)= cw[t, e] * (hT^T @ W2^T + b2)
                for tt in range(TT):
                    tsl = slice(tt * P, (tt + 1) * P)
                    for dt in range(DT):
                        dsl = slice(dt * 512, min((dt + 1) * 512, D))
                        dw = dsl.stop - dsl.start
                        yp = ps_y.tile([P, 512], F32, space="PSUM", name="yp")
                        nc.tensor.matmul(out=yp[:, :dw], lhsT=ones_sb[:],
                                         rhs=b2r_sb[0:1, e, dsl],
                                         start=True, stop=False)
                        for hc in range(HCN):
                            nc.tensor.matmul(out=yp[:, :dw],
                                             lhsT=hT[:, hc, tsl],
                                             rhs=w2sb[:, hc, dsl],
                                             start=False, stop=(hc == HCN - 1))
                        if e == 0:
                            nc.vector.tensor_scalar(
                                out=yacc[:, tt, dsl], in0=yp[:, :dw],
                                scalar1=cw[:, tt, e:e + 1], scalar2=None,
                                op0=OP.mult)
                        else:
                            nc.vector.scalar_tensor_tensor(
                                out=yacc[:, tt, dsl], in0=yp[:, :dw],
                                scalar=cw[:, tt, e:e + 1],
                                in1=yacc[:, tt, dsl],
                                op0=OP.mult, op1=OP.add)

            nc.sync.dma_start(out=y[:], in_=yacc[:])

    nc.compile()
    return nc


def build_nc_dispatch(cfg: Cfg):
    """Dispatched (capacity-routed) variant, permutation-matmul dispatch.

    Token tiles are processed in pairs: per pair of 128-token tiles and
    routed expert e, a triangular-matmul prefix sum (plus a tiny
    count-broadcast matmul for the odd tile) assigns each routed token a
    slot in a capm-wide bucket. One-hot tiles Pe[t, j] = (slot[t] == j)
    then gather x feature-major via matmul (pad slots become zero
    columns). Each expert runs a dense SwiGLU over its CAPE slots and
    stores unscaled outputs (+b2) per slot in DRAM. The combine phase
    rebuilds Pe scaled by the renormalized gate weight, transposes it on
    the PE, and accumulates y_routed = sum_e Pe2w^T @ ye_bucket in PSUM;
    the shared expert (two H-half "virtual experts" over all tokens) is
    added on top. No indirect DMAs anywhere.
    """
    D, H, E, NV, T = cfg.D, cfg.H, cfg.E, cfg.NV, cfg.T
    KD, HCN, TT, DT, FT = cfg.KD, cfg.HCN, cfg.TT, cfg.DT, cfg.FT
    capm, CAPE, ST, NP = cfg.capm, cfg.CAPE, cfg.ST, cfg.NP

    nc = bacc.Bacc("TRN2", target_bir_lowering=False)

    xT = nc.dram_tensor("xT", [P, KD, T], F32, kind="ExternalInput")
    xtok = nc.dram_tensor("xtok", [P, TT, D], BF, kind="ExternalInput")
    xtb = nc.dram_tensor("xtb", [P, KD, T], BF, kind="ExternalInput")
    w1t = nc.dram_tensor("w1t", [NV, HCN, P, KD, P], BF, kind="ExternalInput")
    w3t = nc.dram_tensor("w3t", [NV, HCN, P, KD, P], BF, kind="ExternalInput")
    w2t = nc.dram_tensor("w2t", [NV, P, HCN, D], BF, kind="ExternalInput")
    b1a = nc.dram_tensor("b1a", [NV, P, HCN], F32, kind="ExternalInput")
    b3a = nc.dram_tensor("b3a", [NV, P, HCN], F32, kind="ExternalInput")
    b2r = nc.dram_tensor("b2r", [1, NV, D], BF, kind="ExternalInput")
    gwt = nc.dram_tensor("gwt", [P, KD, E], F32, kind="ExternalInput")
    gb = nc.dram_tensor("gb", [1, E], F32, kind="ExternalInput")
    ones1 = nc.dram_tensor("ones1", [1, P], BF, kind="ExternalInput")
    onesc = nc.dram_tensor("onesc", [P, 1], BF, kind="ExternalInput")
    lt = nc.dram_tensor("lt", [P, P], BF, kind="ExternalInput")
    ident = nc.dram_tensor("ident", [P, P], BF, kind="ExternalInput")
    iota = nc.dram_tensor("iota", [P, cfg.capm], F32, kind="ExternalInput")
    y = nc.dram_tensor("y", [P, TT, D], F32, kind="ExternalOutput")

    OOB = 3.0e6

    with tile.TileContext(nc) as tc:
        with (
            tc.tile_pool(name="const1", bufs=1) as const1,
            tc.tile_pool(name="gchunk", bufs=2) as gchunk,
            tc.tile_pool(name="gtmp", bufs=4) as gtmp,
            tc.tile_pool(name="w1s", bufs=3) as w1s,
            tc.tile_pool(name="b13", bufs=2) as b13,
            tc.tile_pool(name="w2s", bufs=1) as w2s,
            tc.tile_pool(name="hpool", bufs=1) as hpool,
            tc.tile_pool(name="s1p", bufs=2) as s1p,
            tc.tile_pool(name="yebp", bufs=5) as yebp,
            tc.tile_pool(name="xep", bufs=2) as xep,
            tc.tile_pool(name="pep", bufs=8) as pep,
            tc.tile_pool(name="comb", bufs=2) as comb,
            tc.tile_pool(name="dram", bufs=1, space="DRAM") as drp,
            tc.tile_pool(name="ps_l1", bufs=2, space="PSUM") as ps_l1,
            tc.tile_pool(name="ps_y", bufs=2, space="PSUM") as ps_y,
            tc.tile_pool(name="ps_sm", bufs=2, space="PSUM") as ps_sm,
        ):
            ye = drp.tile([E * CAPE, D], BF)   # per-slot expert outputs

            # ---- resident constants / state ----
            xTb = const1.tile([P, KD, T], BF)
            xtok_sb = const1.tile([P, TT, D], BF)
            yshared = const1.tile([P, TT, D], F32)
            cw = const1.tile([P, TT, E], F32)
            posb_all = const1.tile([P, TT, E], F32)
            ones_sb = const1.tile([1, P], BF)
            onesc_sb = const1.tile([P, 1], BF)
            gwt_sb = const1.tile([P, KD, E], F32)
            gb_sb = const1.tile([1, E], F32)
            zerob = const1.tile([P, 1], F32)
            onesf = const1.tile([1, P], F32)
            lt_sb = const1.tile([P, P], BF)
            id_sb = const1.tile([P, P], BF)
            iota_sb = const1.tile([P, capm], F32)

            nc.sync.dma_start(out=gwt_sb[:], in_=gwt[:])
            nc.sync.dma_start(out=gb_sb[:], in_=gb[:])
            nc.sync.dma_start(out=ones_sb[:], in_=ones1[:])
            nc.sync.dma_start(out=onesc_sb[:], in_=onesc[:])
            nc.sync.dma_start(out=lt_sb[:], in_=lt[:])
            nc.sync.dma_start(out=xTb[:], in_=xtb[:])
            nc.sync.dma_start(out=id_sb[:], in_=ident[:])
            nc.sync.dma_start(out=iota_sb[:], in_=iota[:])
            nc.vector.memset(zerob[:], 0.0)
            nc.vector.memset(onesf[:], 1.0)

            # prefetch the shared expert's first L1 weight chunks so its
            # matmuls can start while the gate phase runs
            pre_w = {}
            for hc in range(min(3, HCN)):
                w1c = w1s.tile([P, KD, P], BF, name="w1c", tag="w1c")
                nc.sync.dma_start(out=w1c[:], in_=w1t[E, hc])
                w3c = w1s.tile([P, KD, P], BF, name="w3c", tag="w3c")
                nc.sync.dma_start(out=w3c[:], in_=w3t[E, hc])
                pre_w[hc] = (w1c, w3c)
            nc.sync.dma_start(out=xtok_sb[:], in_=xtok[:])

            # ---- gate + routing, per token tile (paired buckets) ----
            cntb = None
            for m in range(TT):
                xchunk = gchunk.tile([P, KD, P], F32)
                nc.sync.dma_start(out=xchunk[:], in_=xT[:, :, m * P:(m + 1) * P])

                pg = ps_l1.tile([P, P], F32, space="PSUM", name="pg", tag="o1")
                for k in range(KD):
                    nc.tensor.matmul(out=pg[:, :E], lhsT=xchunk[:, k, :],
                                     rhs=gwt_sb[:, k, :],
                                     start=(k == 0), stop=False)
                nc.tensor.matmul(out=pg[:, :E], lhsT=onesf[:], rhs=gb_sb[:],
                                 start=False, stop=True)

                lg = gtmp.tile([P, E], F32)
                nc.scalar.activation(lg[:], pg[:, :E], AF.Copy)
                m8 = gtmp.tile([P, 8], F32)
                nc.vector.max(m8[:], lg[:])
                ex = gtmp.tile([P, E], F32)
                nc.vector.tensor_scalar(out=ex[:], in0=lg[:],
                                        scalar1=m8[:, 0:1], scalar2=None,
                                        op0=OP.subtract)
                nc.scalar.activation(ex[:], ex[:], AF.Exp, bias=zerob[:])
                mask = gtmp.tile([P, E], F32)
                nc.vector.tensor_scalar(out=mask[:], in0=lg[:],
                                        scalar1=m8[:, 1:2], scalar2=None,
                                        op0=OP.is_ge)
                e2 = gtmp.tile([P, 1], F32)
                nc.vector.tensor_tensor(out=e2[:], in0=m8[:, 1:2],
                                        in1=m8[:, 0:1], op=OP.subtract)
                nc.scalar.activation(e2[:], e2[:], AF.Exp, bias=zerob[:])
                den = gtmp.tile([P, 1], F32)
                nc.vector.tensor_scalar(out=den[:], in0=e2[:], scalar1=1.0,
                                        scalar2=None, op0=OP.add)
                rec = gtmp.tile([P, 1], F32)
                nc.vector.reciprocal(rec[:], den[:])
                cwm = gtmp.tile([P, E], F32)
                nc.vector.tensor_mul(cwm[:], ex[:], mask[:])
                nc.vector.tensor_scalar(out=cw[:, m, :], in0=cwm[:],
                                        scalar1=rec[:, 0:1], scalar2=None,
                                        op0=OP.mult)

                # bucket-local slot: pair prefix(mask) - mask; OOB unrouted
                maskb = gtmp.tile([P, E], BF)
                nc.vector.tensor_copy(maskb[:], mask[:])
                pp = ps_y.tile([P, P], F32, space="PSUM", name="pp", tag="yp")
                if m % 2 == 0:
                    nc.tensor.matmul(out=pp[:, :E], lhsT=lt_sb[:],
                                     rhs=maskb[:], start=True, stop=True)
                    # bucket count of the even tile, for the odd tile
                    cnt_ps = ps_sm.tile([1, P], F32, space="PSUM",
                                        name="cntp", tag="sm")
                    nc.tensor.matmul(out=cnt_ps[0:1, :E], lhsT=onesc_sb[:],
                                     rhs=maskb[:], start=True, stop=True)
                    cntb = gtmp.tile([1, E], BF, name="cntb")
                    nc.scalar.activation(cntb[:], cnt_ps[0:1, :E], AF.Copy)
                else:
                    nc.tensor.matmul(out=pp[:, :E], lhsT=lt_sb[:],
                                     rhs=maskb[:], start=True, stop=False)
                    nc.tensor.matmul(out=pp[:, :E], lhsT=ones_sb[:],
                                     rhs=cntb[:], start=False, stop=True)
                t1m = gtmp.tile([P, E], F32)
                nc.vector.scalar_tensor_tensor(out=t1m[:], in0=mask[:],
                                               scalar=-1.0, in1=pp[:, :E],
                                               op0=OP.mult, op1=OP.add)
                notm = gtmp.tile([P, E], F32)
                nc.vector.tensor_scalar(out=notm[:], in0=mask[:],
                                        scalar1=-1.0, scalar2=1.0,
                                        op0=OP.mult, op1=OP.add)
                nc.vector.scalar_tensor_tensor(out=posb_all[:, m, :],
                                               in0=notm[:], scalar=OOB,
                                               in1=t1m[:],
                                               op0=OP.mult, op1=OP.add)

            # ---- shared expert first (dense over all tokens) ----
            for sv in range(cfg.n_sh):
                e = E + sv
                w2sb = w2s.tile([P, HCN, D], BF)
                b1sb = b13.tile([P, HCN], F32)
                nc.sync.dma_start(out=b1sb[:], in_=b1a[e])
                b3sb = b13.tile([P, HCN], F32)
                nc.sync.dma_start(out=b3sb[:], in_=b3a[e])
                b2e = b13.tile([1, D], BF)
                nc.sync.dma_start(out=b2e[:], in_=b2r[0:1, e, :])

                hT = hpool.tile([P, HCN, T], BF, name="hT", tag="hT")
                for hc in range(HCN):
                    if sv == 0 and hc in pre_w:
                        w1c, w3c = pre_w[hc]
                    else:
                        w1c = w1s.tile([P, KD, P], BF, name="w1c", tag="w1c")
                        nc.sync.dma_start(out=w1c[:], in_=w1t[e, hc])
                        w3c = w1s.tile([P, KD, P], BF, name="w3c", tag="w3c")
                        nc.sync.dma_start(out=w3c[:], in_=w3t[e, hc])
                    for ft in range(FT):
                        fsl = slice(ft * 512, min((ft + 1) * 512, T))
                        fw = fsl.stop - fsl.start
                        o1 = ps_l1.tile([P, 512], F32, space="PSUM", name="o1")
                        for k in range(KD):
                            nc.tensor.matmul(out=o1[:, :fw], lhsT=w1c[:, k, :],
                                             rhs=xTb[:, k, fsl],
                                             start=(k == 0), stop=(k == KD - 1))
                        s1 = s1p.tile([P, 512], F32)
                        nc.scalar.activation(s1[:, :fw], o1[:, :fw], AF.Sigmoid,
                                             bias=b1sb[:, hc:hc + 1])
                        t1 = s1p.tile([P, 512], F32)
                        nc.vector.scalar_tensor_tensor(
                            out=t1[:, :fw], in0=o1[:, :fw],
                            scalar=b1sb[:, hc:hc + 1], in1=s1[:, :fw],
                            op0=OP.add, op1=OP.mult)
                        o3 = ps_l1.tile([P, 512], F32, space="PSUM", name="o3")
                        for k in range(KD):
                            nc.tensor.matmul(out=o3[:, :fw], lhsT=w3c[:, k, :],
                                             rhs=xTb[:, k, fsl],
                                             start=(k == 0), stop=(k == KD - 1))
                        nc.vector.scalar_tensor_tensor(
                            out=hT[:, hc, fsl], in0=o3[:, :fw],
                            scalar=b3sb[:, hc:hc + 1], in1=t1[:, :fw],
                            op0=OP.add, op1=OP.mult)

                nc.sync.dma_start(out=w2sb[:], in_=w2t[e])
                for tt in range(TT):
                    tsl = slice(tt * P, (tt + 1) * P)
                    for dt in range(DT):
                        dsl = slice(dt * 512, min((dt + 1) * 512, D))
                        dw = dsl.stop - dsl.start
                        yp = ps_y.tile([P, 512], F32, space="PSUM", name="yp")
                        nc.tensor.matmul(out=yp[:, :dw], lhsT=ones_sb[:],
                                         rhs=b2e[0:1, dsl],
                                         start=True, stop=False)
                        for hc in range(HCN):
                            nc.tensor.matmul(out=yp[:, :dw],
                                             lhsT=hT[:, hc, tsl],
                                             rhs=w2sb[:, hc, dsl],
                                             start=False, stop=(hc == HCN - 1))
                        if sv == 0:
                            nc.vector.tensor_copy(yshared[:, tt, dsl],
                                                  yp[:, :dw])
                        else:
                            nc.vector.tensor_add(yshared[:, tt, dsl],
                                                 yshared[:, tt, dsl],
                                                 yp[:, :dw])

            # ---- routed experts over dispatched slots ----
            for e in range(E):
                w2sb = w2s.tile([P, HCN, D], BF)
                b1sb = b13.tile([P, HCN], F32)
                nc.sync.dma_start(out=b1sb[:], in_=b1a[e])
                b3sb = b13.tile([P, HCN], F32)
                nc.sync.dma_start(out=b3sb[:], in_=b3a[e])
                b2e = b13.tile([1, D], BF)
                nc.sync.dma_start(out=b2e[:], in_=b2r[0:1, e, :])

                # matmul gather: xeT_k[:, pr, :] = sum_pair x_m^T @ Pe_m.
                # Per-k tiles so layer 1's k-th accumulation step only
                # depends on gather step k (gather pipelines under L1).
                pes = []
                for m in range(TT):
                    pe = pep.tile([P, capm], BF, name="pe", tag="pe")
                    nc.vector.tensor_scalar(
                        out=pe[:], in0=iota_sb[:],
                        scalar1=posb_all[:, m, e:e + 1],
                        scalar2=None, op0=OP.is_equal)
                    pes.append(pe)
                xeT_k = []
                for k in range(KD):
                    xk = xep.tile([P, NP, capm], BF, name=f"xeT{k}",
                                  tag=f"xeT{k}")
                    for pp0 in range(0, NP, 2):
                        npp = min(2, NP - pp0)
                        gx = ps_sm.tile([P, 2 * capm], F32, space="PSUM",
                                        name="gx", tag="sm")
                        for pi, m2 in [(a, b) for a in range(npp)
                                       for b in range(2)]:
                            pr = pp0 + pi
                            csl = slice(pi * capm, (pi + 1) * capm)
                            m = 2 * pr + m2
                            nc.tensor.matmul(
                                out=gx[:, csl],
                                lhsT=xtok_sb[:, m, k * P:(k + 1) * P],
                                rhs=pes[m][:], start=(m2 == 0),
                                stop=(m2 == 1))
                        nc.scalar.activation(
                            xk[:, pp0:pp0 + npp, :],
                            gx[:, :npp * capm], AF.Copy)
                    xeT_k.append(xk)

                hT = hpool.tile([P, HCN, T], BF, name="hT", tag="hT")
                for hc in range(HCN):
                    w1c = w1s.tile([P, KD, P], BF, name="w1c", tag="w1c")
                    nc.sync.dma_start(out=w1c[:], in_=w1t[e, hc])
                    w3c = w1s.tile([P, KD, P], BF, name="w3c", tag="w3c")
                    nc.sync.dma_start(out=w3c[:], in_=w3t[e, hc])
                    o1 = ps_l1.tile([P, 512], F32, space="PSUM", name="o1")
                    for k in range(KD):
                        nc.tensor.matmul(out=o1[:, :CAPE], lhsT=w1c[:, k, :],
                                         rhs=xeT_k[k][:, :, :],
                                         start=(k == 0), stop=(k == KD - 1))
                    s1 = s1p.tile([P, 512], F32)
                    nc.scalar.activation(s1[:, :CAPE], o1[:, :CAPE], AF.Sigmoid,
                                         bias=b1sb[:, hc:hc + 1])
                    t1 = s1p.tile([P, 512], F32)
                    nc.vector.scalar_tensor_tensor(
                        out=t1[:, :CAPE], in0=o1[:, :CAPE],
                        scalar=b1sb[:, hc:hc + 1], in1=s1[:, :CAPE],
                        op0=OP.add, op1=OP.mult)
                    o3 = ps_l1.tile([P, 512], F32, space="PSUM", name="o3")
                    for k in range(KD):
                        nc.tensor.matmul(out=o3[:, :CAPE], lhsT=w3c[:, k, :],
                                         rhs=xeT_k[k][:, :, :],
                                         start=(k == 0), stop=(k == KD - 1))
                    nc.vector.scalar_tensor_tensor(
                        out=hT[:, hc, :CAPE], in0=o3[:, :CAPE],
                        scalar=b3sb[:, hc:hc + 1], in1=t1[:, :CAPE],
                        op0=OP.add, op1=OP.mult)

                nc.sync.dma_start(out=w2sb[:], in_=w2t[e])
                for st in range(ST):
                    sw = min(P, CAPE - st * P)
                    ssl = slice(st * P, st * P + sw)
                    for dt in range(DT):
                        dsl = slice(dt * 512, min((dt + 1) * 512, D))
                        dw = dsl.stop - dsl.start
                        yp = ps_y.tile([P, 512], F32, space="PSUM", name="yp")
                        nc.tensor.matmul(out=yp[:sw, :dw], lhsT=ones_sb[:, :sw],
                                         rhs=b2e[0:1, dsl],
                                         start=True, stop=False)
                        for hc in range(HCN):
                            nc.tensor.matmul(out=yp[:sw, :dw],
                                             lhsT=hT[:, hc, ssl],
                                             rhs=w2sb[:, hc, dsl],
                                             start=False, stop=(hc == HCN - 1))
                        yeb = comb.tile([P, 512], BF, name="yeb")
                        nc.scalar.activation(yeb[:sw, :dw], yp[:sw, :dw],
                                             AF.Copy)
                        nc.sync.dma_start(
                            out=ye[e * CAPE + st * P: e * CAPE + st * P + sw,
                                   dsl],
                            in_=yeb[:sw, :dw])

            # ---- combine: y = yshared + sum_e cw_e * ye[slot] ----
            for m in range(TT):
                pr = m // 2
                yps = []
                pool_c, tag_c = (ps_y, "yp") if m % 2 == 0 else (ps_l1, "o1")
                for dt in range(DT):
                    yps.append(pool_c.tile([P, 512], F32, space="PSUM",
                                           name=f"ypc{dt}", tag=tag_c))
                for e in range(E):
                    yeb_sb = yebp.tile([capm, D], BF)
                    nc.sync.dma_start(
                        out=yeb_sb[:],
                        in_=ye[e * CAPE + pr * capm:
                               e * CAPE + (pr + 1) * capm, :])
                    pe = gtmp.tile([P, capm], BF, name="pe")
                    nc.vector.tensor_scalar(out=pe[:], in0=iota_sb[:],
                                            scalar1=posb_all[:, m, e:e + 1],
                                            scalar2=None, op0=OP.is_equal)
                    pew = gtmp.tile([P, capm], BF, name="pew")
                    nc.vector.tensor_scalar(out=pew[:], in0=pe[:],
                                            scalar1=cw[:, m, e:e + 1],
                                            scalar2=None, op0=OP.mult)
                    pool_t, tag_t = (ps_sm, "sm") if e % 2 == 0 else (ps_l1, "o3")
                    p2 = pool_t.tile([P, P], BF, space="PSUM",
                                     name="p2", tag=tag_t)
                    nc.tensor.transpose(out=p2[:capm, :], in_=pew[:],
                                        identity=id_sb[:])
                    p2s = gtmp.tile([capm, P], BF, name="p2s")
                    nc.scalar.activation(p2s[:], p2[:capm, :], AF.Copy)
                    for dt in range(DT):
                        dsl = slice(dt * 512, min((dt + 1) * 512, D))
                        dw = dsl.stop - dsl.start
                        nc.tensor.matmul(out=yps[dt][:, :dw], lhsT=p2s[:],
                                         rhs=yeb_sb[:, dsl],
                                         start=(e == 0), stop=(e == E - 1))
                for dt in range(DT):
                    dsl = slice(dt * 512, min((dt + 1) * 512, D))
                    dw = dsl.stop - dsl.start
                    yt = comb.tile([P, 512], F32, name="yt")
                    nc.vector.tensor_add(yt[:, :dw], yshared[:, m, dsl],
                                         yps[dt][:, :dw])
                    nc.sync.dma_start(out=y[:, m, dsl], in_=yt[:, :dw])

    nc.compile()
    return nc


# ---------------- host-side packing ----------------

def pack_static(cfg: Cfg, gate_w, gate_b, w1, b1, w2, b2, w3, b3,
                sw1, sb1, sw2, sb2, sw3, sb3):
    D, H, E, NV, n_sh = cfg.D, cfg.H, cfg.E, cfg.NV, cfg.n_sh
    KD, HCN = cfg.KD, cfg.HCN

    w1T = np.transpose(w1, (0, 2, 1))                      # [E, D, H]
    w3T = np.transpose(w3, (0, 2, 1))
    w2T = np.transpose(w2, (0, 2, 1))                      # [E, H, D]
    s1T = sw1.T.reshape(D, n_sh, H).transpose(1, 0, 2)     # [n_sh, D, H]
    s3T = sw3.T.reshape(D, n_sh, H).transpose(1, 0, 2)
    s2T = sw2.T.reshape(n_sh, H, D)                        # [n_sh, H, D]
    w1T_all = np.concatenate([w1T, s1T], 0)                # [NV, D, H]
    w3T_all = np.concatenate([w3T, s3T], 0)
    w2T_all = np.concatenate([w2T, s2T], 0)                # [NV, H, D]

    w1t = np.ascontiguousarray(
        w1T_all.reshape(NV, KD, P, HCN, P).transpose(0, 3, 2, 1, 4)).astype(BF16)
    w3t = np.ascontiguousarray(
        w3T_all.reshape(NV, KD, P, HCN, P).transpose(0, 3, 2, 1, 4)).astype(BF16)
    w2t = np.ascontiguousarray(
        w2T_all.reshape(NV, HCN, P, D).transpose(0, 2, 1, 3)).astype(BF16)

    b1_all = np.concatenate([b1, sb1.reshape(n_sh, H)], 0)  # [NV, H]
    b3_all = np.concatenate([b3, sb3.reshape(n_sh, H)], 0)
    b1a = np.ascontiguousarray(
        b1_all.reshape(NV, HCN, P).transpose(0, 2, 1)).astype(np.float32)
    b3a = np.ascontiguousarray(
        b3_all.reshape(NV, HCN, P).transpose(0, 2, 1)).astype(np.float32)

    b2_all = np.concatenate(
        [b2, sb2[None], np.zeros((n_sh - 1, D), np.float32)], 0)  # [NV, D]
    b2r = b2_all[None].astype(BF16)                         # [1, NV, D]

    gwt = np.ascontiguousarray(
        gate_w.T.reshape(KD, P, E).transpose(1, 0, 2)).astype(np.float32)
    gb = gate_b[None].astype(np.float32)
    ones1 = np.ones((1, P), BF16)

    return dict(w1t=w1t, w3t=w3t, w2t=w2t, b1a=b1a, b3a=b3a, b2r=b2r,
                gwt=gwt, gb=gb, ones1=ones1)


def pack_dispatch_consts(cfg: Cfg):
    lt = np.triu(np.ones((P, P))).astype(BF16)          # lt[k, j] = k <= j
    ident = np.eye(P).astype(BF16)
    iota = np.tile(np.arange(cfg.capm, dtype=np.float32), (P, 1))
    onesc = np.ones((P, 1), BF16)
    return dict(lt=lt, ident=ident, iota=iota, onesc=onesc)


def pack_xtok(cfg: Cfg, x_tokens):
    """x_tokens [T, D] fp32 -> token-major [P, TT, D] bf16."""
    T, D = x_tokens.shape
    xt = x_tokens.reshape(cfg.TT, P, D).transpose(1, 0, 2)
    return np.ascontiguousarray(xt).astype(BF16)


def pack_xT(cfg: Cfg, x_tokens):
    """x_tokens [T, D] fp32 -> xT device layout [P, KD, T]."""
    T, D = x_tokens.shape
    xT = x_tokens.T.reshape(cfg.KD, P, T).transpose(1, 0, 2)
    return np.ascontiguousarray(xT).astype(np.float32)


def unpack_y(cfg: Cfg, y_dev):
    """y device layout [P, TT, D] -> [T, D]."""
    return np.ascontiguousarray(y_dev.transpose(1, 0, 2).reshape(cfg.T, cfg.D))


_CACHE = {}
DISPATCH = True


def _get_nc(cfg: Cfg, dispatch=None):
    if dispatch is None:
        dispatch = DISPATCH
    key = (cfg.D, cfg.H, cfg.E, cfg.n_sh, cfg.T, cfg.capm, dispatch)
    if key not in _CACHE:
        _CACHE[key] = (build_nc_dispatch(cfg) if dispatch else build_nc(cfg))
    return _CACHE[key]


def make_in_maps(cfg: Cfg, inputs, dispatch=None):
    static = pack_static(
        cfg,
        np.asarray(inputs["gate_w"], np.float32), np.asarray(inputs["gate_b"], np.float32),
        np.asarray(inputs["w1"], np.float32), np.asarray(inputs["b1"], np.float32),
        np.asarray(inputs["w2"], np.float32), np.asarray(inputs["b2"], np.float32),
        np.asarray(inputs["w3"], np.float32), np.asarray(inputs["b3"], np.float32),
        np.asarray(inputs["sw1"], np.float32), np.asarray(inputs["sb1"], np.float32),
        np.asarray(inputs["sw2"], np.float32), np.asarray(inputs["sb2"], np.float32),
        np.asarray(inputs["sw3"], np.float32), np.asarray(inputs["sb3"], np.float32),
    )
    if dispatch is None:
        dispatch = DISPATCH
    if dispatch:
        static.update(pack_dispatch_consts(cfg))
    x = np.asarray(inputs["x"], np.float32)
    B, S, D = x.shape
    xf = x.reshape(-1, D)
    in_maps = []
    for c in range(cfg.n_cores):
        m = dict(static)
        xc = xf[c * cfg.T:(c + 1) * cfg.T]
        m["xT"] = pack_xT(cfg, xc)
        if dispatch:
            m["xtok"] = pack_xtok(cfg, xc)
            m["xtb"] = m["xT"].astype(BF16)
        in_maps.append(m)
    return in_maps


def kernel(**inputs) -> np.ndarray:
    x = np.asarray(inputs["x"], np.float32)
    B, S, D = x.shape
    N = B * S
    cfg = Cfg(D=D, T=N // 8, n_cores=8)
    nc = _get_nc(cfg)
    in_maps = make_in_maps(cfg, inputs)
    res = run_bass_kernel_spmd(nc, in_maps, list(range(cfg.n_cores)))
    outs = [unpack_y(cfg, res.results[c]["y"]) for c in range(cfg.n_cores)]
    return np.concatenate(outs, 0).reshape(B, S, D)



# revision 4
# speedup vs baseline: 1.1315x; 1.1315x over previous
"""MoE (8 routed experts, top-2, + shared expert) on 8 NeuronCores.

Data-parallel over tokens (1024/core), weights replicated, capacity-routed
dispatch (pair buckets, capm=96) as in the bf16 baseline — but all large
GEMMs run as fp8e4m3 DoubleRow matmuls with 3-term error compensation:

    A @ B  ~=  A_hi@B_hi + A_hi@B_lo + A_lo@B_hi

where X_hi = fp8(X), X_lo = fp8(X - X_hi).  DoubleRow consumes two
128-deep K-tiles per instruction at 0.5 cycles/row, so each compensated
GEMM costs 0.75x its bf16 schedule while adding only ~1e-3 relative
error.  Weights are pre-scaled by 64 on the host (fp8 subnormal cutoff),
descaled in the PSUM eviction.  The gate runs in fp32 so routing matches
the reference.  Half of the shared expert runs after the combine phase to
hide the combine's DRAM round-trip latency in the schedule tail.
"""

import numpy as np
import ml_dtypes

import concourse.bacc as bacc
import concourse.bass as bass
import concourse.tile as tile
import concourse.mybir as mybir
from concourse.bass_utils import run_bass_kernel_spmd

BF16 = ml_dtypes.bfloat16
FP8 = ml_dtypes.float8_e4m3
F32 = mybir.dt.float32
BF = mybir.dt.bfloat16
F8 = mybir.dt.float8e4
AF = mybir.ActivationFunctionType
OP = mybir.AluOpType
DR = mybir.MatmulPerfMode.DoubleRow

P = 128
WS = 64.0          # host-side weight scale before fp8 quantization
WSI = 1.0 / WS
WSI2 = 1.0 / (WS * WS)


class Cfg:
    def __init__(self, D=1024, H=2048, E=8, n_sh=2, T=1024, n_cores=8, capm=96):
        self.D, self.H, self.E, self.n_sh, self.T = D, H, E, n_sh, T
        self.NV = E + n_sh          # virtual experts
        self.HS = n_sh * H          # shared hidden
        self.KD = D // P            # K chunks over D
        self.HCN = H // P           # h chunks over H
        self.TT = T // P            # token 128-tiles
        self.DT = (D + 511) // 512  # output d 512-tiles
        self.FT = (T + 511) // 512  # layer-1 free 512-tiles
        self.n_cores = n_cores
        self.capm = capm            # per-(expert, tile-pair) dispatch capacity
        self.NP = self.TT // 2      # token-tile pairs
        self.CAPE = self.NP * capm  # slots per expert
        self.ST = (self.CAPE + P - 1) // P  # slot 128-tiles per expert


def build_nc_fp8(cfg: Cfg):
    D, H, E, NV, T = cfg.D, cfg.H, cfg.E, cfg.NV, cfg.T
    KD, HCN, TT, DT, FT = cfg.KD, cfg.HCN, cfg.TT, cfg.DT, cfg.FT
    capm, CAPE, ST, NP = cfg.capm, cfg.CAPE, cfg.ST, cfg.NP
    KD2, HCN2 = KD // 2, HCN // 2

    nc = bacc.Bacc("TRN2", target_bir_lowering=False)

    xT = nc.dram_tensor("xT", [P, KD, T], F32, kind="ExternalInput")
    xtbh = nc.dram_tensor("xtbh", [P, KD, T], F8, kind="ExternalInput")
    xtbl = nc.dram_tensor("xtbl", [P, KD, T], F8, kind="ExternalInput")
    xtokh = nc.dram_tensor("xtokh", [P, TT, D], F8, kind="ExternalInput")
    xtokl = nc.dram_tensor("xtokl", [P, TT, D], F8, kind="ExternalInput")
    w1h = nc.dram_tensor("w1h", [NV, HCN, P, KD, P], F8, kind="ExternalInput")
    w1l = nc.dram_tensor("w1l", [NV, HCN, P, KD, P], F8, kind="ExternalInput")
    w3h = nc.dram_tensor("w3h", [NV, HCN, P, KD, P], F8, kind="ExternalInput")
    w3l = nc.dram_tensor("w3l", [NV, HCN, P, KD, P], F8, kind="ExternalInput")
    w2h = nc.dram_tensor("w2h", [NV, P, HCN, D], F8, kind="ExternalInput")
    w2l = nc.dram_tensor("w2l", [NV, P, HCN, D], F8, kind="ExternalInput")
    b1a = nc.dram_tensor("b1a", [NV, P, HCN], F32, kind="ExternalInput")
    b1s = nc.dram_tensor("b1s", [NV, P, HCN], F32, kind="ExternalInput")
    b3s = nc.dram_tensor("b3s", [NV, P, HCN], F32, kind="ExternalInput")
    b2e8 = nc.dram_tensor("b2e8", [E, D], BF, kind="ExternalInput")
    sb2x = nc.dram_tensor("sb2x", [1, D], BF, kind="ExternalInput")
    gwt = nc.dram_tensor("gwt", [P, KD, E], F32, kind="ExternalInput")
    gb = nc.dram_tensor("gb", [1, E], F32, kind="ExternalInput")
    ones1 = nc.dram_tensor("ones1", [1, P], BF, kind="ExternalInput")
    onesc = nc.dram_tensor("onesc", [P, 1], BF, kind="ExternalInput")
    lt = nc.dram_tensor("lt", [P, P], BF, kind="ExternalInput")
    ident = nc.dram_tensor("ident", [P, P], BF, kind="ExternalInput")
    iota = nc.dram_tensor("iota", [P, capm], F32, kind="ExternalInput")
    y = nc.dram_tensor("y", [P, TT, D], F32, kind="ExternalOutput")

    OOB = 3.0e6

    with tile.TileContext(nc) as tc:
        with (
            tc.tile_pool(name="const1", bufs=1) as const1,
            tc.tile_pool(name="gchunk", bufs=2) as gchunk,
            tc.tile_pool(name="gtmp", bufs=4) as gtmp,
            tc.tile_pool(name="w1s", bufs=3) as w1s,
            tc.tile_pool(name="b13", bufs=2) as b13,
            tc.tile_pool(name="w2s", bufs=1) as w2s,
            tc.tile_pool(name="hpool", bufs=1) as hpool,
            tc.tile_pool(name="s1p", bufs=3) as s1p,
            tc.tile_pool(name="yebp", bufs=5) as yebp,
            tc.tile_pool(name="xep", bufs=2) as xep,
            tc.tile_pool(name="pep", bufs=8) as pep,
            tc.tile_pool(name="comb", bufs=2) as comb,
            tc.tile_pool(name="dram", bufs=1, space="DRAM") as drp,
            tc.tile_pool(name="ps_l1", bufs=2, space="PSUM") as ps_l1,
            tc.tile_pool(name="ps_y", bufs=2, space="PSUM") as ps_y,
            tc.tile_pool(name="ps_sm", bufs=2, space="PSUM") as ps_sm,
        ):
            ye = drp.tile([E * CAPE, D], BF)   # per-slot expert outputs

            # ---- resident constants / state ----
            xbh = const1.tile([P, KD, T], F8)
            xbl = const1.tile([P, KD, T], F8)
            xth = const1.tile([P, TT, D], F8)
            xtl = const1.tile([P, TT, D], F8)
            yshared = const1.tile([P, TT, D], BF)
            cw = const1.tile([P, TT, E], F32)
            cwT = const1.tile([8, TT, P], BF)
            posb_all = const1.tile([P, TT, E], F32)
            ones_sb = const1.tile([1, P], BF)
            onesc_sb = const1.tile([P, 1], BF)
            gwt_sb = const1.tile([P, KD, E], F32)
            gb_sb = const1.tile([1, E], F32)
            zerob = const1.tile([P, 1], F32)
            onesf = const1.tile([1, P], F32)
            lt_sb = const1.tile([P, P], BF)
            id_sb = const1.tile([P, P], BF)
            iota_sb = const1.tile([P, capm], F32)
            b2e8_sb = const1.tile([8, D], BF)
            sb2x_sb = const1.tile([1, D], BF)

            nc.sync.dma_start(out=gwt_sb[:], in_=gwt[:])
            nc.sync.dma_start(out=gb_sb[:], in_=gb[:])
            nc.sync.dma_start(out=ones_sb[:], in_=ones1[:])
            nc.sync.dma_start(out=onesc_sb[:], in_=onesc[:])
            nc.sync.dma_start(out=lt_sb[:], in_=lt[:])
            nc.sync.dma_start(out=id_sb[:], in_=ident[:])
            nc.sync.dma_start(out=iota_sb[:], in_=iota[:])
            nc.sync.dma_start(out=b2e8_sb[:], in_=b2e8[:])
            nc.sync.dma_start(out=sb2x_sb[:], in_=sb2x[:])
            nc.sync.dma_start(out=xbh[:], in_=xtbh[:])
            nc.sync.dma_start(out=xbl[:], in_=xtbl[:])
            nc.vector.memset(zerob[:], 0.0)
            nc.vector.memset(onesf[:], 1.0)

            # prefetch the first shared-half L1 weight chunks so its matmuls
            # can start while the gate phase runs
            pre_w = {}
            for hc in range(min(3, HCN)):
                ws_t = []
                for nm, src in (("w1ch", w1h), ("w1cl", w1l),
                                ("w3ch", w3h), ("w3cl", w3l)):
                    t = w1s.tile([P, KD, P], F8, name=nm, tag=nm)
                    nc.sync.dma_start(out=t[:], in_=src[E, hc])
                    ws_t.append(t)
                pre_w[hc] = ws_t
            nc.sync.dma_start(out=xth[:], in_=xtokh[:])
            nc.sync.dma_start(out=xtl[:], in_=xtokl[:])

            # ---- gate + routing, per token tile (paired buckets) ----
            cntb = None
            for m in range(TT):
                xchunk = gchunk.tile([P, KD, P], F32)
                nc.sync.dma_start(out=xchunk[:], in_=xT[:, :, m * P:(m + 1) * P])

                pg = ps_l1.tile([P, P], F32, space="PSUM", name="pg", tag="o1")
                for k in range(KD):
                    nc.tensor.matmul(out=pg[:, :E], lhsT=xchunk[:, k, :],
                                     rhs=gwt_sb[:, k, :],
                                     start=(k == 0), stop=False)
                nc.tensor.matmul(out=pg[:, :E], lhsT=onesf[:], rhs=gb_sb[:],
                                 start=False, stop=True)

                lg = gtmp.tile([P, E], F32)
                nc.scalar.activation(lg[:], pg[:, :E], AF.Copy)
                m8 = gtmp.tile([P, 8], F32)
                nc.vector.max(m8[:], lg[:])
                ex = gtmp.tile([P, E], F32)
                nc.vector.tensor_scalar(out=ex[:], in0=lg[:],
                                        scalar1=m8[:, 0:1], scalar2=None,
                                        op0=OP.subtract)
                nc.scalar.activation(ex[:], ex[:], AF.Exp, bias=zerob[:])
                mask = gtmp.tile([P, E], F32)
                nc.vector.tensor_scalar(out=mask[:], in0=lg[:],
                                        scalar1=m8[:, 1:2], scalar2=None,
                                        op0=OP.is_ge)
                e2 = gtmp.tile([P, 1], F32)
                nc.vector.tensor_tensor(out=e2[:], in0=m8[:, 1:2],
                                        in1=m8[:, 0:1], op=OP.subtract)
                nc.scalar.activation(e2[:], e2[:], AF.Exp, bias=zerob[:])
                den = gtmp.tile([P, 1], F32)
                nc.vector.tensor_scalar(out=den[:], in0=e2[:], scalar1=1.0,
                                        scalar2=None, op0=OP.add)
                rec = gtmp.tile([P, 1], F32)
                nc.vector.reciprocal(rec[:], den[:])
                cwm = gtmp.tile([P, E], F32)
                nc.vector.tensor_mul(cwm[:], ex[:], mask[:])
                nc.vector.tensor_scalar(out=cw[:, m, :], in0=cwm[:],
                                        scalar1=rec[:, 0:1], scalar2=None,
                                        op0=OP.mult)
                # bf16 transpose of the combine weights for the b2 matmul
                cwb = gtmp.tile([P, E], BF, name="cwb")
                nc.vector.tensor_copy(cwb[:], cw[:, m, :])
                ptp = ps_sm.tile([P, P], BF, space="PSUM", name="ptp", tag="sm")
                nc.tensor.transpose(out=ptp[:E, :], in_=cwb[:], identity=id_sb[:])
                nc.scalar.activation(cwT[:, m, :], ptp[:E, :], AF.Copy)

                # bucket-local slot: pair prefix(mask) - mask; OOB unrouted
                maskb = gtmp.tile([P, E], BF)
                nc.vector.tensor_copy(maskb[:], mask[:])
                pp = ps_y.tile([P, P], F32, space="PSUM", name="pp", tag="yp")
                if m % 2 == 0:
                    nc.tensor.matmul(out=pp[:, :E], lhsT=lt_sb[:],
                                     rhs=maskb[:], start=True, stop=True)
                    cnt_ps = ps_sm.tile([1, P], F32, space="PSUM",
                                        name="cntp", tag="sm")
                    nc.tensor.matmul(out=cnt_ps[0:1, :E], lhsT=onesc_sb[:],
                                     rhs=maskb[:], start=True, stop=True)
                    cntb = gtmp.tile([1, E], BF, name="cntb")
                    nc.scalar.activation(cntb[:], cnt_ps[0:1, :E], AF.Copy)
                else:
                    nc.tensor.matmul(out=pp[:, :E], lhsT=lt_sb[:],
                                     rhs=maskb[:], start=True, stop=False)
                    nc.tensor.matmul(out=pp[:, :E], lhsT=ones_sb[:],
                                     rhs=cntb[:], start=False, stop=True)
                t1m = gtmp.tile([P, E], F32)
                nc.vector.scalar_tensor_tensor(out=t1m[:], in0=mask[:],
                                               scalar=-1.0, in1=pp[:, :E],
                                               op0=OP.mult, op1=OP.add)
                notm = gtmp.tile([P, E], F32)
                nc.vector.tensor_scalar(out=notm[:], in0=mask[:],
                                        scalar1=-1.0, scalar2=1.0,
                                        op0=OP.mult, op1=OP.add)
                nc.vector.scalar_tensor_tensor(out=posb_all[:, m, :],
                                               in0=notm[:], scalar=OOB,
                                               in1=t1m[:],
                                               op0=OP.mult, op1=OP.add)

            def l1_evict(o1, o3, b1c, b1sc, b3sc, hh, hl, hc, fsl, fw):
                """h = silu(o1/WS + b1) * (o3/WS + b3) -> fp8 hi/lo pair."""
                s1 = s1p.tile([P, 512], F32, name="s1")
                nc.scalar.activation(s1[:, :fw], o1[:, :fw], AF.Sigmoid,
                                     bias=b1c, scale=WSI)
                t1 = s1p.tile([P, 512], F32, name="t1")
                nc.vector.scalar_tensor_tensor(
                    out=t1[:, :fw], in0=o1[:, :fw], scalar=b1sc,
                    in1=s1[:, :fw], op0=OP.add, op1=OP.mult)
                tmp = s1p.tile([P, 512], F32, name="tmp")
                nc.vector.scalar_tensor_tensor(
                    out=tmp[:, :fw], in0=o3[:, :fw], scalar=b3sc,
                    in1=t1[:, :fw], op0=OP.add, op1=OP.mult)
                nc.scalar.activation(hh[:, hc, fsl], tmp[:, :fw], AF.Copy,
                                     scale=WSI2)
                nc.vector.scalar_tensor_tensor(
                    out=hl[:, hc, fsl], in0=tmp[:, :fw], scalar=WSI2,
                    in1=hh[:, hc, fsl], op0=OP.mult, op1=OP.subtract)

            def dr3(out_ap, lh, ll, rh, rl, kn, fsl, first_start, last_stop):
                """3-term compensated fp8 DoubleRow accumulation group."""
                terms = [(lh, rh), (lh, rl), (ll, rh)]
                for ti, (lt_, rt_) in enumerate(terms):
                    for k2 in range(kn):
                        ksl2 = slice(2 * k2, 2 * k2 + 2)
                        rhs = (rt_[:, ksl2, fsl] if fsl is not None
                               else rt_[:, ksl2, :])
                        nc.tensor.matmul(
                            out=out_ap,
                            lhsT=lt_[:, ksl2, :],
                            rhs=rhs,
                            start=(first_start and ti == 0 and k2 == 0),
                            stop=(last_stop and ti == 2 and k2 == kn - 1),
                            perf_mode=DR)

            def expert_body(e, is_shared, pre=None):
                b1sb = b13.tile([P, HCN], F32, name="b1sb")
                nc.sync.dma_start(out=b1sb[:], in_=b1a[e])
                b1ssb = b13.tile([P, HCN], F32, name="b1ssb")
                nc.sync.dma_start(out=b1ssb[:], in_=b1s[e])
                b3ssb = b13.tile([P, HCN], F32, name="b3ssb")
                nc.sync.dma_start(out=b3ssb[:], in_=b3s[e])

                if not is_shared:
                    # one-hot dispatch tiles per token-tile pair
                    pes = []
                    for pr in range(NP):
                        pe2 = pep.tile([P, 2, capm], F8, name="pe2", tag="pe2")
                        for half in range(2):
                            nc.vector.tensor_scalar(
                                out=pe2[:, half, :], in0=iota_sb[:],
                                scalar1=posb_all[:, 2 * pr + half, e:e + 1],
                                scalar2=None, op0=OP.is_equal)
                        pes.append(pe2)
                    # matmul gather, feature-major, hi+lo
                    xeh = xep.tile([P, KD, CAPE], F8, name="xeh", tag="xeh")
                    xel = xep.tile([P, KD, CAPE], F8, name="xel", tag="xel")
                    for src, dst, nm in ((xth, xeh, "gxh"), (xtl, xel, "gxl")):
                        for k in range(KD):
                            ksl = slice(k * P, (k + 1) * P)
                            for pp0 in range(0, NP, 2):
                                gx = ps_sm.tile([P, 2 * capm], F32,
                                                space="PSUM", name=nm, tag="sm")
                                for pi in range(2):
                                    pr = pp0 + pi
                                    csl = slice(pi * capm, (pi + 1) * capm)
                                    nc.tensor.matmul(
                                        out=gx[:, csl],
                                        lhsT=src[:, 2 * pr:2 * pr + 2, ksl],
                                        rhs=pes[pr][:],
                                        start=True, stop=True, perf_mode=DR)
                                nc.scalar.activation(
                                    dst[:, k, pp0 * capm:(pp0 + 2) * capm],
                                    gx[:], AF.Copy)
                    rh_h, rl_h = xeh, xel
                    fw1 = CAPE
                else:
                    rh_h, rl_h = xbh, xbl
                    fw1 = None  # uses ft tiling

                hh = hpool.tile([P, HCN, T], F8, name="hh", tag="hh")
                hl = hpool.tile([P, HCN, T], F8, name="hl", tag="hl")
                for hc in range(HCN):
                    if pre is not None and hc in pre:
                        w1ch, w1cl, w3ch, w3cl = pre[hc]
                    else:
                        w1ch = w1s.tile([P, KD, P], F8, name="w1ch", tag="w1ch")
                        nc.sync.dma_start(out=w1ch[:], in_=w1h[e, hc])
                        w1cl = w1s.tile([P, KD, P], F8, name="w1cl", tag="w1cl")
                        nc.sync.dma_start(out=w1cl[:], in_=w1l[e, hc])
                        w3ch = w1s.tile([P, KD, P], F8, name="w3ch", tag="w3ch")
                        nc.sync.dma_start(out=w3ch[:], in_=w3h[e, hc])
                        w3cl = w1s.tile([P, KD, P], F8, name="w3cl", tag="w3cl")
                        nc.sync.dma_start(out=w3cl[:], in_=w3l[e, hc])
                    if is_shared:
                        for ft in range(FT):
                            fsl = slice(ft * 512, min((ft + 1) * 512, T))
                            fw = fsl.stop - fsl.start
                            o1 = ps_l1.tile([P, 512], F32, space="PSUM",
                                            name="o1")
                            dr3(o1[:, :fw], w1ch, w1cl, rh_h, rl_h,
                                KD2, fsl, True, True)
                            o3 = ps_l1.tile([P, 512], F32, space="PSUM",
                                            name="o3")
                            dr3(o3[:, :fw], w3ch, w3cl, rh_h, rl_h,
                                KD2, fsl, True, True)
                            l1_evict(o1, o3, b1sb[:, hc:hc + 1],
                                     b1ssb[:, hc:hc + 1], b3ssb[:, hc:hc + 1],
                                     hh, hl, hc, fsl, fw)
                    else:
                        fsl = slice(0, fw1)
                        o1 = ps_l1.tile([P, 512], F32, space="PSUM", name="o1")
                        dr3(o1[:, :fw1], w1ch, w1cl, rh_h, rl_h,
                            KD2, None, True, True)
                        o3 = ps_l1.tile([P, 512], F32, space="PSUM", name="o3")
                        dr3(o3[:, :fw1], w3ch, w3cl, rh_h, rl_h,
                            KD2, None, True, True)
                        l1_evict(o1, o3, b1sb[:, hc:hc + 1],
                                 b1ssb[:, hc:hc + 1], b3ssb[:, hc:hc + 1],
                                 hh, hl, hc, fsl, fw1)

                w2sh = w2s.tile([P, HCN, D], F8, name="w2sh")
                nc.sync.dma_start(out=w2sh[:], in_=w2h[e])
                w2sl = w2s.tile([P, HCN, D], F8, name="w2sl")
                nc.sync.dma_start(out=w2sl[:], in_=w2l[e])
                return hh, hl, w2sh, w2sl

            def l2_matmuls(yp, hh, hl, w2sh, w2sl, tsl, dsl, dw, sw,
                           first_start):
                terms = [(hh, w2sh), (hh, w2sl), (hl, w2sh)]
                for ti, (lt_, rt_) in enumerate(terms):
                    for h2 in range(HCN2):
                        nc.tensor.matmul(
                            out=yp[:sw, :dw],
                            lhsT=lt_[:, 2 * h2:2 * h2 + 2, tsl],
                            rhs=rt_[:, 2 * h2:2 * h2 + 2, dsl],
                            start=(first_start and ti == 0 and h2 == 0),
                            stop=(ti == 2 and h2 == HCN2 - 1),
                            perf_mode=DR)

            # ---- shared expert, first half (dense over all tokens) ----
            hh, hl, w2sh, w2sl = expert_body(E, True, pre=pre_w)
            for tt in range(TT):
                tsl = slice(tt * P, (tt + 1) * P)
                for dt in range(DT):
                    dsl = slice(dt * 512, min((dt + 1) * 512, D))
                    dw = dsl.stop - dsl.start
                    yp = ps_y.tile([P, 512], F32, space="PSUM", name="yp")
                    nc.tensor.matmul(out=yp[:, :dw], lhsT=ones_sb[:],
                                     rhs=sb2x_sb[0:1, dsl],
                                     start=True, stop=False)
                    l2_matmuls(yp, hh, hl, w2sh, w2sl, tsl, dsl, dw, P, False)
                    nc.scalar.activation(yshared[:, tt, dsl], yp[:, :dw],
                                         AF.Copy, scale=WSI)

            # ---- routed experts over dispatched slots ----
            for e in range(E):
                hh, hl, w2sh, w2sl = expert_body(e, False)
                for st in range(ST):
                    sw = min(P, CAPE - st * P)
                    ssl = slice(st * P, st * P + sw)
                    for dt in range(DT):
                        dsl = slice(dt * 512, min((dt + 1) * 512, D))
                        dw = dsl.stop - dsl.start
                        yp = ps_y.tile([P, 512], F32, space="PSUM", name="yp")
                        l2_matmuls(yp, hh, hl, w2sh, w2sl, ssl, dsl, dw, sw,
                                   True)
                        yeb = comb.tile([P, 512], BF, name="yeb")
                        nc.scalar.activation(yeb[:sw, :dw], yp[:sw, :dw],
                                             AF.Copy, scale=WSI)
                        nc.sync.dma_start(
                            out=ye[e * CAPE + st * P: e * CAPE + st * P + sw,
                                   dsl],
                            in_=yeb[:sw, :dw])

            # ---- combine: yshared += b2@cw + sum_e cw_e * ye[slot] ----
            for m in range(TT):
                pr = m // 2
                yps = []
                pool_c, tag_c = (ps_y, "yp") if m % 2 == 0 else (ps_l1, "o1")
                for dt in range(DT):
                    yps.append(pool_c.tile([P, 512], F32, space="PSUM",
                                           name=f"ypc{dt}", tag=tag_c))
                for dt in range(DT):
                    dsl = slice(dt * 512, min((dt + 1) * 512, D))
                    dw = dsl.stop - dsl.start
                    nc.tensor.matmul(out=yps[dt][:, :dw],
                                     lhsT=cwT[:, m, :], rhs=b2e8_sb[:, dsl],
                                     start=True, stop=False)
                for e in range(E):
                    yeb_sb = yebp.tile([capm, D], BF)
                    nc.sync.dma_start(
                        out=yeb_sb[:],
                        in_=ye[e * CAPE + pr * capm:
                               e * CAPE + (pr + 1) * capm, :])
                    pe = gtmp.tile([P, capm], BF, name="pe")
                    nc.vector.tensor_scalar(out=pe[:], in0=iota_sb[:],
                                            scalar1=posb_all[:, m, e:e + 1],
                                            scalar2=None, op0=OP.is_equal)
                    pew = gtmp.tile([P, capm], BF, name="pew")
                    nc.vector.tensor_scalar(out=pew[:], in0=pe[:],
                                            scalar1=cw[:, m, e:e + 1],
                                            scalar2=None, op0=OP.mult)
                    pool_t, tag_t = (ps_sm, "sm") if e % 2 == 0 else (ps_l1, "o3")
                    p2 = pool_t.tile([P, P], BF, space="PSUM",
                                     name="p2", tag=tag_t)
                    nc.tensor.transpose(out=p2[:capm, :], in_=pew[:],
                                        identity=id_sb[:])
                    p2s = gtmp.tile([capm, P], BF, name="p2s")
                    nc.scalar.activation(p2s[:], p2[:capm, :], AF.Copy)
                    for dt in range(DT):
                        dsl = slice(dt * 512, min((dt + 1) * 512, D))
                        dw = dsl.stop - dsl.start
                        nc.tensor.matmul(out=yps[dt][:, :dw], lhsT=p2s[:],
                                         rhs=yeb_sb[:, dsl],
                                         start=False, stop=(e == E - 1))
                for dt in range(DT):
                    dsl = slice(dt * 512, min((dt + 1) * 512, D))
                    dw = dsl.stop - dsl.start
                    nc.vector.tensor_add(yshared[:, m, dsl],
                                         yshared[:, m, dsl],
                                         yps[dt][:, :dw])

            # ---- shared expert, second half; emits final y ----
            hh, hl, w2sh, w2sl = expert_body(E + 1, True)
            for tt in range(TT):
                tsl = slice(tt * P, (tt + 1) * P)
                for dt in range(DT):
                    dsl = slice(dt * 512, min((dt + 1) * 512, D))
                    dw = dsl.stop - dsl.start
                    yp = ps_y.tile([P, 512], F32, space="PSUM", name="yp")
                    l2_matmuls(yp, hh, hl, w2sh, w2sl, tsl, dsl, dw, P, True)
                    yt = comb.tile([P, 512], F32, name="yt")
                    nc.vector.scalar_tensor_tensor(
                        out=yt[:, :dw], in0=yp[:, :dw], scalar=WSI,
                        in1=yshared[:, tt, dsl], op0=OP.mult, op1=OP.add)
                    nc.sync.dma_start(out=y[:, tt, dsl], in_=yt[:, :dw])

    nc.compile()
    return nc


# ---------------- host-side packing ----------------

def _split_fp8(a, scale=1.0):
    a = np.asarray(a, np.float32) * scale
    hi = a.astype(FP8)
    lo = (a - hi.astype(np.float32)).astype(FP8)
    return hi, lo


def pack_static(cfg: Cfg, gate_w, gate_b, w1, b1, w2, b2, w3, b3,
                sw1, sb1, sw2, sb2, sw3, sb3):
    D, H, E, NV, n_sh = cfg.D, cfg.H, cfg.E, cfg.NV, cfg.n_sh
    KD, HCN = cfg.KD, cfg.HCN

    w1T = np.transpose(w1, (0, 2, 1))                      # [E, D, H]
    w3T = np.transpose(w3, (0, 2, 1))
    w2T = np.transpose(w2, (0, 2, 1))                      # [E, H, D]
    s1T = sw1.T.reshape(D, n_sh, H).transpose(1, 0, 2)     # [n_sh, D, H]
    s3T = sw3.T.reshape(D, n_sh, H).transpose(1, 0, 2)
    s2T = sw2.T.reshape(n_sh, H, D)                        # [n_sh, H, D]
    w1T_all = np.concatenate([w1T, s1T], 0)                # [NV, D, H]
    w3T_all = np.concatenate([w3T, s3T], 0)
    w2T_all = np.concatenate([w2T, s2T], 0)                # [NV, H, D]

    w1t = np.ascontiguousarray(
        w1T_all.reshape(NV, KD, P, HCN, P).transpose(0, 3, 2, 1, 4))
    w3t = np.ascontiguousarray(
        w3T_all.reshape(NV, KD, P, HCN, P).transpose(0, 3, 2, 1, 4))
    w2t = np.ascontiguousarray(
        w2T_all.reshape(NV, HCN, P, D).transpose(0, 2, 1, 3))
    w1h_, w1l_ = _split_fp8(w1t, WS)
    w3h_, w3l_ = _split_fp8(w3t, WS)
    w2h_, w2l_ = _split_fp8(w2t, WS)

    b1_all = np.concatenate([b1, sb1.reshape(n_sh, H)], 0)  # [NV, H]
    b3_all = np.concatenate([b3, sb3.reshape(n_sh, H)], 0)
    b1a = np.ascontiguousarray(
        b1_all.reshape(NV, HCN, P).transpose(0, 2, 1)).astype(np.float32)
    b3a = np.ascontiguousarray(
        b3_all.reshape(NV, HCN, P).transpose(0, 2, 1)).astype(np.float32)

    gwt = np.ascontiguousarray(
        gate_w.T.reshape(KD, P, E).transpose(1, 0, 2)).astype(np.float32)

    return dict(
        w1h=w1h_, w1l=w1l_, w3h=w3h_, w3l=w3l_, w2h=w2h_, w2l=w2l_,
        b1a=b1a, b1s=(WS * b1a).astype(np.float32),
        b3s=(WS * b3a).astype(np.float32),
        b2e8=b2.astype(BF16),                               # [E, D]
        sb2x=(WS * sb2[None]).astype(BF16),                 # [1, D]
        gwt=gwt, gb=gate_b[None].astype(np.float32),
        ones1=np.ones((1, P), BF16),
        onesc=np.ones((P, 1), BF16),
        lt=np.triu(np.ones((P, P))).astype(BF16),
        ident=np.eye(P).astype(BF16),
        iota=np.tile(np.arange(cfg.capm, dtype=np.float32), (P, 1)),
    )


def pack_x(cfg: Cfg, x_tokens):
    """x_tokens [T, D] fp32 -> device layouts (gate fp32 + fp8 hi/lo)."""
    T, D = x_tokens.shape
    xT = np.ascontiguousarray(
        x_tokens.T.reshape(cfg.KD, P, T).transpose(1, 0, 2)).astype(np.float32)
    xh, xl = _split_fp8(xT)
    xtok = np.ascontiguousarray(
        x_tokens.reshape(cfg.TT, P, D).transpose(1, 0, 2))
    xth, xtl = _split_fp8(xtok)
    return dict(xT=xT, xtbh=xh, xtbl=xl, xtokh=xth, xtokl=xtl)


def unpack_y(cfg: Cfg, y_dev):
    """y device layout [P, TT, D] -> [T, D]."""
    return np.ascontiguousarray(y_dev.transpose(1, 0, 2).reshape(cfg.T, cfg.D))


_CACHE = {}


def _get_nc(cfg: Cfg):
    key = (cfg.D, cfg.H, cfg.E, cfg.n_sh, cfg.T, cfg.capm)
    if key not in _CACHE:
        _CACHE[key] = build_nc_fp8(cfg)
    return _CACHE[key]


def make_in_maps(cfg: Cfg, inputs):
    static = pack_static(
        cfg,
        np.asarray(inputs["gate_w"], np.float32), np.asarray(inputs["gate_b"], np.float32),
        np.asarray(inputs["w1"], np.float32), np.asarray(inputs["b1"], np.float32),
        np.asarray(inputs["w2"], np.float32), np.asarray(inputs["b2"], np.float32),
        np.asarray(inputs["w3"], np.float32), np.asarray(inputs["b3"], np.float32),
        np.asarray(inputs["sw1"], np.float32), np.asarray(inputs["sb1"], np.float32),
        np.asarray(inputs["sw2"], np.float32), np.asarray(inputs["sb2"], np.float32),
        np.asarray(inputs["sw3"], np.float32), np.asarray(inputs["sb3"], np.float32),
    )
    x = np.asarray(inputs["x"], np.float32)
    B, S, D = x.shape
    xf = x.reshape(-1, D)
    in_maps = []
    for c in range(cfg.n_cores):
        m = dict(static)
        m.update(pack_x(cfg, xf[c * cfg.T:(c + 1) * cfg.T]))
        in_maps.append(m)
    return in_maps


def kernel(**inputs) -> np.ndarray:
    x = np.asarray(inputs["x"], np.float32)
    B, S, D = x.shape
    N = B * S
    cfg = Cfg(D=D, T=N // 8, n_cores=8)
    nc = _get_nc(cfg)
    in_maps = make_in_maps(cfg, inputs)
    res = run_bass_kernel_spmd(nc, in_maps, list(range(cfg.n_cores)))
    outs = [unpack_y(cfg, res.results[c]["y"]) for c in range(cfg.n_cores)]
    return np.concatenate(outs, 0).reshape(B, S, D)


# revision 11
# speedup vs baseline: 1.1803x; 1.0431x over previous
"""MoE (8 routed experts, top-2, + shared expert) on 8 NeuronCores.

Data-parallel over tokens (1024/core), weights replicated, capacity-routed
dispatch (pair buckets, capm=96) as in the bf16 baseline — but all large
GEMMs run as fp8e4m3 DoubleRow matmuls with 3-term error compensation:

    A @ B  ~=  A_hi@B_hi + A_hi@B_lo + A_lo@B_hi

where X_hi = fp8(X), X_lo = fp8(X - X_hi).  DoubleRow consumes two
128-deep K-tiles per instruction at 0.5 cycles/row, so each compensated
GEMM costs 0.75x its bf16 schedule while adding only ~1e-3 relative
error.  Weights are pre-scaled by 64 on the host (fp8 subnormal cutoff),
descaled in the PSUM eviction.  The gate runs in fp32 so routing matches
the reference.

Schedule notes: w1/w3 hi+lo chunks ride one DMA per hc-pair (the SP
sequencer costs ~0.9us per DMA, so descriptor count is a real resource);
expert e+1's token gather is issued between expert e's L1 and L2 so its
PSUM evictions hide under L2 matmuls; the L1 eviction chain is
Silu -> scale -> mul with the fp8 split offloaded to the idle GPSIMD
engine; half of the shared expert runs after the combine phase so the
combine's DRAM round-trip sits under shared-expert matmuls, not at the
kernel tail.
"""

import numpy as np
import ml_dtypes

import concourse.bacc as bacc
import concourse.bass as bass
import concourse.tile as tile
import concourse.mybir as mybir
from concourse.bass_utils import run_bass_kernel_spmd

BF16 = ml_dtypes.bfloat16
FP8 = ml_dtypes.float8_e4m3
F32 = mybir.dt.float32
BF = mybir.dt.bfloat16
F8 = mybir.dt.float8e4
AF = mybir.ActivationFunctionType
OP = mybir.AluOpType
DR = mybir.MatmulPerfMode.DoubleRow

P = 128
WS = 64.0          # host-side weight scale before fp8 quantization
WSI = 1.0 / WS


class Cfg:
    def __init__(self, D=1024, H=2048, E=8, n_sh=2, T=1024, n_cores=8, capm=96):
        self.D, self.H, self.E, self.n_sh, self.T = D, H, E, n_sh, T
        self.NV = E + n_sh          # virtual experts
        self.HS = n_sh * H          # shared hidden
        self.KD = D // P            # K chunks over D
        self.HCN = H // P           # h chunks over H
        self.TT = T // P            # token 128-tiles
        self.DT = (D + 511) // 512  # output d 512-tiles
        self.FT = (T + 511) // 512  # layer-1 free 512-tiles
        self.n_cores = n_cores
        self.capm = capm            # per-(expert, tile-pair) dispatch capacity
        self.NP = self.TT // 2      # token-tile pairs
        self.CAPE = self.NP * capm  # slots per expert
        self.ST = (self.CAPE + P - 1) // P  # slot 128-tiles per expert


def build_nc_fp8(cfg: Cfg):
    D, H, E, NV, T = cfg.D, cfg.H, cfg.E, cfg.NV, cfg.T
    KD, HCN, TT, DT, FT = cfg.KD, cfg.HCN, cfg.TT, cfg.DT, cfg.FT
    capm, CAPE, ST, NP = cfg.capm, cfg.CAPE, cfg.ST, cfg.NP
    KD2, HCN2 = KD // 2, HCN // 2

    nc = bacc.Bacc("TRN2", target_bir_lowering=False)

    xT = nc.dram_tensor("xT", [P, KD, T], F32, kind="ExternalInput")
    xtbh = nc.dram_tensor("xtbh", [P, KD, T], F8, kind="ExternalInput")
    xtbl = nc.dram_tensor("xtbl", [P, KD, T], F8, kind="ExternalInput")
    xtokh = nc.dram_tensor("xtokh", [P, TT, D], F8, kind="ExternalInput")
    xtokl = nc.dram_tensor("xtokl", [P, TT, D], F8, kind="ExternalInput")
    # w1/w3 hi+lo packed per hc-pair: [e, hcp, p, i(2), which(4), KD, P]
    wq = nc.dram_tensor("wq", [NV, HCN2, P, 2, 4, KD, P], F8,
                        kind="ExternalInput")
    # w2 hi+lo packed per expert: [e, p, which(2), HCN, D]
    w2q = nc.dram_tensor("w2q", [NV, P, 2, HCN, D], F8, kind="ExternalInput")
    # b1, b3 packed per expert: [e, p, which(2), HCN]
    bq = nc.dram_tensor("bq", [NV, P, 2, HCN], F32, kind="ExternalInput")
    b2e8 = nc.dram_tensor("b2e8", [E, D], BF, kind="ExternalInput")
    sb2x = nc.dram_tensor("sb2x", [1, D], BF, kind="ExternalInput")
    gwt = nc.dram_tensor("gwt", [P, KD, E], F32, kind="ExternalInput")
    gb = nc.dram_tensor("gb", [1, E], F32, kind="ExternalInput")
    ones1 = nc.dram_tensor("ones1", [1, P], BF, kind="ExternalInput")
    onesc = nc.dram_tensor("onesc", [P, 1], BF, kind="ExternalInput")
    lt = nc.dram_tensor("lt", [P, P], BF, kind="ExternalInput")
    ident = nc.dram_tensor("ident", [P, P], BF, kind="ExternalInput")
    iota = nc.dram_tensor("iota", [P, capm], F32, kind="ExternalInput")
    y = nc.dram_tensor("y", [P, TT, D], F32, kind="ExternalOutput")

    OOB = 3.0e6

    with tile.TileContext(nc) as tc:
        with (
            tc.tile_pool(name="const1", bufs=1) as const1,
            tc.tile_pool(name="gchunk", bufs=1) as gchunk,
            tc.tile_pool(name="gtmp", bufs=4) as gtmp,
            tc.tile_pool(name="w1s", bufs=2) as w1s,
            tc.tile_pool(name="b13", bufs=2) as b13,
            tc.tile_pool(name="w2s", bufs=1) as w2s,
            tc.tile_pool(name="hpool", bufs=1) as hpool,
            tc.tile_pool(name="s1p", bufs=2) as s1p,
            tc.tile_pool(name="yebp", bufs=2) as yebp,
            tc.tile_pool(name="yea", bufs=1) as yea,
            tc.tile_pool(name="xep", bufs=2) as xep,
            tc.tile_pool(name="pep", bufs=8) as pep,
            tc.tile_pool(name="comb", bufs=2) as comb,
            tc.tile_pool(name="dram", bufs=1, space="DRAM") as drp,
            tc.tile_pool(name="ps_l1", bufs=2, space="PSUM") as ps_l1,
            tc.tile_pool(name="ps_y", bufs=2, space="PSUM") as ps_y,
            tc.tile_pool(name="ps_sm", bufs=2, space="PSUM") as ps_sm,
        ):
            # per-slot expert outputs, slot-major / expert-minor so combine
            # reads one contiguous [rows, e, D] block per bucket window
            ye = drp.tile([ST, P, E, D], BF)

            # ---- resident constants / state ----
            xbh = const1.tile([P, KD, T], F8)
            xbl = const1.tile([P, KD, T], F8)
            xth = const1.tile([P, TT, D], F8)
            xtl = const1.tile([P, TT, D], F8)
            yshared = const1.tile([P, TT, D], BF)
            cw = const1.tile([P, TT, E], F32)
            cwT = const1.tile([8, TT, P], BF)
            posb_all = const1.tile([P, TT, E], F32)
            ones_sb = const1.tile([1, P], BF)
            onesc_sb = const1.tile([P, 1], BF)
            gwt_sb = const1.tile([P, KD, E], F32)
            gb_sb = const1.tile([1, E], F32)
            zerob = const1.tile([P, 1], F32)
            onesf = const1.tile([1, P], F32)
            lt_sb = const1.tile([P, P], BF)
            id_sb = const1.tile([P, P], BF)
            iota_sb = const1.tile([P, capm], F32)
            b2e8_sb = const1.tile([8, D], BF)
            sb2x_sb = const1.tile([1, D], BF)

            nc.sync.dma_start(out=gwt_sb[:], in_=gwt[:])
            nc.sync.dma_start(out=gb_sb[:], in_=gb[:])
            nc.sync.dma_start(out=ones_sb[:], in_=ones1[:])
            nc.sync.dma_start(out=onesc_sb[:], in_=onesc[:])
            nc.sync.dma_start(out=lt_sb[:], in_=lt[:])
            nc.sync.dma_start(out=id_sb[:], in_=ident[:])
            nc.sync.dma_start(out=iota_sb[:], in_=iota[:])
            nc.sync.dma_start(out=b2e8_sb[:], in_=b2e8[:])
            nc.sync.dma_start(out=sb2x_sb[:], in_=sb2x[:])
            nc.vector.memset(zerob[:], 0.0)
            nc.vector.memset(onesf[:], 1.0)

            # prefetch the first shared-half L1 weight chunks so its matmuls
            # can start while the gate phase runs
            pre_w = {}
            for hcp in range(min(2, HCN2)):
                t = w1s.tile([P, 2, 4, KD, P], F8, name="wqt", tag="wqt")
                nc.sync.dma_start(out=t[:], in_=wq[E, hcp])
                pre_w[hcp] = t
            nc.sync.dma_start(out=xbh[:], in_=xtbh[:])
            nc.sync.dma_start(out=xbl[:], in_=xtbl[:])
            nc.sync.dma_start(out=xth[:], in_=xtokh[:])
            nc.sync.dma_start(out=xtl[:], in_=xtokl[:])

            # ---- gate + routing, per token tile (paired buckets) ----
            cntb = None
            for m in range(TT):
                xchunk = gchunk.tile([P, KD, P], F32)
                nc.sync.dma_start(out=xchunk[:], in_=xT[:, :, m * P:(m + 1) * P])

                pg = ps_l1.tile([P, P], F32, space="PSUM", name="pg", tag="o1")
                for k in range(KD):
                    nc.tensor.matmul(out=pg[:, :E], lhsT=xchunk[:, k, :],
                                     rhs=gwt_sb[:, k, :],
                                     start=(k == 0), stop=False)
                nc.tensor.matmul(out=pg[:, :E], lhsT=onesf[:], rhs=gb_sb[:],
                                 start=False, stop=True)

                lg = gtmp.tile([P, E], F32)
                nc.scalar.activation(lg[:], pg[:, :E], AF.Copy)
                m8 = gtmp.tile([P, 8], F32)
                nc.vector.max(m8[:], lg[:])
                ex = gtmp.tile([P, E], F32)
                nc.vector.tensor_scalar(out=ex[:], in0=lg[:],
                                        scalar1=m8[:, 0:1], scalar2=None,
                                        op0=OP.subtract)
                nc.scalar.activation(ex[:], ex[:], AF.Exp, bias=zerob[:])
                mask = gtmp.tile([P, E], F32)
                nc.vector.tensor_scalar(out=mask[:], in0=lg[:],
                                        scalar1=m8[:, 1:2], scalar2=None,
                                        op0=OP.is_ge)
                e2 = gtmp.tile([P, 1], F32)
                nc.vector.tensor_tensor(out=e2[:], in0=m8[:, 1:2],
                                        in1=m8[:, 0:1], op=OP.subtract)
                nc.scalar.activation(e2[:], e2[:], AF.Exp, bias=zerob[:])
                den = gtmp.tile([P, 1], F32)
                nc.vector.tensor_scalar(out=den[:], in0=e2[:], scalar1=1.0,
                                        scalar2=None, op0=OP.add)
                rec = gtmp.tile([P, 1], F32)
                nc.vector.reciprocal(rec[:], den[:])
                cwm = gtmp.tile([P, E], F32)
                nc.vector.tensor_mul(cwm[:], ex[:], mask[:])
                nc.vector.tensor_scalar(out=cw[:, m, :], in0=cwm[:],
                                        scalar1=rec[:, 0:1], scalar2=None,
                                        op0=OP.mult)
                # bf16 transpose of the combine weights for the b2 matmul
                cwb = gtmp.tile([P, E], BF, name="cwb")
                nc.vector.tensor_copy(cwb[:], cw[:, m, :])
                ptp = ps_sm.tile([P, P], BF, space="PSUM", name="ptp", tag="sm")
                nc.tensor.transpose(out=ptp[:E, :], in_=cwb[:], identity=id_sb[:])
                nc.scalar.activation(cwT[:, m, :], ptp[:E, :], AF.Copy)

                # bucket-local slot: pair prefix(mask) - mask; OOB unrouted
                maskb = gtmp.tile([P, E], BF)
                nc.vector.tensor_copy(maskb[:], mask[:])
                pp = ps_y.tile([P, P], F32, space="PSUM", name="pp", tag="yp")
                if m % 2 == 0:
                    nc.tensor.matmul(out=pp[:, :E], lhsT=lt_sb[:],
                                     rhs=maskb[:], start=True, stop=True)
                    cnt_ps = ps_sm.tile([1, P], F32, space="PSUM",
                                        name="cntp", tag="sm")
                    nc.tensor.matmul(out=cnt_ps[0:1, :E], lhsT=onesc_sb[:],
                                     rhs=maskb[:], start=True, stop=True)
                    cntb = gtmp.tile([1, E], BF, name="cntb")
                    nc.scalar.activation(cntb[:], cnt_ps[0:1, :E], AF.Copy)
                else:
                    nc.tensor.matmul(out=pp[:, :E], lhsT=lt_sb[:],
                                     rhs=maskb[:], start=True, stop=False)
                    nc.tensor.matmul(out=pp[:, :E], lhsT=ones_sb[:],
                                     rhs=cntb[:], start=False, stop=True)
                t1m = gtmp.tile([P, E], F32)
                nc.vector.scalar_tensor_tensor(out=t1m[:], in0=mask[:],
                                               scalar=-1.0, in1=pp[:, :E],
                                               op0=OP.mult, op1=OP.add)
                notm = gtmp.tile([P, E], F32)
                nc.vector.tensor_scalar(out=notm[:], in0=mask[:],
                                        scalar1=-1.0, scalar2=1.0,
                                        op0=OP.mult, op1=OP.add)
                nc.vector.scalar_tensor_tensor(out=posb_all[:, m, :],
                                               in0=notm[:], scalar=OOB,
                                               in1=t1m[:],
                                               op0=OP.mult, op1=OP.add)

            def dr3(out_ap, lh, ll, rh, rl, kn, fsl, sel=None):
                """3-term compensated fp8 DoubleRow accumulation group.

                lh/ll: either plain [P, KD, P] tiles or a packed wqt tile
                indexed via sel=(i, jh, jl).  k2-major so gather evictions
                unblock the group incrementally.
                """
                for k2 in range(kn):
                    ksl2 = slice(2 * k2, 2 * k2 + 2)
                    if sel is None:
                        lhs_h, lhs_l = lh[:, ksl2, :], ll[:, ksl2, :]
                    else:
                        i, jh, jl = sel
                        lhs_h = lh[:, i, jh, ksl2, :]
                        lhs_l = lh[:, i, jl, ksl2, :]
                    rhs_h = rh[:, ksl2, fsl] if fsl is not None else rh[:, ksl2]
                    rhs_l = rl[:, ksl2, fsl] if fsl is not None else rl[:, ksl2]
                    for ti, (lt_, rt_) in enumerate(
                            ((lhs_h, rhs_h), (lhs_h, rhs_l), (lhs_l, rhs_h))):
                        nc.tensor.matmul(
                            out=out_ap, lhsT=lt_, rhs=rt_,
                            start=(k2 == 0 and ti == 0),
                            stop=(k2 == kn - 1 and ti == 2),
                            perf_mode=DR)

            def l1_evict(o1, o3, b1c, b3c, hh, hl, hc, fsl, fw):
                """h = silu(o1/WS + b1) * (o3/WS + b3) -> fp8 hi/lo pair."""
                s = s1p.tile([P, 512], F32, name="s")
                nc.scalar.activation(s[:, :fw], o1[:, :fw], AF.Silu,
                                     bias=b1c, scale=WSI)
                v = s1p.tile([P, 512], F32, name="v")
                nc.vector.tensor_scalar(out=v[:, :fw], in0=o3[:, :fw],
                                        scalar1=WSI, scalar2=b3c,
                                        op0=OP.mult, op1=OP.add)
                hf = s1p.tile([P, 512], F32, name="hf")
                nc.vector.tensor_mul(hf[:, :fw], s[:, :fw], v[:, :fw])
                nc.scalar.activation(hh[:, hc, fsl], hf[:, :fw], AF.Copy)
                nc.gpsimd.tensor_sub(hl[:, hc, fsl], hf[:, :fw],
                                     hh[:, hc, fsl])

            def gather(e, xeh, xel):
                """One-hot dispatch + feature-major token gather, hi+lo."""
                pes = []
                for pr in range(NP):
                    pe2 = pep.tile([P, 2, capm], F8, name="pe2", tag="pe2")
                    for half in range(2):
                        nc.vector.tensor_scalar(
                            out=pe2[:, half, :], in0=iota_sb[:],
                            scalar1=posb_all[:, 2 * pr + half, e:e + 1],
                            scalar2=None, op0=OP.is_equal)
                    pes.append(pe2)
                for k in range(KD):
                    ksl = slice(k * P, (k + 1) * P)
                    for src, dst, nm in ((xth, xeh, "gxh"), (xtl, xel, "gxl")):
                        gx = ps_sm.tile([P, NP * capm], F32,
                                        space="PSUM", name=nm, tag="sm")
                        for pr in range(NP):
                            nc.tensor.matmul(
                                out=gx[:, pr * capm:(pr + 1) * capm],
                                lhsT=src[:, 2 * pr:2 * pr + 2, ksl],
                                rhs=pes[pr][:],
                                start=True, stop=True, perf_mode=DR)
                        nc.scalar.activation(dst[:, k, :], gx[:], AF.Copy)

            def l1_phase(e, is_shared, rh_h, rl_h, pre=None):
                bqt = b13.tile([P, 2, HCN], F32, name="bqt")
                nc.sync.dma_start(out=bqt[:], in_=bq[e])
                hh = hpool.tile([P, HCN, T], F8, name="hh", tag="hh")
                hl = hpool.tile([P, HCN, T], F8, name="hl", tag="hl")
                for hcp in range(HCN2):
                    if pre is not None and hcp in pre:
                        wqt = pre[hcp]
                    else:
                        wqt = w1s.tile([P, 2, 4, KD, P], F8, name="wqt",
                                       tag="wqt")
                        nc.sync.dma_start(out=wqt[:], in_=wq[e, hcp])
                    for i in range(2):
                        hc = 2 * hcp + i
                        b1c = bqt[:, 0, hc:hc + 1]
                        b3c = bqt[:, 1, hc:hc + 1]
                        if is_shared:
                            for ft in range(FT):
                                fsl = slice(ft * 512, min((ft + 1) * 512, T))
                                fw = fsl.stop - fsl.start
                                o1 = ps_l1.tile([P, 512], F32, space="PSUM",
                                                name="o1")
                                dr3(o1[:, :fw], wqt, None, rh_h, rl_h,
                                    KD2, fsl, sel=(i, 0, 1))
                                o3 = ps_l1.tile([P, 512], F32, space="PSUM",
                                                name="o3")
                                dr3(o3[:, :fw], wqt, None, rh_h, rl_h,
                                    KD2, fsl, sel=(i, 2, 3))
                                l1_evict(o1, o3, b1c, b3c, hh, hl, hc, fsl, fw)
                        else:
                            fsl = slice(0, CAPE)
                            o1 = ps_l1.tile([P, 512], F32, space="PSUM",
                                            name="o1")
                            dr3(o1[:, :CAPE], wqt, None, rh_h, rl_h,
                                KD2, None, sel=(i, 0, 1))
                            o3 = ps_l1.tile([P, 512], F32, space="PSUM",
                                            name="o3")
                            dr3(o3[:, :CAPE], wqt, None, rh_h, rl_h,
                                KD2, None, sel=(i, 2, 3))
                            l1_evict(o1, o3, b1c, b3c, hh, hl, hc, fsl, CAPE)
                w2qt = w2s.tile([P, 2, HCN, D], F8, name="w2qt")
                nc.sync.dma_start(out=w2qt[:], in_=w2q[e])
                return hh, hl, w2qt

            def l2_matmuls(yp, hh, hl, w2qt, tsl, dsl, dw, sw, first_start):
                for h2 in range(HCN2):
                    hsl = slice(2 * h2, 2 * h2 + 2)
                    for ti, (lt_, rt_) in enumerate((
                            (hh[:, hsl, tsl], w2qt[:, 0, hsl, dsl]),
                            (hl[:, hsl, tsl], w2qt[:, 0, hsl, dsl]),
                            (hh[:, hsl, tsl], w2qt[:, 1, hsl, dsl]))):
                        nc.tensor.matmul(
                            out=yp[:sw, :dw], lhsT=lt_, rhs=rt_,
                            start=(first_start and h2 == 0 and ti == 0),
                            stop=(h2 == HCN2 - 1 and ti == 2),
                            perf_mode=DR)

            # ---- shared expert, first half (dense over all tokens) ----
            hh, hl, w2qt = l1_phase(E, True, xbh, xbl, pre=pre_w)
            # expert 0's gather hides under the shared L2 matmuls
            xeh = xep.tile([P, KD, CAPE], F8, name="xeh", tag="xeh")
            xel = xep.tile([P, KD, CAPE], F8, name="xel", tag="xel")
            gather(0, xeh, xel)
            for tt in range(TT):
                tsl = slice(tt * P, (tt + 1) * P)
                for dt in range(DT):
                    dsl = slice(dt * 512, min((dt + 1) * 512, D))
                    dw = dsl.stop - dsl.start
                    yp = ps_y.tile([P, 512], F32, space="PSUM", name="yp")
                    nc.tensor.matmul(out=yp[:, :dw], lhsT=ones_sb[:],
                                     rhs=sb2x_sb[0:1, dsl],
                                     start=True, stop=False)
                    l2_matmuls(yp, hh, hl, w2qt, tsl, dsl, dw, P, False)
                    nc.scalar.activation(yshared[:, tt, dsl], yp[:, :dw],
                                         AF.Copy, scale=WSI)

            # ---- routed experts over dispatched slots ----
            for e in range(E):
                hh, hl, w2qt = l1_phase(e, False, xeh, xel)
                if e < E - 1:
                    xeh = xep.tile([P, KD, CAPE], F8, name="xeh", tag="xeh")
                    xel = xep.tile([P, KD, CAPE], F8, name="xel", tag="xel")
                    gather(e + 1, xeh, xel)
                yebA = yea.tile([P, ST, D], BF, name="yebA")
                for st in range(ST):
                    sw = min(P, CAPE - st * P)
                    ssl = slice(st * P, st * P + sw)
                    for dt in range(DT):
                        dsl = slice(dt * 512, min((dt + 1) * 512, D))
                        dw = dsl.stop - dsl.start
                        yp = ps_y.tile([P, 512], F32, space="PSUM", name="yp")
                        l2_matmuls(yp, hh, hl, w2qt, ssl, dsl, dw, sw, True)
                        nc.scalar.activation(yebA[:sw, st, dsl], yp[:sw, :dw],
                                             AF.Copy, scale=WSI)
                for st in range(ST):
                    nc.sync.dma_start(out=ye[st, :, e, :], in_=yebA[:, st, :])

            # ---- combine: yshared += b2@cw + sum_e cw_e * ye[slot] ----
            for m in range(TT):
                pr = m // 2
                yps = []
                pool_c, tag_c = (ps_y, "yp") if m % 2 == 0 else (ps_l1, "o1")
                for dt in range(DT):
                    yps.append(pool_c.tile([P, 512], F32, space="PSUM",
                                           name=f"ypc{dt}", tag=tag_c))
                # bucket window rows (pr*capm .. pr*capm+capm) as (st, p)
                # pieces; one DMA per piece covers 4 experts per half-tile
                halves = []
                for hf4 in range(2):
                    yeb_sb = yebp.tile([capm, E // 2, D], BF, name="yebh")
                    r0 = pr * capm
                    got = 0
                    while got < capm:
                        st0, p0 = divmod(r0 + got, P)
                        ln = min(P - p0, capm - got)
                        nc.sync.dma_start(
                            out=yeb_sb[got:got + ln, :, :],
                            in_=ye[st0, p0:p0 + ln,
                                   hf4 * (E // 2):(hf4 + 1) * (E // 2), :])
                        got += ln
                    halves.append(yeb_sb)
                for dt in range(DT):
                    dsl = slice(dt * 512, min((dt + 1) * 512, D))
                    dw = dsl.stop - dsl.start
                    nc.tensor.matmul(out=yps[dt][:, :dw],
                                     lhsT=cwT[:, m, :], rhs=b2e8_sb[:, dsl],
                                     start=True, stop=False)
                for e in range(E):
                    pe = gtmp.tile([P, capm], BF, name="pe")
                    nc.vector.tensor_scalar(out=pe[:], in0=iota_sb[:],
                                            scalar1=posb_all[:, m, e:e + 1],
                                            scalar2=None, op0=OP.is_equal)
                    pew = gtmp.tile([P, capm], BF, name="pew")
                    nc.vector.tensor_scalar(out=pew[:], in0=pe[:],
                                            scalar1=cw[:, m, e:e + 1],
                                            scalar2=None, op0=OP.mult)
                    pool_t, tag_t = (ps_sm, "sm") if e % 2 == 0 else (ps_l1, "o3")
                    p2 = pool_t.tile([P, P], BF, space="PSUM",
                                     name="p2", tag=tag_t)
                    nc.tensor.transpose(out=p2[:capm, :], in_=pew[:],
                                        identity=id_sb[:])
                    p2s = gtmp.tile([capm, P], BF, name="p2s")
                    nc.scalar.activation(p2s[:], p2[:capm, :], AF.Copy)
                    for dt in range(DT):
                        dsl = slice(dt * 512, min((dt + 1) * 512, D))
                        dw = dsl.stop - dsl.start
                        nc.tensor.matmul(out=yps[dt][:, :dw], lhsT=p2s[:],
                                         rhs=halves[e // 4][:, e % 4, dsl],
                                         start=False, stop=(e == E - 1))
                for dt in range(DT):
                    dsl = slice(dt * 512, min((dt + 1) * 512, D))
                    dw = dsl.stop - dsl.start
                    nc.vector.tensor_add(yshared[:, m, dsl],
                                         yshared[:, m, dsl],
                                         yps[dt][:, :dw])

            # ---- shared expert, second half; emits final y ----
            hh, hl, w2qt = l1_phase(E + 1, True, xbh, xbl)
            for tt in range(TT):
                tsl = slice(tt * P, (tt + 1) * P)
                for dt in range(DT):
                    dsl = slice(dt * 512, min((dt + 1) * 512, D))
                    dw = dsl.stop - dsl.start
                    yp = ps_y.tile([P, 512], F32, space="PSUM", name="yp")
                    l2_matmuls(yp, hh, hl, w2qt, tsl, dsl, dw, P, True)
                    yt = comb.tile([P, 512], F32, name="yt")
                    nc.vector.scalar_tensor_tensor(
                        out=yt[:, :dw], in0=yp[:, :dw], scalar=WSI,
                        in1=yshared[:, tt, dsl], op0=OP.mult, op1=OP.add)
                    nc.sync.dma_start(out=y[:, tt, dsl], in_=yt[:, :dw])

    nc.compile()
    return nc


# ---------------- host-side packing ----------------

def _split_fp8(a, scale=1.0):
    a = np.asarray(a, np.float32) * scale
    hi = a.astype(FP8)
    lo = (a - hi.astype(np.float32)).astype(FP8)
    return hi, lo


def pack_static(cfg: Cfg, gate_w, gate_b, w1, b1, w2, b2, w3, b3,
                sw1, sb1, sw2, sb2, sw3, sb3):
    D, H, E, NV, n_sh = cfg.D, cfg.H, cfg.E, cfg.NV, cfg.n_sh
    KD, HCN = cfg.KD, cfg.HCN
    HCN2 = HCN // 2

    w1T = np.transpose(w1, (0, 2, 1))                      # [E, D, H]
    w3T = np.transpose(w3, (0, 2, 1))
    w2T = np.transpose(w2, (0, 2, 1))                      # [E, H, D]
    s1T = sw1.T.reshape(D, n_sh, H).transpose(1, 0, 2)     # [n_sh, D, H]
    s3T = sw3.T.reshape(D, n_sh, H).transpose(1, 0, 2)
    s2T = sw2.T.reshape(n_sh, H, D)                        # [n_sh, H, D]
    w1T_all = np.concatenate([w1T, s1T], 0)                # [NV, D, H]
    w3T_all = np.concatenate([w3T, s3T], 0)
    w2T_all = np.concatenate([w2T, s2T], 0)                # [NV, H, D]

    w1t = np.ascontiguousarray(
        w1T_all.reshape(NV, KD, P, HCN, P).transpose(0, 3, 2, 1, 4))
    w3t = np.ascontiguousarray(
        w3T_all.reshape(NV, KD, P, HCN, P).transpose(0, 3, 2, 1, 4))
    w2t = np.ascontiguousarray(
        w2T_all.reshape(NV, HCN, P, D).transpose(0, 2, 1, 3))
    w1h_, w1l_ = _split_fp8(w1t, WS)
    w3h_, w3l_ = _split_fp8(w3t, WS)
    w2h_, w2l_ = _split_fp8(w2t, WS)

    # wq: [NV, HCN2, P, 2, 4, KD, P]
    wq_ = np.stack([w1h_, w1l_, w3h_, w3l_], axis=2)   # [NV, HCN, 4, P, KD, P]
    wq_ = wq_.reshape(NV, HCN2, 2, 4, P, KD, P).transpose(0, 1, 4, 2, 3, 5, 6)
    wq_ = np.ascontiguousarray(wq_)

    # w2q: [NV, P, 2, HCN, D]
    w2q_ = np.ascontiguousarray(
        np.stack([w2h_, w2l_], axis=1).transpose(0, 2, 1, 3, 4))

    b1_all = np.concatenate([b1, sb1.reshape(n_sh, H)], 0)  # [NV, H]
    b3_all = np.concatenate([b3, sb3.reshape(n_sh, H)], 0)
    b1a = b1_all.reshape(NV, HCN, P).transpose(0, 2, 1)     # [NV, P, HCN]
    b3a = b3_all.reshape(NV, HCN, P).transpose(0, 2, 1)
    bq_ = np.ascontiguousarray(
        np.stack([b1a, b3a], axis=2)).astype(np.float32)    # [NV, P, 2, HCN]

    gwt = np.ascontiguousarray(
        gate_w.T.reshape(KD, P, E).transpose(1, 0, 2)).astype(np.float32)

    return dict(
        wq=wq_, w2q=w2q_, bq=bq_,
        b2e8=b2.astype(BF16),                               # [E, D]
        sb2x=(WS * sb2[None]).astype(BF16),                 # [1, D]
        gwt=gwt, gb=gate_b[None].astype(np.float32),
        ones1=np.ones((1, P), BF16),
        onesc=np.ones((P, 1), BF16),
        lt=np.triu(np.ones((P, P))).astype(BF16),
        ident=np.eye(P).astype(BF16),
        iota=np.tile(np.arange(cfg.capm, dtype=np.float32), (P, 1)),
    )


def pack_x(cfg: Cfg, x_tokens):
    """x_tokens [T, D] fp32 -> device layouts (gate fp32 + fp8 hi/lo)."""
    T, D = x_tokens.shape
    xT = np.ascontiguousarray(
        x_tokens.T.reshape(cfg.KD, P, T).transpose(1, 0, 2)).astype(np.float32)
    xh, xl = _split_fp8(xT)
    xtok = np.ascontiguousarray(
        x_tokens.reshape(cfg.TT, P, D).transpose(1, 0, 2))
    xth, xtl = _split_fp8(xtok)
    return dict(xT=xT, xtbh=xh, xtbl=xl, xtokh=xth, xtokl=xtl)


def unpack_y(cfg: Cfg, y_dev):
    """y device layout [P, TT, D] -> [T, D]."""
    return np.ascontiguousarray(y_dev.transpose(1, 0, 2).reshape(cfg.T, cfg.D))


_CACHE = {}


def _get_nc(cfg: Cfg):
    key = (cfg.D, cfg.H, cfg.E, cfg.n_sh, cfg.T, cfg.capm)
    if key not in _CACHE:
        _CACHE[key] = build_nc_fp8(cfg)
    return _CACHE[key]


def make_in_maps(cfg: Cfg, inputs):
    static = pack_static(
        cfg,
        np.asarray(inputs["gate_w"], np.float32), np.asarray(inputs["gate_b"], np.float32),
        np.asarray(inputs["w1"], np.float32), np.asarray(inputs["b1"], np.float32),
        np.asarray(inputs["w2"], np.float32), np.asarray(inputs["b2"], np.float32),
        np.asarray(inputs["w3"], np.float32), np.asarray(inputs["b3"], np.float32),
        np.asarray(inputs["sw1"], np.float32), np.asarray(inputs["sb1"], np.float32),
        np.asarray(inputs["sw2"], np.float32), np.asarray(inputs["sb2"], np.float32),
        np.asarray(inputs["sw3"], np.float32), np.asarray(inputs["sb3"], np.float32),
    )
    x = np.asarray(inputs["x"], np.float32)
    B, S, D = x.shape
    xf = x.reshape(-1, D)
    in_maps = []
    for c in range(cfg.n_cores):
        m = dict(static)
        m.update(pack_x(cfg, xf[c * cfg.T:(c + 1) * cfg.T]))
        in_maps.append(m)
    return in_maps


def kernel(**inputs) -> np.ndarray:
    x = np.asarray(inputs["x"], np.float32)
    B, S, D = x.shape
    N = B * S
    cfg = Cfg(D=D, T=N // 8, n_cores=8)
    nc = _get_nc(cfg)
    in_maps = make_in_maps(cfg, inputs)
    res = run_bass_kernel_spmd(nc, in_maps, list(range(cfg.n_cores)))
    outs = [unpack_y(cfg, res.results[c]["y"]) for c in range(cfg.n_cores)]
    return np.concatenate(outs, 0).reshape(B, S, D)


# revision 19
# speedup vs baseline: 1.2302x; 1.0423x over previous
"""MoE (8 routed experts, top-2, + shared expert) on 8 NeuronCores.

Data-parallel over tokens (1024/core), weights replicated, capacity-routed
dispatch (pair buckets, capm=96) as in the bf16 baseline — but all large
GEMMs run as fp8e4m3 DoubleRow matmuls with 3-term error compensation:

    A @ B  ~=  A_hi@B_hi + A_hi@B_lo + A_lo@B_hi

where X_hi = fp8(X), X_lo = fp8(X - X_hi).  DoubleRow consumes two
128-deep K-tiles per instruction at 0.5 cycles/row, so each compensated
GEMM costs 0.75x its bf16 schedule while adding only ~1e-3 relative
error.  Weights are pre-scaled by 64 on the host (fp8 subnormal cutoff),
descaled in the PSUM eviction.  The gate runs in fp32 so routing matches
the reference.

Schedule notes: w1/w3 hi+lo chunks ride one DMA per hc-pair (the SP
sequencer costs ~0.9us per DMA, so descriptor count is a real resource);
expert e+1's token gather is issued between expert e's L1 and L2 so its
PSUM evictions hide under L2 matmuls; the L1 eviction chain is
Silu -> scale -> mul with the fp8 split offloaded to the idle GPSIMD
engine; half of the shared expert runs after the combine phase so the
combine's DRAM round-trip sits under shared-expert matmuls, not at the
kernel tail.
"""

import numpy as np
import ml_dtypes

import concourse.bacc as bacc
import concourse.bass as bass
import concourse.tile as tile
import concourse.mybir as mybir
from concourse.bass_utils import run_bass_kernel_spmd

BF16 = ml_dtypes.bfloat16
FP8 = ml_dtypes.float8_e4m3
F32 = mybir.dt.float32
BF = mybir.dt.bfloat16
F8 = mybir.dt.float8e4
AF = mybir.ActivationFunctionType
OP = mybir.AluOpType
DR = mybir.MatmulPerfMode.DoubleRow

P = 128
WS = 64.0          # host-side weight scale before fp8 quantization
WSI = 1.0 / WS


class Cfg:
    def __init__(self, D=1024, H=2048, E=8, n_sh=2, T=1024, n_cores=8, capm=96):
        self.D, self.H, self.E, self.n_sh, self.T = D, H, E, n_sh, T
        self.NV = E + n_sh          # virtual experts
        self.HS = n_sh * H          # shared hidden
        self.KD = D // P            # K chunks over D
        self.HCN = H // P           # h chunks over H
        self.TT = T // P            # token 128-tiles
        self.DT = (D + 511) // 512  # output d 512-tiles
        self.FT = (T + 511) // 512  # layer-1 free 512-tiles
        self.n_cores = n_cores
        self.capm = capm            # per-(expert, tile-pair) dispatch capacity
        self.NP = self.TT // 2      # token-tile pairs
        self.CAPE = self.NP * capm  # slots per expert
        self.ST = (self.CAPE + P - 1) // P  # slot 128-tiles per expert


def build_nc_fp8(cfg: Cfg):
    D, H, E, NV, T = cfg.D, cfg.H, cfg.E, cfg.NV, cfg.T
    KD, HCN, TT, DT, FT = cfg.KD, cfg.HCN, cfg.TT, cfg.DT, cfg.FT
    capm, CAPE, ST, NP = cfg.capm, cfg.CAPE, cfg.ST, cfg.NP
    KD2, HCN2 = KD // 2, HCN // 2

    nc = bacc.Bacc("TRN2", target_bir_lowering=False)

    xT = nc.dram_tensor("xT", [P, KD, T], F32, kind="ExternalInput")
    xtbh = nc.dram_tensor("xtbh", [P, KD, T], F8, kind="ExternalInput")
    xtbl = nc.dram_tensor("xtbl", [P, KD, T], F8, kind="ExternalInput")
    xtokh = nc.dram_tensor("xtokh", [P, TT, D], F8, kind="ExternalInput")
    xtokl = nc.dram_tensor("xtokl", [P, TT, D], F8, kind="ExternalInput")
    # w1/w3 hi+lo packed per hc-pair: [e, hcp, p, i(2), which(4), KD, P]
    wq = nc.dram_tensor("wq", [NV, HCN2, P, 2, 4, KD, P], F8,
                        kind="ExternalInput")
    # w2 hi+lo packed per expert: [e, p, which(2), HCN, D]
    w2q = nc.dram_tensor("w2q", [NV, P, 2, HCN, D], F8, kind="ExternalInput")
    # b1, b3 packed per expert: [e, p, which(2), HCN]
    bq = nc.dram_tensor("bq", [NV, P, 2, HCN], F32, kind="ExternalInput")
    b2e8 = nc.dram_tensor("b2e8", [E, D], BF, kind="ExternalInput")
    sb2x = nc.dram_tensor("sb2x", [1, D], BF, kind="ExternalInput")
    gwt = nc.dram_tensor("gwt", [P, KD, E], F32, kind="ExternalInput")
    gb = nc.dram_tensor("gb", [1, E], F32, kind="ExternalInput")
    ones1 = nc.dram_tensor("ones1", [1, P], BF, kind="ExternalInput")
    onesc = nc.dram_tensor("onesc", [P, 1], BF, kind="ExternalInput")
    lt = nc.dram_tensor("lt", [P, P], BF, kind="ExternalInput")
    ident = nc.dram_tensor("ident", [P, P], BF, kind="ExternalInput")
    iota = nc.dram_tensor("iota", [P, capm], F32, kind="ExternalInput")
    y = nc.dram_tensor("y", [P, TT, D], F32, kind="ExternalOutput")

    OOB = 3.0e6

    with tile.TileContext(nc) as tc:
        with (
            tc.tile_pool(name="const1", bufs=1) as const1,
            tc.tile_pool(name="gchunk", bufs=1) as gchunk,
            tc.tile_pool(name="gtmp", bufs=4) as gtmp,
            tc.tile_pool(name="w1s", bufs=2) as w1s,
            tc.tile_pool(name="b13", bufs=2) as b13,
            tc.tile_pool(name="w2s", bufs=1) as w2s,
            tc.tile_pool(name="hpool", bufs=1) as hpool,
            tc.tile_pool(name="s1p", bufs=2) as s1p,
            tc.tile_pool(name="yebp", bufs=2) as yebp,
            tc.tile_pool(name="yea", bufs=1) as yea,
            tc.tile_pool(name="xep", bufs=2) as xep,
            tc.tile_pool(name="pep", bufs=8) as pep,
            tc.tile_pool(name="comb", bufs=2) as comb,
            tc.tile_pool(name="dram", bufs=1, space="DRAM") as drp,
            tc.tile_pool(name="ps_l1", bufs=2, space="PSUM") as ps_l1,
            tc.tile_pool(name="ps_y", bufs=2, space="PSUM") as ps_y,
            tc.tile_pool(name="ps_sm", bufs=2, space="PSUM") as ps_sm,
        ):
            # per-slot expert outputs, slot-major / expert-minor so combine
            # reads one contiguous [rows, e, D] block per bucket window
            ye = drp.tile([ST, P, E, D], BF)

            # ---- resident constants / state ----
            xbh = const1.tile([P, KD, T], F8)
            xbl = const1.tile([P, KD, T], F8)
            xth = const1.tile([P, TT, D], F8)
            xtl = const1.tile([P, TT, D], F8)
            yshared = const1.tile([P, TT, D], BF)
            cw = const1.tile([P, TT, E], F32)
            cwT = const1.tile([8, TT, P], BF)
            posb_all = const1.tile([P, TT, E], F32)
            ones_sb = const1.tile([1, P], BF)
            onesc_sb = const1.tile([P, 1], BF)
            gwt_sb = const1.tile([P, KD, E], F32)
            gb_sb = const1.tile([1, E], F32)
            zerob = const1.tile([P, 1], F32)
            onesf = const1.tile([1, P], F32)
            lt_sb = const1.tile([P, P], BF)
            id_sb = const1.tile([P, P], BF)
            iota_sb = const1.tile([P, capm], F32)
            b2e8_sb = const1.tile([8, D], BF)
            sb2x_sb = const1.tile([1, D], BF)

            nc.sync.dma_start(out=gwt_sb[:], in_=gwt[:])
            nc.sync.dma_start(out=gb_sb[:], in_=gb[:])
            nc.sync.dma_start(out=ones_sb[:], in_=ones1[:])
            nc.sync.dma_start(out=onesc_sb[:], in_=onesc[:])
            nc.sync.dma_start(out=lt_sb[:], in_=lt[:])
            nc.sync.dma_start(out=id_sb[:], in_=ident[:])
            nc.sync.dma_start(out=iota_sb[:], in_=iota[:])
            nc.sync.dma_start(out=b2e8_sb[:], in_=b2e8[:])
            nc.sync.dma_start(out=sb2x_sb[:], in_=sb2x[:])
            nc.vector.memset(zerob[:], 0.0)
            nc.vector.memset(onesf[:], 1.0)

            # prefetch the first shared-half L1 weight chunks so its matmuls
            # can start while the gate phase runs
            pre_w = {}
            for hcp in range(min(2, HCN2)):
                t = w1s.tile([P, 2, 4, KD, P], F8, name="wqt", tag="wqt")
                nc.sync.dma_start(out=t[:], in_=wq[E, hcp])
                pre_w[hcp] = t
            nc.sync.dma_start(out=xbh[:], in_=xtbh[:])
            nc.sync.dma_start(out=xbl[:], in_=xtbl[:])
            nc.sync.dma_start(out=xth[:], in_=xtokh[:])
            nc.sync.dma_start(out=xtl[:], in_=xtokl[:])

            # ---- gate + routing, per token tile (paired buckets) ----
            cntb = None

            def gate_tile(m):
                nonlocal cntb
                xchunk = gchunk.tile([P, KD, P], F32)
                nc.sync.dma_start(out=xchunk[:], in_=xT[:, :, m * P:(m + 1) * P])

                pg = ps_y.tile([P, P], F32, space="PSUM", name="pg", tag="yp")
                for k in range(KD):
                    nc.tensor.matmul(out=pg[:, :E], lhsT=xchunk[:, k, :],
                                     rhs=gwt_sb[:, k, :],
                                     start=(k == 0), stop=False)
                nc.tensor.matmul(out=pg[:, :E], lhsT=onesf[:], rhs=gb_sb[:],
                                 start=False, stop=True)

                lg = gtmp.tile([P, E], F32)
                nc.scalar.activation(lg[:], pg[:, :E], AF.Copy)
                m8 = gtmp.tile([P, 8], F32)
                nc.vector.max(m8[:], lg[:])
                ex = gtmp.tile([P, E], F32)
                nc.vector.tensor_scalar(out=ex[:], in0=lg[:],
                                        scalar1=m8[:, 0:1], scalar2=None,
                                        op0=OP.subtract)
                nc.scalar.activation(ex[:], ex[:], AF.Exp, bias=zerob[:])
                mask = gtmp.tile([P, E], F32)
                nc.vector.tensor_scalar(out=mask[:], in0=lg[:],
                                        scalar1=m8[:, 1:2], scalar2=None,
                                        op0=OP.is_ge)
                e2 = gtmp.tile([P, 1], F32)
                nc.vector.tensor_tensor(out=e2[:], in0=m8[:, 1:2],
                                        in1=m8[:, 0:1], op=OP.subtract)
                nc.scalar.activation(e2[:], e2[:], AF.Exp, bias=zerob[:])
                den = gtmp.tile([P, 1], F32)
                nc.vector.tensor_scalar(out=den[:], in0=e2[:], scalar1=1.0,
                                        scalar2=None, op0=OP.add)
                rec = gtmp.tile([P, 1], F32)
                nc.vector.reciprocal(rec[:], den[:])
                cwm = gtmp.tile([P, E], F32)
                nc.vector.tensor_mul(cwm[:], ex[:], mask[:])
                nc.vector.tensor_scalar(out=cw[:, m, :], in0=cwm[:],
                                        scalar1=rec[:, 0:1], scalar2=None,
                                        op0=OP.mult)
                # bf16 transpose of the combine weights for the b2 matmul
                cwb = gtmp.tile([P, E], BF, name="cwb")
                nc.vector.tensor_copy(cwb[:], cw[:, m, :])
                ptp = ps_sm.tile([P, P], BF, space="PSUM", name="ptp", tag="sm")
                nc.tensor.transpose(out=ptp[:E, :], in_=cwb[:], identity=id_sb[:])
                nc.scalar.activation(cwT[:, m, :], ptp[:E, :], AF.Copy)

                # bucket-local slot: pair prefix(mask) - mask; OOB unrouted
                maskb = gtmp.tile([P, E], BF)
                nc.vector.tensor_copy(maskb[:], mask[:])
                pp = ps_y.tile([P, P], F32, space="PSUM", name="pp", tag="yp")
                if m % 2 == 0:
                    nc.tensor.matmul(out=pp[:, :E], lhsT=lt_sb[:],
                                     rhs=maskb[:], start=True, stop=True)
                    cnt_ps = ps_sm.tile([1, P], F32, space="PSUM",
                                        name="cntp", tag="sm")
                    nc.tensor.matmul(out=cnt_ps[0:1, :E], lhsT=onesc_sb[:],
                                     rhs=maskb[:], start=True, stop=True)
                    cntb = gtmp.tile([1, E], BF, name="cntb")
                    nc.scalar.activation(cntb[:], cnt_ps[0:1, :E], AF.Copy)
                else:
                    nc.tensor.matmul(out=pp[:, :E], lhsT=lt_sb[:],
                                     rhs=maskb[:], start=True, stop=False)
                    nc.tensor.matmul(out=pp[:, :E], lhsT=ones_sb[:],
                                     rhs=cntb[:], start=False, stop=True)
                t1m = gtmp.tile([P, E], F32)
                nc.vector.scalar_tensor_tensor(out=t1m[:], in0=mask[:],
                                               scalar=-1.0, in1=pp[:, :E],
                                               op0=OP.mult, op1=OP.add)
                notm = gtmp.tile([P, E], F32)
                nc.vector.tensor_scalar(out=notm[:], in0=mask[:],
                                        scalar1=-1.0, scalar2=1.0,
                                        op0=OP.mult, op1=OP.add)
                nc.vector.scalar_tensor_tensor(out=posb_all[:, m, :],
                                               in0=notm[:], scalar=OOB,
                                               in1=t1m[:],
                                               op0=OP.mult, op1=OP.add)

            def dr3(out_ap, lh, ll, rh, rl, kn, fsl, sel=None):
                """3-term compensated fp8 DoubleRow accumulation group.

                lh/ll: either plain [P, KD, P] tiles or a packed wqt tile
                indexed via sel=(i, jh, jl).  k2-major so gather evictions
                unblock the group incrementally.
                """
                for k2 in range(kn):
                    ksl2 = slice(2 * k2, 2 * k2 + 2)
                    if sel is None:
                        lhs_h, lhs_l = lh[:, ksl2, :], ll[:, ksl2, :]
                    else:
                        i, jh, jl = sel
                        lhs_h = lh[:, i, jh, ksl2, :]
                        lhs_l = lh[:, i, jl, ksl2, :]
                    rhs_h = rh[:, ksl2, fsl] if fsl is not None else rh[:, ksl2]
                    rhs_l = rl[:, ksl2, fsl] if fsl is not None else rl[:, ksl2]
                    for ti, (lt_, rt_) in enumerate(
                            ((lhs_h, rhs_h), (lhs_h, rhs_l), (lhs_l, rhs_h))):
                        nc.tensor.matmul(
                            out=out_ap, lhsT=lt_, rhs=rt_,
                            start=(k2 == 0 and ti == 0),
                            stop=(k2 == kn - 1 and ti == 2),
                            perf_mode=DR)

            def l1_evict(o1, o3, b1c, b3c, hh, hl, hc, fsl, fw):
                """h = silu(o1/WS + b1) * (o3/WS + b3) -> fp8 hi/lo pair."""
                s = s1p.tile([P, 512], F32, name="s")
                nc.scalar.activation(s[:, :fw], o1[:, :fw], AF.Silu,
                                     bias=b1c, scale=WSI)
                v = s1p.tile([P, 512], F32, name="v")
                nc.vector.tensor_scalar(out=v[:, :fw], in0=o3[:, :fw],
                                        scalar1=WSI, scalar2=b3c,
                                        op0=OP.mult, op1=OP.add)
                hf = s1p.tile([P, 512], F32, name="hf")
                nc.vector.tensor_mul(hf[:, :fw], s[:, :fw], v[:, :fw])
                nc.scalar.activation(hh[:, hc, fsl], hf[:, :fw], AF.Copy)
                nc.gpsimd.tensor_sub(hl[:, hc, fsl], hf[:, :fw],
                                     hh[:, hc, fsl])

            def gather(e, xeh, xel):
                """One-hot dispatch + feature-major token gather, hi+lo."""
                pes = []
                for pr in range(NP):
                    pe2 = pep.tile([P, 2, capm], F8, name="pe2", tag="pe2")
                    for half in range(2):
                        nc.vector.tensor_scalar(
                            out=pe2[:, half, :], in0=iota_sb[:],
                            scalar1=posb_all[:, 2 * pr + half, e:e + 1],
                            scalar2=None, op0=OP.is_equal)
                    pes.append(pe2)
                for k in range(KD):
                    ksl = slice(k * P, (k + 1) * P)
                    for src, dst, nm in ((xth, xeh, "gxh"), (xtl, xel, "gxl")):
                        gx = ps_sm.tile([P, NP * capm], F32,
                                        space="PSUM", name=nm, tag="sm")
                        for pr in range(NP):
                            nc.tensor.matmul(
                                out=gx[:, pr * capm:(pr + 1) * capm],
                                lhsT=src[:, 2 * pr:2 * pr + 2, ksl],
                                rhs=pes[pr][:],
                                start=True, stop=True, perf_mode=DR)
                        nc.scalar.activation(dst[:, k, :], gx[:], AF.Copy)

            def l1_phase(e, is_shared, rh_h, rl_h, pre=None, interleave=None):
                bqt = b13.tile([P, 2, HCN], F32, name="bqt")
                nc.sync.dma_start(out=bqt[:], in_=bq[e])
                hh = hpool.tile([P, HCN, T], F8, name="hh", tag="hh")
                hl = hpool.tile([P, HCN, T], F8, name="hl", tag="hl")
                for hcp in range(HCN2):
                    if interleave is not None and hcp in interleave:
                        interleave[hcp]()
                    if pre is not None and hcp in pre:
                        wqt = pre[hcp]
                    else:
                        wqt = w1s.tile([P, 2, 4, KD, P], F8, name="wqt",
                                       tag="wqt")
                        nc.sync.dma_start(out=wqt[:], in_=wq[e, hcp])
                    for i in range(2):
                        hc = 2 * hcp + i
                        b1c = bqt[:, 0, hc:hc + 1]
                        b3c = bqt[:, 1, hc:hc + 1]
                        if is_shared:
                            for ft in range(FT):
                                fsl = slice(ft * 512, min((ft + 1) * 512, T))
                                fw = fsl.stop - fsl.start
                                o1 = ps_l1.tile([P, 512], F32, space="PSUM",
                                                name="o1")
                                dr3(o1[:, :fw], wqt, None, rh_h, rl_h,
                                    KD2, fsl, sel=(i, 0, 1))
                                o3 = ps_l1.tile([P, 512], F32, space="PSUM",
                                                name="o3")
                                dr3(o3[:, :fw], wqt, None, rh_h, rl_h,
                                    KD2, fsl, sel=(i, 2, 3))
                                l1_evict(o1, o3, b1c, b3c, hh, hl, hc, fsl, fw)
                        else:
                            fsl = slice(0, CAPE)
                            o1 = ps_l1.tile([P, 512], F32, space="PSUM",
                                            name="o1")
                            dr3(o1[:, :CAPE], wqt, None, rh_h, rl_h,
                                KD2, None, sel=(i, 0, 1))
                            o3 = ps_l1.tile([P, 512], F32, space="PSUM",
                                            name="o3")
                            dr3(o3[:, :CAPE], wqt, None, rh_h, rl_h,
                                KD2, None, sel=(i, 2, 3))
                            l1_evict(o1, o3, b1c, b3c, hh, hl, hc, fsl, CAPE)
                w2qt = w2s.tile([P, 2, HCN, D], F8, name="w2qt")
                nc.sync.dma_start(out=w2qt[:], in_=w2q[e])
                return hh, hl, w2qt

            def l2_matmuls(yp, hh, hl, w2qt, tsl, dsl, dw, sw, first_start):
                for h2 in range(HCN2):
                    hsl = slice(2 * h2, 2 * h2 + 2)
                    for ti, (lt_, rt_) in enumerate((
                            (hh[:, hsl, tsl], w2qt[:, 0, hsl, dsl]),
                            (hl[:, hsl, tsl], w2qt[:, 0, hsl, dsl]),
                            (hh[:, hsl, tsl], w2qt[:, 1, hsl, dsl]))):
                        nc.tensor.matmul(
                            out=yp[:sw, :dw], lhsT=lt_, rhs=rt_,
                            start=(first_start and h2 == 0 and ti == 0),
                            stop=(h2 == HCN2 - 1 and ti == 2),
                            perf_mode=DR)

            # ---- gate (first two tiles), then shared-half L1 with the
            # remaining gate tiles interleaved so the PE never waits on the
            # fp32 x stream ----
            gate_tile(0)
            gate_tile(1)
            hh, hl, w2qt = l1_phase(
                E, True, xbh, xbl, pre=pre_w,
                interleave={1: lambda: [gate_tile(m) for m in (2, 3)],
                            2: lambda: [gate_tile(m) for m in (4, 5)],
                            3: lambda: [gate_tile(m) for m in (6, 7)]})
            # expert 0's gather hides under the shared L2 matmuls
            xeh = xep.tile([P, KD, CAPE], F8, name="xeh", tag="xeh")
            xel = xep.tile([P, KD, CAPE], F8, name="xel", tag="xel")
            gather(0, xeh, xel)
            for tt in range(TT):
                tsl = slice(tt * P, (tt + 1) * P)
                for dt in range(DT):
                    dsl = slice(dt * 512, min((dt + 1) * 512, D))
                    dw = dsl.stop - dsl.start
                    yp = ps_y.tile([P, 512], F32, space="PSUM", name="yp")
                    nc.tensor.matmul(out=yp[:, :dw], lhsT=ones_sb[:],
                                     rhs=sb2x_sb[0:1, dsl],
                                     start=True, stop=False)
                    l2_matmuls(yp, hh, hl, w2qt, tsl, dsl, dw, P, False)
                    nc.scalar.activation(yshared[:, tt, dsl], yp[:, :dw],
                                         AF.Copy, scale=WSI)

            # ---- routed experts over dispatched slots ----
            for e in range(E):
                hh, hl, w2qt = l1_phase(e, False, xeh, xel)
                if e < E - 1:
                    xeh = xep.tile([P, KD, CAPE], F8, name="xeh", tag="xeh")
                    xel = xep.tile([P, KD, CAPE], F8, name="xel", tag="xel")
                    gather(e + 1, xeh, xel)
                yebA = yea.tile([P, ST, D], BF, name="yebA")
                for st in range(ST):
                    sw = min(P, CAPE - st * P)
                    ssl = slice(st * P, st * P + sw)
                    for dt in range(DT):
                        dsl = slice(dt * 512, min((dt + 1) * 512, D))
                        dw = dsl.stop - dsl.start
                        yp = ps_y.tile([P, 512], F32, space="PSUM", name="yp")
                        l2_matmuls(yp, hh, hl, w2qt, ssl, dsl, dw, sw, True)
                        nc.scalar.activation(yebA[:sw, st, dsl], yp[:sw, :dw],
                                             AF.Copy, scale=WSI)
                for st in range(ST):
                    nc.scalar.dma_start(out=ye[st, :, e, :], in_=yebA[:, st, :])

            # prefetch the second shared-half's first L1 chunks so its
            # matmuls start right after the combine drains
            pre_w2 = {}
            for hcp in range(min(1, HCN2)):
                t = w1s.tile([P, 2, 4, KD, P], F8, name="wqt", tag="wqt")
                nc.sync.dma_start(out=t[:], in_=wq[E + 1, hcp])
                pre_w2[hcp] = t

            # ---- combine: yshared += b2@cw + sum_e cw_e * ye[slot] ----
            for m in range(TT):
                pr = m // 2
                yps = []
                pool_c, tag_c = (ps_y, "yp") if m % 2 == 0 else (ps_l1, "o1")
                for dt in range(DT):
                    yps.append(pool_c.tile([P, 512], F32, space="PSUM",
                                           name=f"ypc{dt}", tag=tag_c))
                # bucket window rows (pr*capm .. pr*capm+capm) as (st, p)
                # pieces; one DMA per piece covers 4 experts per half-tile
                halves = []
                for hf4 in range(2):
                    yeb_sb = yebp.tile([capm, E // 2, D], BF, name="yebh")
                    r0 = pr * capm
                    got = 0
                    while got < capm:
                        st0, p0 = divmod(r0 + got, P)
                        ln = min(P - p0, capm - got)
                        nc.sync.dma_start(
                            out=yeb_sb[got:got + ln, :, :],
                            in_=ye[st0, p0:p0 + ln,
                                   hf4 * (E // 2):(hf4 + 1) * (E // 2), :])
                        got += ln
                    halves.append(yeb_sb)
                for dt in range(DT):
                    dsl = slice(dt * 512, min((dt + 1) * 512, D))
                    dw = dsl.stop - dsl.start
                    nc.tensor.matmul(out=yps[dt][:, :dw],
                                     lhsT=cwT[:, m, :], rhs=b2e8_sb[:, dsl],
                                     start=True, stop=False)
                for e in range(E):
                    pe = gtmp.tile([P, capm], BF, name="pe")
                    nc.vector.tensor_scalar(out=pe[:], in0=iota_sb[:],
                                            scalar1=posb_all[:, m, e:e + 1],
                                            scalar2=None, op0=OP.is_equal)
                    pew = gtmp.tile([P, capm], BF, name="pew")
                    nc.vector.tensor_scalar(out=pew[:], in0=pe[:],
                                            scalar1=cw[:, m, e:e + 1],
                                            scalar2=None, op0=OP.mult)
                    pool_t, tag_t = (ps_sm, "sm") if e % 2 == 0 else (ps_l1, "o3")
                    p2 = pool_t.tile([P, P], BF, space="PSUM",
                                     name="p2", tag=tag_t)
                    nc.tensor.transpose(out=p2[:capm, :], in_=pew[:],
                                        identity=id_sb[:])
                    p2s = gtmp.tile([capm, P], BF, name="p2s")
                    nc.scalar.activation(p2s[:], p2[:capm, :], AF.Copy)
                    for dt in range(DT):
                        dsl = slice(dt * 512, min((dt + 1) * 512, D))
                        dw = dsl.stop - dsl.start
                        nc.tensor.matmul(out=yps[dt][:, :dw], lhsT=p2s[:],
                                         rhs=halves[e // 4][:, e % 4, dsl],
                                         start=False, stop=(e == E - 1))
                for dt in range(DT):
                    dsl = slice(dt * 512, min((dt + 1) * 512, D))
                    dw = dsl.stop - dsl.start
                    nc.vector.tensor_add(yshared[:, m, dsl],
                                         yshared[:, m, dsl],
                                         yps[dt][:, :dw])

            # ---- shared expert, second half; emits final y ----
            hh, hl, w2qt = l1_phase(E + 1, True, xbh, xbl, pre=pre_w2)
            for tt in range(TT):
                tsl = slice(tt * P, (tt + 1) * P)
                for dt in range(DT):
                    dsl = slice(dt * 512, min((dt + 1) * 512, D))
                    dw = dsl.stop - dsl.start
                    yp = ps_y.tile([P, 512], F32, space="PSUM", name="yp")
                    l2_matmuls(yp, hh, hl, w2qt, tsl, dsl, dw, P, True)
                    yt = comb.tile([P, 512], F32, name="yt")
                    nc.vector.scalar_tensor_tensor(
                        out=yt[:, :dw], in0=yp[:, :dw], scalar=WSI,
                        in1=yshared[:, tt, dsl], op0=OP.mult, op1=OP.add)
                    nc.scalar.dma_start(out=y[:, tt, dsl], in_=yt[:, :dw])

    nc.compile()
    return nc


# ---------------- host-side packing ----------------

def _split_fp8(a, scale=1.0):
    a = np.asarray(a, np.float32) * scale
    hi = a.astype(FP8)
    lo = (a - hi.astype(np.float32)).astype(FP8)
    return hi, lo


def pack_static(cfg: Cfg, gate_w, gate_b, w1, b1, w2, b2, w3, b3,
                sw1, sb1, sw2, sb2, sw3, sb3):
    D, H, E, NV, n_sh = cfg.D, cfg.H, cfg.E, cfg.NV, cfg.n_sh
    KD, HCN = cfg.KD, cfg.HCN
    HCN2 = HCN // 2

    w1T = np.transpose(w1, (0, 2, 1))                      # [E, D, H]
    w3T = np.transpose(w3, (0, 2, 1))
    w2T = np.transpose(w2, (0, 2, 1))                      # [E, H, D]
    s1T = sw1.T.reshape(D, n_sh, H).transpose(1, 0, 2)     # [n_sh, D, H]
    s3T = sw3.T.reshape(D, n_sh, H).transpose(1, 0, 2)
    s2T = sw2.T.reshape(n_sh, H, D)                        # [n_sh, H, D]
    w1T_all = np.concatenate([w1T, s1T], 0)                # [NV, D, H]
    w3T_all = np.concatenate([w3T, s3T], 0)
    w2T_all = np.concatenate([w2T, s2T], 0)                # [NV, H, D]

    w1t = np.ascontiguousarray(
        w1T_all.reshape(NV, KD, P, HCN, P).transpose(0, 3, 2, 1, 4))
    w3t = np.ascontiguousarray(
        w3T_all.reshape(NV, KD, P, HCN, P).transpose(0, 3, 2, 1, 4))
    w2t = np.ascontiguousarray(
        w2T_all.reshape(NV, HCN, P, D).transpose(0, 2, 1, 3))
    w1h_, w1l_ = _split_fp8(w1t, WS)
    w3h_, w3l_ = _split_fp8(w3t, WS)
    w2h_, w2l_ = _split_fp8(w2t, WS)

    # wq: [NV, HCN2, P, 2, 4, KD, P]
    wq_ = np.stack([w1h_, w1l_, w3h_, w3l_], axis=2)   # [NV, HCN, 4, P, KD, P]
    wq_ = wq_.reshape(NV, HCN2, 2, 4, P, KD, P).transpose(0, 1, 4, 2, 3, 5, 6)
    wq_ = np.ascontiguousarray(wq_)

    # w2q: [NV, P, 2, HCN, D]
    w2q_ = np.ascontiguousarray(
        np.stack([w2h_, w2l_], axis=1).transpose(0, 2, 1, 3, 4))

    b1_all = np.concatenate([b1, sb1.reshape(n_sh, H)], 0)  # [NV, H]
    b3_all = np.concatenate([b3, sb3.reshape(n_sh, H)], 0)
    b1a = b1_all.reshape(NV, HCN, P).transpose(0, 2, 1)     # [NV, P, HCN]
    b3a = b3_all.reshape(NV, HCN, P).transpose(0, 2, 1)
    bq_ = np.ascontiguousarray(
        np.stack([b1a, b3a], axis=2)).astype(np.float32)    # [NV, P, 2, HCN]

    gwt = np.ascontiguousarray(
        gate_w.T.reshape(KD, P, E).transpose(1, 0, 2)).astype(np.float32)

    return dict(
        wq=wq_, w2q=w2q_, bq=bq_,
        b2e8=b2.astype(BF16),                               # [E, D]
        sb2x=(WS * sb2[None]).astype(BF16),                 # [1, D]
        gwt=gwt, gb=gate_b[None].astype(np.float32),
        ones1=np.ones((1, P), BF16),
        onesc=np.ones((P, 1), BF16),
        lt=np.triu(np.ones((P, P))).astype(BF16),
        ident=np.eye(P).astype(BF16),
        iota=np.tile(np.arange(cfg.capm, dtype=np.float32), (P, 1)),
    )


def pack_x(cfg: Cfg, x_tokens):
    """x_tokens [T, D] fp32 -> device layouts (gate fp32 + fp8 hi/lo)."""
    T, D = x_tokens.shape
    xT = np.ascontiguousarray(
        x_tokens.T.reshape(cfg.KD, P, T).transpose(1, 0, 2)).astype(np.float32)
    xh, xl = _split_fp8(xT)
    xtok = np.ascontiguousarray(
        x_tokens.reshape(cfg.TT, P, D).transpose(1, 0, 2))
    xth, xtl = _split_fp8(xtok)
    return dict(xT=xT, xtbh=xh, xtbl=xl, xtokh=xth, xtokl=xtl)


def unpack_y(cfg: Cfg, y_dev):
    """y device layout [P, TT, D] -> [T, D]."""
    return np.ascontiguousarray(y_dev.transpose(1, 0, 2).reshape(cfg.T, cfg.D))


_CACHE = {}


def _get_nc(cfg: Cfg):
    key = (cfg.D, cfg.H, cfg.E, cfg.n_sh, cfg.T, cfg.capm)
    if key not in _CACHE:
        _CACHE[key] = build_nc_fp8(cfg)
    return _CACHE[key]


def make_in_maps(cfg: Cfg, inputs):
    static = pack_static(
        cfg,
        np.asarray(inputs["gate_w"], np.float32), np.asarray(inputs["gate_b"], np.float32),
        np.asarray(inputs["w1"], np.float32), np.asarray(inputs["b1"], np.float32),
        np.asarray(inputs["w2"], np.float32), np.asarray(inputs["b2"], np.float32),
        np.asarray(inputs["w3"], np.float32), np.asarray(inputs["b3"], np.float32),
        np.asarray(inputs["sw1"], np.float32), np.asarray(inputs["sb1"], np.float32),
        np.asarray(inputs["sw2"], np.float32), np.asarray(inputs["sb2"], np.float32),
        np.asarray(inputs["sw3"], np.float32), np.asarray(inputs["sb3"], np.float32),
    )
    x = np.asarray(inputs["x"], np.float32)
    B, S, D = x.shape
    xf = x.reshape(-1, D)
    in_maps = []
    for c in range(cfg.n_cores):
        m = dict(static)
        m.update(pack_x(cfg, xf[c * cfg.T:(c + 1) * cfg.T]))
        in_maps.append(m)
    return in_maps


def kernel(**inputs) -> np.ndarray:
    x = np.asarray(inputs["x"], np.float32)
    B, S, D = x.shape
    N = B * S
    cfg = Cfg(D=D, T=N // 8, n_cores=8)
    nc = _get_nc(cfg)
    in_maps = make_in_maps(cfg, inputs)
    res = run_bass_kernel_spmd(nc, in_maps, list(range(cfg.n_cores)))
    outs = [unpack_y(cfg, res.results[c]["y"]) for c in range(cfg.n_cores)]
    return np.concatenate(outs, 0).reshape(B, S, D)


# revision 31
# speedup vs baseline: 1.2918x; 1.0500x over previous
"""MoE (8 routed experts, top-2, + shared expert) on 8 NeuronCores.

Data-parallel over tokens (1024/core), weights replicated, capacity-routed
dispatch (pair buckets, capm=96) as in the bf16 baseline — but all large
GEMMs run as fp8e4m3 DoubleRow matmuls with 3-term error compensation:

    A @ B  ~=  A_hi@B_hi + A_hi@B_lo + A_lo@B_hi

where X_hi = fp8(X), X_lo = fp8(X - X_hi).  DoubleRow consumes two
128-deep K-tiles per instruction at 0.5 cycles/row, so each compensated
GEMM costs 0.75x its bf16 schedule while adding only ~1e-3 relative
error.  Weights are pre-scaled by 64 on the host (fp8 subnormal cutoff),
descaled in the PSUM eviction.  The gate runs in fp32 so routing matches
the reference.

Schedule notes: w1/w3 hi+lo chunks ride one DMA per hc-pair (the SP
sequencer costs ~0.9us per DMA, so descriptor count is a real resource);
expert e+1's token gather is issued between expert e's L1 and L2 so its
PSUM evictions hide under L2 matmuls; the L1 eviction chain is
Silu -> scale -> mul with the fp8 split offloaded to the idle GPSIMD
engine; half of the shared expert runs after the combine phase so the
combine's DRAM round-trip sits under shared-expert matmuls, not at the
kernel tail.
"""

import numpy as np
import ml_dtypes

import concourse.bacc as bacc
import concourse.bass as bass
import concourse.tile as tile
import concourse.mybir as mybir
from concourse.bass_utils import run_bass_kernel_spmd

BF16 = ml_dtypes.bfloat16
FP8 = ml_dtypes.float8_e4m3
F32 = mybir.dt.float32
BF = mybir.dt.bfloat16
F8 = mybir.dt.float8e4
AF = mybir.ActivationFunctionType
OP = mybir.AluOpType
DR = mybir.MatmulPerfMode.DoubleRow

P = 128
WS = 64.0          # host-side weight scale before fp8 quantization
WSI = 1.0 / WS


class Cfg:
    def __init__(self, D=1024, H=2048, E=8, n_sh=2, T=1024, n_cores=8, capm=96):
        self.D, self.H, self.E, self.n_sh, self.T = D, H, E, n_sh, T
        self.NV = E + n_sh          # virtual experts
        self.HS = n_sh * H          # shared hidden
        self.KD = D // P            # K chunks over D
        self.HCN = H // P           # h chunks over H
        self.TT = T // P            # token 128-tiles
        self.DT = (D + 511) // 512  # output d 512-tiles
        self.FT = (T + 511) // 512  # layer-1 free 512-tiles
        self.n_cores = n_cores
        self.capm = capm            # per-(expert, tile-pair) dispatch capacity
        self.NP = self.TT // 2      # token-tile pairs
        self.CAPE = self.NP * capm  # slots per expert
        self.ST = (self.CAPE + P - 1) // P  # slot 128-tiles per expert


def build_nc_fp8(cfg: Cfg):
    D, H, E, NV, T = cfg.D, cfg.H, cfg.E, cfg.NV, cfg.T
    KD, HCN, TT, DT, FT = cfg.KD, cfg.HCN, cfg.TT, cfg.DT, cfg.FT
    capm, CAPE, ST, NP = cfg.capm, cfg.CAPE, cfg.ST, cfg.NP
    KD2, HCN2 = KD // 2, HCN // 2

    nc = bacc.Bacc("TRN2", target_bir_lowering=False)

    xT = nc.dram_tensor("xT", [P, KD, T], F32, kind="ExternalInput")
    xtbh = nc.dram_tensor("xtbh", [P, KD, T], F8, kind="ExternalInput")
    xtbl = nc.dram_tensor("xtbl", [P, KD, T], F8, kind="ExternalInput")
    xtokh = nc.dram_tensor("xtokh", [P, TT, D], F8, kind="ExternalInput")
    xtokl = nc.dram_tensor("xtokl", [P, TT, D], F8, kind="ExternalInput")
    # w1/w3 hi+lo packed per hc-pair: [e, hcp, p, i(2), which(4), KD, P]
    wq = nc.dram_tensor("wq", [NV, HCN2, P, 2, 4, KD, P], F8,
                        kind="ExternalInput")
    # w2 hi+lo packed per expert: [e, p, which(2), HCN, D]
    w2q = nc.dram_tensor("w2q", [NV, P, 2, HCN, D], F8, kind="ExternalInput")
    # b1, b3 packed per expert: [e, p, which(2), HCN]
    bq = nc.dram_tensor("bq", [NV, P, 2, HCN], F32, kind="ExternalInput")
    b2e8 = nc.dram_tensor("b2e8", [E, D], BF, kind="ExternalInput")
    sb2x = nc.dram_tensor("sb2x", [1, D], BF, kind="ExternalInput")
    gwt = nc.dram_tensor("gwt", [P, KD, E], F32, kind="ExternalInput")
    gb = nc.dram_tensor("gb", [1, E], F32, kind="ExternalInput")
    ones1 = nc.dram_tensor("ones1", [1, P], BF, kind="ExternalInput")
    onesc = nc.dram_tensor("onesc", [P, 1], BF, kind="ExternalInput")
    lt = nc.dram_tensor("lt", [P, P], BF, kind="ExternalInput")
    ident = nc.dram_tensor("ident", [P, P], BF, kind="ExternalInput")
    iota = nc.dram_tensor("iota", [P, P], F32, kind="ExternalInput")
    y = nc.dram_tensor("y", [P, TT, D], F32, kind="ExternalOutput")

    OOB = 3.0e6

    with tile.TileContext(nc) as tc:
        with (
            tc.tile_pool(name="const1", bufs=1) as const1,
            tc.tile_pool(name="gchunk", bufs=2) as gchunk,
            tc.tile_pool(name="gtmp", bufs=4) as gtmp,
            tc.tile_pool(name="w1s", bufs=2) as w1s,
            tc.tile_pool(name="b13", bufs=2) as b13,
            tc.tile_pool(name="w2s", bufs=1) as w2s,
            tc.tile_pool(name="hpool", bufs=1) as hpool,
            tc.tile_pool(name="s1p", bufs=2) as s1p,
            tc.tile_pool(name="yea", bufs=2) as yea,
            tc.tile_pool(name="xep", bufs=2) as xep,
            tc.tile_pool(name="pep", bufs=8) as pep,
            tc.tile_pool(name="comb", bufs=2) as comb,
            tc.tile_pool(name="ps_l1", bufs=2, space="PSUM") as ps_l1,
            tc.tile_pool(name="ps_y", bufs=2, space="PSUM") as ps_y,
            tc.tile_pool(name="ps_sm", bufs=2, space="PSUM") as ps_sm,
        ):
            # ---- resident constants / state ----
            xbh = const1.tile([P, KD, T], F8)
            xbl = const1.tile([P, KD, T], F8)
            xth = const1.tile([P, TT, D], F8)
            xtl = const1.tile([P, TT, D], F8)
            yshared = const1.tile([P, TT, D], BF)
            cw = const1.tile([P, TT, E], F32)
            cwT = const1.tile([8, TT, P], BF)
            posb_all = const1.tile([P, TT, E], F32)
            ones_sb = const1.tile([1, P], BF)
            onesc_sb = const1.tile([P, 1], BF)
            gwt_sb = const1.tile([P, KD, E], F32)
            gb_sb = const1.tile([1, E], F32)
            zerob = const1.tile([P, 1], F32)
            onesf = const1.tile([1, P], F32)
            lt_sb = const1.tile([P, P], BF)
            id_sb = const1.tile([P, P], BF)
            iota_sb = const1.tile([P, P], F32)
            b2e8_sb = const1.tile([8, D], BF)
            sb2x_sb = const1.tile([1, D], BF)

            nc.sync.dma_start(out=gwt_sb[:], in_=gwt[:])
            nc.sync.dma_start(out=gb_sb[:], in_=gb[:])
            nc.sync.dma_start(out=ones_sb[:], in_=ones1[:])
            nc.sync.dma_start(out=onesc_sb[:], in_=onesc[:])
            nc.sync.dma_start(out=lt_sb[:], in_=lt[:])
            nc.sync.dma_start(out=id_sb[:], in_=ident[:])
            nc.sync.dma_start(out=iota_sb[:], in_=iota[:])
            nc.sync.dma_start(out=b2e8_sb[:], in_=b2e8[:])
            nc.sync.dma_start(out=sb2x_sb[:], in_=sb2x[:])
            nc.vector.memset(zerob[:], 0.0)
            nc.vector.memset(onesf[:], 1.0)

            # the gate's first two x chunks go out first so the PE starts
            # within ~2us; the big weight/x streams queue up behind them
            pre_xc = {}
            for m in range(2):
                xchunk = gchunk.tile([P, KD, P], F32)
                nc.sync.dma_start(out=xchunk[:],
                                  in_=xT[:, :, m * P:(m + 1) * P])
                pre_xc[m] = xchunk
            # prefetch the first shared-half L1 weight chunks so its matmuls
            # can start while the gate phase runs
            pre_w = {}
            for hcp in range(min(2, HCN2)):
                t = w1s.tile([P, 2, 4, KD, P], F8, name="wqt", tag="wqt")
                nc.sync.dma_start(out=t[:], in_=wq[E, hcp])
                pre_w[hcp] = t
            nc.sync.dma_start(out=xbh[:], in_=xtbh[:])
            nc.sync.dma_start(out=xbl[:], in_=xtbl[:])

            # ---- gate + routing, per token tile (paired buckets) ----
            cntb = None

            def gate_tile(m):
                nonlocal cntb
                if m in pre_xc:
                    xchunk = pre_xc[m]
                else:
                    xchunk = gchunk.tile([P, KD, P], F32)
                    nc.sync.dma_start(out=xchunk[:],
                                      in_=xT[:, :, m * P:(m + 1) * P])

                pg = ps_y.tile([P, P], F32, space="PSUM", name="pg", tag="yp")
                for k in range(KD):
                    nc.tensor.matmul(out=pg[:, :E], lhsT=xchunk[:, k, :],
                                     rhs=gwt_sb[:, k, :],
                                     start=(k == 0), stop=False)
                nc.tensor.matmul(out=pg[:, :E], lhsT=onesf[:], rhs=gb_sb[:],
                                 start=False, stop=True)

                lg = gtmp.tile([P, E], F32)
                nc.scalar.activation(lg[:], pg[:, :E], AF.Copy)
                m8 = gtmp.tile([P, 8], F32)
                nc.vector.max(m8[:], lg[:])
                ex = gtmp.tile([P, E], F32)
                nc.vector.tensor_scalar(out=ex[:], in0=lg[:],
                                        scalar1=m8[:, 0:1], scalar2=None,
                                        op0=OP.subtract)
                nc.scalar.activation(ex[:], ex[:], AF.Exp, bias=zerob[:])
                mask = gtmp.tile([P, E], F32)
                nc.vector.tensor_scalar(out=mask[:], in0=lg[:],
                                        scalar1=m8[:, 1:2], scalar2=None,
                                        op0=OP.is_ge)
                e2 = gtmp.tile([P, 1], F32)
                nc.vector.tensor_tensor(out=e2[:], in0=m8[:, 1:2],
                                        in1=m8[:, 0:1], op=OP.subtract)
                nc.scalar.activation(e2[:], e2[:], AF.Exp, bias=zerob[:])
                den = gtmp.tile([P, 1], F32)
                nc.vector.tensor_scalar(out=den[:], in0=e2[:], scalar1=1.0,
                                        scalar2=None, op0=OP.add)
                rec = gtmp.tile([P, 1], F32)
                nc.vector.reciprocal(rec[:], den[:])
                cwm = gtmp.tile([P, E], F32)
                nc.vector.tensor_mul(cwm[:], ex[:], mask[:])
                nc.vector.tensor_scalar(out=cw[:, m, :], in0=cwm[:],
                                        scalar1=rec[:, 0:1], scalar2=None,
                                        op0=OP.mult)
                # bf16 transpose of the combine weights for the b2 matmul
                cwb = gtmp.tile([P, E], BF, name="cwb")
                nc.vector.tensor_copy(cwb[:], cw[:, m, :])
                ptp = ps_sm.tile([P, P], BF, space="PSUM", name="ptp", tag="sm")
                nc.tensor.transpose(out=ptp[:E, :], in_=cwb[:], identity=id_sb[:])
                nc.scalar.activation(cwT[:, m, :], ptp[:E, :], AF.Copy)

                # bucket-local slot: pair prefix(mask) - mask; OOB unrouted
                maskb = gtmp.tile([P, E], BF)
                nc.vector.tensor_copy(maskb[:], mask[:])
                pp = ps_y.tile([P, P], F32, space="PSUM", name="pp", tag="yp")
                if m % 2 == 0:
                    nc.tensor.matmul(out=pp[:, :E], lhsT=lt_sb[:],
                                     rhs=maskb[:], start=True, stop=True)
                    cnt_ps = ps_sm.tile([1, P], F32, space="PSUM",
                                        name="cntp", tag="sm")
                    nc.tensor.matmul(out=cnt_ps[0:1, :E], lhsT=onesc_sb[:],
                                     rhs=maskb[:], start=True, stop=True)
                    cntb = gtmp.tile([1, E], BF, name="cntb")
                    nc.scalar.activation(cntb[:], cnt_ps[0:1, :E], AF.Copy)
                else:
                    nc.tensor.matmul(out=pp[:, :E], lhsT=lt_sb[:],
                                     rhs=maskb[:], start=True, stop=False)
                    nc.tensor.matmul(out=pp[:, :E], lhsT=ones_sb[:],
                                     rhs=cntb[:], start=False, stop=True)
                t1m = gtmp.tile([P, E], F32)
                nc.vector.scalar_tensor_tensor(out=t1m[:], in0=mask[:],
                                               scalar=-1.0, in1=pp[:, :E],
                                               op0=OP.mult, op1=OP.add)
                notm = gtmp.tile([P, E], F32)
                nc.vector.tensor_scalar(out=notm[:], in0=mask[:],
                                        scalar1=-1.0, scalar2=1.0,
                                        op0=OP.mult, op1=OP.add)
                nc.vector.scalar_tensor_tensor(out=posb_all[:, m, :],
                                               in0=notm[:], scalar=OOB,
                                               in1=t1m[:],
                                               op0=OP.mult, op1=OP.add)

            def dr3(out_ap, lh, ll, rh, rl, kn, fsl, sel=None):
                """3-term compensated fp8 DoubleRow accumulation group.

                lh/ll: either plain [P, KD, P] tiles or a packed wqt tile
                indexed via sel=(i, jh, jl).  k2-major so gather evictions
                unblock the group incrementally.
                """
                for k2 in range(kn):
                    ksl2 = slice(2 * k2, 2 * k2 + 2)
                    if sel is None:
                        lhs_h, lhs_l = lh[:, ksl2, :], ll[:, ksl2, :]
                    else:
                        i, jh, jl = sel
                        lhs_h = lh[:, i, jh, ksl2, :]
                        lhs_l = lh[:, i, jl, ksl2, :]
                    rhs_h = rh[:, ksl2, fsl] if fsl is not None else rh[:, ksl2]
                    rhs_l = rl[:, ksl2, fsl] if fsl is not None else rl[:, ksl2]
                    for ti, (lt_, rt_) in enumerate(
                            ((lhs_h, rhs_h), (lhs_h, rhs_l), (lhs_l, rhs_h))):
                        nc.tensor.matmul(
                            out=out_ap, lhsT=lt_, rhs=rt_,
                            start=(k2 == 0 and ti == 0),
                            stop=(k2 == kn - 1 and ti == 2),
                            perf_mode=DR)

            def l1_evict(o1, o3, b1c, b3c, hh, hl, hc, fsl, fw):
                """h = silu(o1/WS + b1) * (o3/WS + b3) -> fp8 hi/lo pair."""
                s = s1p.tile([P, 512], F32, name="s")
                nc.scalar.activation(s[:, :fw], o1[:, :fw], AF.Silu,
                                     bias=b1c, scale=WSI)
                v = s1p.tile([P, 512], F32, name="v")
                nc.vector.tensor_scalar(out=v[:, :fw], in0=o3[:, :fw],
                                        scalar1=WSI, scalar2=b3c,
                                        op0=OP.mult, op1=OP.add)
                hf = s1p.tile([P, 512], F32, name="hf")
                nc.vector.tensor_mul(hf[:, :fw], s[:, :fw], v[:, :fw])
                nc.scalar.activation(hh[:, hc, fsl], hf[:, :fw], AF.Copy)
                nc.gpsimd.tensor_sub(hl[:, hc, fsl], hf[:, :fw],
                                     hh[:, hc, fsl])

            def gather(e, xeh, xel):
                """One-hot dispatch + feature-major token gather, hi+lo."""
                pes = []
                for pr in range(NP):
                    pe2 = pep.tile([P, 2, capm], F8, name="pe2", tag="pe2")
                    for half in range(2):
                        nc.vector.tensor_scalar(
                            out=pe2[:, half, :], in0=iota_sb[:, :capm],
                            scalar1=posb_all[:, 2 * pr + half, e:e + 1],
                            scalar2=None, op0=OP.is_equal)
                    pes.append(pe2)
                for k in range(KD):
                    ksl = slice(k * P, (k + 1) * P)
                    for src, dst, nm in ((xth, xeh, "gxh"), (xtl, xel, "gxl")):
                        gx = ps_sm.tile([P, NP * capm], F32,
                                        space="PSUM", name=nm, tag="sm")
                        for pr in range(NP):
                            nc.tensor.matmul(
                                out=gx[:, pr * capm:(pr + 1) * capm],
                                lhsT=src[:, 2 * pr:2 * pr + 2, ksl],
                                rhs=pes[pr][:],
                                start=True, stop=True, perf_mode=DR)
                        nc.scalar.activation(dst[:, k, :], gx[:], AF.Copy)

            def l1_phase(e, is_shared, rh_h, rl_h, pre=None, interleave=None):
                bqt = b13.tile([P, 2, HCN], F32, name="bqt")
                nc.sync.dma_start(out=bqt[:], in_=bq[e])
                hh = hpool.tile([P, HCN, T], F8, name="hh", tag="hh")
                hl = hpool.tile([P, HCN, T], F8, name="hl", tag="hl")
                for hcp in range(HCN2):
                    if interleave is not None and hcp in interleave:
                        interleave[hcp]()
                    if pre is not None and hcp in pre:
                        wqt = pre[hcp]
                    else:
                        wqt = w1s.tile([P, 2, 4, KD, P], F8, name="wqt",
                                       tag="wqt")
                        nc.sync.dma_start(out=wqt[:], in_=wq[e, hcp])
                    for i in range(2):
                        hc = 2 * hcp + i
                        b1c = bqt[:, 0, hc:hc + 1]
                        b3c = bqt[:, 1, hc:hc + 1]
                        if is_shared:
                            for ft in range(FT):
                                fsl = slice(ft * 512, min((ft + 1) * 512, T))
                                fw = fsl.stop - fsl.start
                                o1 = ps_l1.tile([P, 512], F32, space="PSUM",
                                                name="o1")
                                dr3(o1[:, :fw], wqt, None, rh_h, rl_h,
                                    KD2, fsl, sel=(i, 0, 1))
                                o3 = ps_l1.tile([P, 512], F32, space="PSUM",
                                                name="o3")
                                dr3(o3[:, :fw], wqt, None, rh_h, rl_h,
                                    KD2, fsl, sel=(i, 2, 3))
                                l1_evict(o1, o3, b1c, b3c, hh, hl, hc, fsl, fw)
                        else:
                            fsl = slice(0, CAPE)
                            o1 = ps_l1.tile([P, 512], F32, space="PSUM",
                                            name="o1")
                            dr3(o1[:, :CAPE], wqt, None, rh_h, rl_h,
                                KD2, None, sel=(i, 0, 1))
                            o3 = ps_l1.tile([P, 512], F32, space="PSUM",
                                            name="o3")
                            dr3(o3[:, :CAPE], wqt, None, rh_h, rl_h,
                                KD2, None, sel=(i, 2, 3))
                            l1_evict(o1, o3, b1c, b3c, hh, hl, hc, fsl, CAPE)
                w2qt = w2s.tile([P, 2, HCN, D], F8, name="w2qt")
                nc.sync.dma_start(out=w2qt[:], in_=w2q[e])
                return hh, hl, w2qt

            def l2_matmuls(yp, hh, hl, w2qt, tsl, dsl, dw, sw, first_start):
                for h2 in range(HCN2):
                    hsl = slice(2 * h2, 2 * h2 + 2)
                    for ti, (lt_, rt_) in enumerate((
                            (hh[:, hsl, tsl], w2qt[:, 0, hsl, dsl]),
                            (hl[:, hsl, tsl], w2qt[:, 0, hsl, dsl]),
                            (hh[:, hsl, tsl], w2qt[:, 1, hsl, dsl]))):
                        nc.tensor.matmul(
                            out=yp[:sw, :dw], lhsT=lt_, rhs=rt_,
                            start=(first_start and h2 == 0 and ti == 0),
                            stop=(h2 == HCN2 - 1 and ti == 2),
                            perf_mode=DR)

            # ---- gate (first two tiles), then shared-half L1 with the
            # remaining gate tiles interleaved so the PE never waits on the
            # fp32 x stream ----
            def _gate_then_xtok():
                gate_tile(6)
                gate_tile(7)
                nc.sync.dma_start(out=xth[:], in_=xtokh[:])
                nc.sync.dma_start(out=xtl[:], in_=xtokl[:])

            gate_tile(0)
            gate_tile(1)
            hh, hl, w2qt = l1_phase(
                E, True, xbh, xbl, pre=pre_w,
                interleave={1: lambda: [gate_tile(m) for m in (2, 3)],
                            2: lambda: [gate_tile(m) for m in (4, 5)],
                            3: _gate_then_xtok})
            # expert 0's gather hides under the shared L2 matmuls
            xeh = xep.tile([P, KD, CAPE], F8, name="xeh", tag="xeh")
            xel = xep.tile([P, KD, CAPE], F8, name="xel", tag="xel")
            gather(0, xeh, xel)
            for tt in range(TT):
                tsl = slice(tt * P, (tt + 1) * P)
                for dt in range(DT):
                    dsl = slice(dt * 512, min((dt + 1) * 512, D))
                    dw = dsl.stop - dsl.start
                    yp = ps_y.tile([P, 512], F32, space="PSUM", name="yp")
                    nc.tensor.matmul(out=yp[:, :dw], lhsT=ones_sb[:],
                                     rhs=sb2x_sb[0:1, dsl],
                                     start=True, stop=False)
                    # routed-expert bias term: 64 * sum_e cw[t,e] b2[e]
                    nc.tensor.matmul(out=yp[:, :dw], lhsT=cwT[:, tt, :],
                                     rhs=b2e8_sb[:, dsl],
                                     start=False, stop=False)
                    l2_matmuls(yp, hh, hl, w2qt, tsl, dsl, dw, P, False)
                    nc.scalar.activation(yshared[:, tt, dsl], yp[:, :dw],
                                         AF.Copy, scale=WSI)

            def combine_slice(e, yebA):
                """yshared[:, m, :] += cw[:, m, e] * yebA[slot(t, e)]."""
                for m in range(TT):
                    pr = m // 2
                    sts = sorted({(pr * capm) // P, (pr * capm + capm - 1) // P})
                    p2ss = []
                    for st0 in sts:
                        c = st0 * P - pr * capm
                        pes = gtmp.tile([P, P], BF, name="pcs")
                        nc.vector.tensor_scalar(
                            out=pes[:], in0=iota_sb[:], scalar1=float(c),
                            scalar2=posb_all[:, m, e:e + 1],
                            op0=OP.add, op1=OP.is_equal)
                        pew = gtmp.tile([P, P], BF, name="pws")
                        nc.vector.tensor_scalar(out=pew[:], in0=pes[:],
                                                scalar1=cw[:, m, e:e + 1],
                                                scalar2=None, op0=OP.mult)
                        p2 = ps_sm.tile([P, P], BF, space="PSUM",
                                        name="p2", tag="sm")
                        nc.tensor.transpose(out=p2[:], in_=pew[:],
                                            identity=id_sb[:])
                        p2s = gtmp.tile([P, P], BF, name="p2s")
                        nc.scalar.activation(p2s[:], p2[:], AF.Copy)
                        p2ss.append(p2s)
                    for dt in range(DT):
                        dsl = slice(dt * 512, min((dt + 1) * 512, D))
                        dw = dsl.stop - dsl.start
                        tmp = ps_y.tile([P, 512], F32, space="PSUM",
                                        name="ypc", tag="yp")
                        for pi, st0 in enumerate(sts):
                            nc.tensor.matmul(
                                out=tmp[:, :dw], lhsT=p2ss[pi][:],
                                rhs=yebA[:, st0, dsl],
                                start=(pi == 0), stop=(pi == len(sts) - 1))
                        nc.vector.tensor_add(yshared[:, m, dsl],
                                             yshared[:, m, dsl],
                                             tmp[:, :dw])

            # ---- routed experts over dispatched slots ----
            for e in range(E):
                hh, hl, w2qt = l1_phase(e, False, xeh, xel)
                if e < E - 1:
                    xeh = xep.tile([P, KD, CAPE], F8, name="xeh", tag="xeh")
                    xel = xep.tile([P, KD, CAPE], F8, name="xel", tag="xel")
                    gather(e + 1, xeh, xel)
                yebA = yea.tile([P, ST, D], BF, name="yebA")
                for st in range(ST):
                    sw = min(P, CAPE - st * P)
                    ssl = slice(st * P, st * P + sw)
                    for dt in range(DT):
                        dsl = slice(dt * 512, min((dt + 1) * 512, D))
                        dw = dsl.stop - dsl.start
                        yp = ps_y.tile([P, 512], F32, space="PSUM", name="yp")
                        l2_matmuls(yp, hh, hl, w2qt, ssl, dsl, dw, sw, True)
                        nc.scalar.activation(yebA[:sw, st, dsl], yp[:sw, :dw],
                                             AF.Copy, scale=WSI)
                combine_slice(e, yebA)

            # prefetch the second shared-half's first L1 chunks so its
            # matmuls start right after the last combine slice
            pre_w2 = {}
            for hcp in range(min(2, HCN2)):
                t = w1s.tile([P, 2, 4, KD, P], F8, name="wqt", tag="wqt")
                nc.sync.dma_start(out=t[:], in_=wq[E + 1, hcp])
                pre_w2[hcp] = t

            # ---- shared expert, second half; emits final y ----
            hh, hl, w2qt = l1_phase(E + 1, True, xbh, xbl, pre=pre_w2)
            for tt in range(TT):
                tsl = slice(tt * P, (tt + 1) * P)
                for dt in range(DT):
                    dsl = slice(dt * 512, min((dt + 1) * 512, D))
                    dw = dsl.stop - dsl.start
                    yp = ps_y.tile([P, 512], F32, space="PSUM", name="yp")
                    l2_matmuls(yp, hh, hl, w2qt, tsl, dsl, dw, P, True)
                    yt = comb.tile([P, 512], F32, name="yt")
                    nc.vector.scalar_tensor_tensor(
                        out=yt[:, :dw], in0=yp[:, :dw], scalar=WSI,
                        in1=yshared[:, tt, dsl], op0=OP.mult, op1=OP.add)
                    nc.scalar.dma_start(out=y[:, tt, dsl], in_=yt[:, :dw])

    nc.compile()
    return nc


# ---------------- host-side packing ----------------

def _split_fp8(a, scale=1.0):
    a = np.asarray(a, np.float32) * scale
    hi = a.astype(FP8)
    lo = (a - hi.astype(np.float32)).astype(FP8)
    return hi, lo


def pack_static(cfg: Cfg, gate_w, gate_b, w1, b1, w2, b2, w3, b3,
                sw1, sb1, sw2, sb2, sw3, sb3):
    D, H, E, NV, n_sh = cfg.D, cfg.H, cfg.E, cfg.NV, cfg.n_sh
    KD, HCN = cfg.KD, cfg.HCN
    HCN2 = HCN // 2

    w1T = np.transpose(w1, (0, 2, 1))                      # [E, D, H]
    w3T = np.transpose(w3, (0, 2, 1))
    w2T = np.transpose(w2, (0, 2, 1))                      # [E, H, D]
    s1T = sw1.T.reshape(D, n_sh, H).transpose(1, 0, 2)     # [n_sh, D, H]
    s3T = sw3.T.reshape(D, n_sh, H).transpose(1, 0, 2)
    s2T = sw2.T.reshape(n_sh, H, D)                        # [n_sh, H, D]
    w1T_all = np.concatenate([w1T, s1T], 0)                # [NV, D, H]
    w3T_all = np.concatenate([w3T, s3T], 0)
    w2T_all = np.concatenate([w2T, s2T], 0)                # [NV, H, D]

    w1t = np.ascontiguousarray(
        w1T_all.reshape(NV, KD, P, HCN, P).transpose(0, 3, 2, 1, 4))
    w3t = np.ascontiguousarray(
        w3T_all.reshape(NV, KD, P, HCN, P).transpose(0, 3, 2, 1, 4))
    w2t = np.ascontiguousarray(
        w2T_all.reshape(NV, HCN, P, D).transpose(0, 2, 1, 3))
    w1h_, w1l_ = _split_fp8(w1t, WS)
    w3h_, w3l_ = _split_fp8(w3t, WS)
    w2h_, w2l_ = _split_fp8(w2t, WS)

    # wq: [NV, HCN2, P, 2, 4, KD, P]
    wq_ = np.stack([w1h_, w1l_, w3h_, w3l_], axis=2)   # [NV, HCN, 4, P, KD, P]
    wq_ = wq_.reshape(NV, HCN2, 2, 4, P, KD, P).transpose(0, 1, 4, 2, 3, 5, 6)
    wq_ = np.ascontiguousarray(wq_)

    # w2q: [NV, P, 2, HCN, D]
    w2q_ = np.ascontiguousarray(
        np.stack([w2h_, w2l_], axis=1).transpose(0, 2, 1, 3, 4))

    b1_all = np.concatenate([b1, sb1.reshape(n_sh, H)], 0)  # [NV, H]
    b3_all = np.concatenate([b3, sb3.reshape(n_sh, H)], 0)
    b1a = b1_all.reshape(NV, HCN, P).transpose(0, 2, 1)     # [NV, P, HCN]
    b3a = b3_all.reshape(NV, HCN, P).transpose(0, 2, 1)
    bq_ = np.ascontiguousarray(
        np.stack([b1a, b3a], axis=2)).astype(np.float32)    # [NV, P, 2, HCN]

    gwt = np.ascontiguousarray(
        gate_w.T.reshape(KD, P, E).transpose(1, 0, 2)).astype(np.float32)

    return dict(
        wq=wq_, w2q=w2q_, bq=bq_,
        b2e8=(WS * b2).astype(BF16),                        # [E, D] x WS
        sb2x=(WS * sb2[None]).astype(BF16),                 # [1, D]
        gwt=gwt, gb=gate_b[None].astype(np.float32),
        ones1=np.ones((1, P), BF16),
        onesc=np.ones((P, 1), BF16),
        lt=np.triu(np.ones((P, P))).astype(BF16),
        ident=np.eye(P).astype(BF16),
        iota=np.tile(np.arange(P, dtype=np.float32), (P, 1)),
    )


def pack_x(cfg: Cfg, x_tokens):
    """x_tokens [T, D] fp32 -> device layouts (gate fp32 + fp8 hi/lo)."""
    T, D = x_tokens.shape
    xT = np.ascontiguousarray(
        x_tokens.T.reshape(cfg.KD, P, T).transpose(1, 0, 2)).astype(np.float32)
    xh, xl = _split_fp8(xT)
    xtok = np.ascontiguousarray(
        x_tokens.reshape(cfg.TT, P, D).transpose(1, 0, 2))
    xth, xtl = _split_fp8(xtok)
    return dict(xT=xT, xtbh=xh, xtbl=xl, xtokh=xth, xtokl=xtl)


def unpack_y(cfg: Cfg, y_dev):
    """y device layout [P, TT, D] -> [T, D]."""
    return np.ascontiguousarray(y_dev.transpose(1, 0, 2).reshape(cfg.T, cfg.D))


_CACHE = {}


def _get_nc(cfg: Cfg):
    key = (cfg.D, cfg.H, cfg.E, cfg.n_sh, cfg.T, cfg.capm)
    if key not in _CACHE:
        _CACHE[key] = build_nc_fp8(cfg)
    return _CACHE[key]


def make_in_maps(cfg: Cfg, inputs):
    static = pack_static(
        cfg,
        np.asarray(inputs["gate_w"], np.float32), np.asarray(inputs["gate_b"], np.float32),
        np.asarray(inputs["w1"], np.float32), np.asarray(inputs["b1"], np.float32),
        np.asarray(inputs["w2"], np.float32), np.asarray(inputs["b2"], np.float32),
        np.asarray(inputs["w3"], np.float32), np.asarray(inputs["b3"], np.float32),
        np.asarray(inputs["sw1"], np.float32), np.asarray(inputs["sb1"], np.float32),
        np.asarray(inputs["sw2"], np.float32), np.asarray(inputs["sb2"], np.float32),
        np.asarray(inputs["sw3"], np.float32), np.asarray(inputs["sb3"], np.float32),
    )
    x = np.asarray(inputs["x"], np.float32)
    B, S, D = x.shape
    xf = x.reshape(-1, D)
    in_maps = []
    for c in range(cfg.n_cores):
        m = dict(static)
        m.update(pack_x(cfg, xf[c * cfg.T:(c + 1) * cfg.T]))
        in_maps.append(m)
    return in_maps


def kernel(**inputs) -> np.ndarray:
    x = np.asarray(inputs["x"], np.float32)
    B, S, D = x.shape
    N = B * S
    cfg = Cfg(D=D, T=N // 8, n_cores=8)
    nc = _get_nc(cfg)
    in_maps = make_in_maps(cfg, inputs)
    res = run_bass_kernel_spmd(nc, in_maps, list(range(cfg.n_cores)))
    outs = [unpack_y(cfg, res.results[c]["y"]) for c in range(cfg.n_cores)]
    return np.concatenate(outs, 0).reshape(B, S, D)


# revision 35
# speedup vs baseline: 1.3759x; 1.0651x over previous
"""MoE (8 routed experts, top-2, + shared expert) on 8 NeuronCores.

Data-parallel over tokens (1024/core), weights replicated, capacity-routed
dispatch (pair buckets, capm=96) as in the bf16 baseline — but all large
GEMMs run as fp8e4m3 DoubleRow matmuls with 3-term error compensation:

    A @ B  ~=  A_hi@B_hi + A_hi@B_lo + A_lo@B_hi

where X_hi = fp8(X), X_lo = fp8(X - X_hi).  DoubleRow consumes two
128-deep K-tiles per instruction at 0.5 cycles/row, so each compensated
GEMM costs 0.75x its bf16 schedule while adding only ~1e-3 relative
error.  Weights are pre-scaled by 64 on the host (fp8 subnormal cutoff),
descaled in the PSUM eviction.  The gate runs in fp32 so routing matches
the reference.

Schedule notes: w1/w3 hi+lo chunks ride one DMA per hc-pair (the SP
sequencer costs ~0.9us per DMA, so descriptor count is a real resource);
expert e+1's token gather is issued between expert e's L1 and L2 so its
PSUM evictions hide under L2 matmuls; the L1 eviction chain is
Silu -> scale -> mul with the fp8 split offloaded to the idle GPSIMD
engine; half of the shared expert runs after the combine phase so the
combine's DRAM round-trip sits under shared-expert matmuls, not at the
kernel tail.
"""

import numpy as np
import ml_dtypes

import concourse.bacc as bacc
import concourse.bass as bass
import concourse.tile as tile
import concourse.mybir as mybir
from concourse.bass_utils import run_bass_kernel_spmd

BF16 = ml_dtypes.bfloat16
FP8 = ml_dtypes.float8_e4m3
F32 = mybir.dt.float32
BF = mybir.dt.bfloat16
F8 = mybir.dt.float8e4
AF = mybir.ActivationFunctionType
OP = mybir.AluOpType
DR = mybir.MatmulPerfMode.DoubleRow

P = 128
WS = 64.0          # host-side weight scale before fp8 quantization
WSI = 1.0 / WS


class Cfg:
    def __init__(self, D=1024, H=2048, E=8, n_sh=2, T=1024, n_cores=8, capm=92):
        self.D, self.H, self.E, self.n_sh, self.T = D, H, E, n_sh, T
        self.NV = E + n_sh          # virtual experts
        self.HS = n_sh * H          # shared hidden
        self.KD = D // P            # K chunks over D
        self.HCN = H // P           # h chunks over H
        self.TT = T // P            # token 128-tiles
        self.DT = (D + 511) // 512  # output d 512-tiles
        self.FT = (T + 511) // 512  # layer-1 free 512-tiles
        self.n_cores = n_cores
        self.capm = capm            # per-(expert, tile-pair) dispatch capacity
        self.NP = self.TT // 2      # token-tile pairs
        self.CAPE = self.NP * capm  # slots per expert
        self.ST = (self.CAPE + P - 1) // P  # slot 128-tiles per expert


def build_nc_fp8(cfg: Cfg):
    D, H, E, NV, T = cfg.D, cfg.H, cfg.E, cfg.NV, cfg.T
    KD, HCN, TT, DT, FT = cfg.KD, cfg.HCN, cfg.TT, cfg.DT, cfg.FT
    capm, CAPE, ST, NP = cfg.capm, cfg.CAPE, cfg.ST, cfg.NP
    KD2, HCN2 = KD // 2, HCN // 2

    nc = bacc.Bacc("TRN2", target_bir_lowering=False)

    xT = nc.dram_tensor("xT", [P, KD, T], F32, kind="ExternalInput")
    xtbh = nc.dram_tensor("xtbh", [P, KD, T], F8, kind="ExternalInput")
    xtbl = nc.dram_tensor("xtbl", [P, KD, T], F8, kind="ExternalInput")
    xtokh = nc.dram_tensor("xtokh", [P, TT, D], F8, kind="ExternalInput")
    xtokl = nc.dram_tensor("xtokl", [P, TT, D], F8, kind="ExternalInput")
    # w1/w3 hi+lo packed per hc-pair: [e, hcp, p, i(2), which(4), KD, P]
    wq = nc.dram_tensor("wq", [NV, HCN2, P, 2, 4, KD, P], F8,
                        kind="ExternalInput")
    # w2 hi+lo packed per expert: [e, p, which(2), HCN, D]
    w2q = nc.dram_tensor("w2q", [NV, P, 2, HCN, D], F8, kind="ExternalInput")
    # b1, b3 packed per expert: [e, p, which(2), HCN]
    bq = nc.dram_tensor("bq", [NV, P, 2, HCN], F32, kind="ExternalInput")
    b2e8 = nc.dram_tensor("b2e8", [E, D], BF, kind="ExternalInput")
    sb2x = nc.dram_tensor("sb2x", [1, D], BF, kind="ExternalInput")
    gwt = nc.dram_tensor("gwt", [P, KD, E], F32, kind="ExternalInput")
    gb = nc.dram_tensor("gb", [1, E], F32, kind="ExternalInput")
    ones1 = nc.dram_tensor("ones1", [1, P], BF, kind="ExternalInput")
    onesc = nc.dram_tensor("onesc", [P, 1], BF, kind="ExternalInput")
    lt = nc.dram_tensor("lt", [P, P], BF, kind="ExternalInput")
    ident = nc.dram_tensor("ident", [P, P], BF, kind="ExternalInput")
    iota = nc.dram_tensor("iota", [P, P], F32, kind="ExternalInput")
    y = nc.dram_tensor("y", [P, TT, D], F32, kind="ExternalOutput")

    OOB = 3.0e6

    with tile.TileContext(nc) as tc:
        with (
            tc.tile_pool(name="const1", bufs=1) as const1,
            tc.tile_pool(name="gchunk", bufs=2) as gchunk,
            tc.tile_pool(name="gtmp", bufs=4) as gtmp,
            tc.tile_pool(name="w1s", bufs=3) as w1s,
            tc.tile_pool(name="b13", bufs=2) as b13,
            tc.tile_pool(name="w2s", bufs=1) as w2s,
            tc.tile_pool(name="hpool", bufs=1) as hpool,
            tc.tile_pool(name="s1p", bufs=2) as s1p,
            tc.tile_pool(name="yea", bufs=2) as yea,
            tc.tile_pool(name="xep", bufs=2) as xep,
            tc.tile_pool(name="pep", bufs=8) as pep,
            tc.tile_pool(name="comb", bufs=2) as comb,
            tc.tile_pool(name="ps_l1", bufs=2, space="PSUM") as ps_l1,
            tc.tile_pool(name="ps_y", bufs=2, space="PSUM") as ps_y,
            tc.tile_pool(name="ps_sm", bufs=2, space="PSUM") as ps_sm,
        ):
            # ---- resident constants / state ----
            xbh = const1.tile([P, KD, T], F8)
            xbl = const1.tile([P, KD, T], F8)
            xth = const1.tile([P, TT, D], F8)
            xtl = const1.tile([P, TT, D], F8)
            yshared = const1.tile([P, TT, D], BF)
            cw = const1.tile([P, TT, E], F32)
            cwT = const1.tile([8, TT, P], BF)
            posb_all = const1.tile([P, TT, E], F32)
            ones_sb = const1.tile([1, P], BF)
            onesc_sb = const1.tile([P, 1], BF)
            gwt_sb = const1.tile([P, KD, E], F32)
            gb_sb = const1.tile([1, E], F32)
            zerob = const1.tile([P, 1], F32)
            onesf = const1.tile([1, P], F32)
            lt_sb = const1.tile([P, P], BF)
            id_sb = const1.tile([P, P], BF)
            iota_sb = const1.tile([P, P], F32)
            b2e8_sb = const1.tile([8, D], BF)
            sb2x_sb = const1.tile([1, D], BF)

            # the gate's first two x chunks and the gate weights go out
            # first so the PE starts within ~3us; everything else queues up
            # behind them in order of first use
            pre_xc = {}
            for m in range(2):
                xchunk = gchunk.tile([P, KD, P], F32)
                nc.sync.dma_start(out=xchunk[:],
                                  in_=xT[:, :, m * P:(m + 1) * P])
                pre_xc[m] = xchunk
            nc.sync.dma_start(out=gwt_sb[:], in_=gwt[:])
            nc.sync.dma_start(out=gb_sb[:], in_=gb[:])
            nc.sync.dma_start(out=ones_sb[:], in_=ones1[:])
            nc.sync.dma_start(out=onesc_sb[:], in_=onesc[:])
            nc.sync.dma_start(out=lt_sb[:], in_=lt[:])
            nc.sync.dma_start(out=id_sb[:], in_=ident[:])
            nc.sync.dma_start(out=iota_sb[:], in_=iota[:])
            nc.vector.memset(zerob[:], 0.0)
            nc.vector.memset(onesf[:], 1.0)

            # prefetch the first shared-half L1 weight chunks so its matmuls
            # can start while the gate phase runs
            pre_w = {}
            t = w1s.tile([P, 2, 4, KD, P], F8, name="wqt", tag="wqt")
            nc.sync.dma_start(out=t[:], in_=wq[E, 0])
            pre_w[0] = t
            nc.sync.dma_start(out=xbh[:], in_=xtbh[:])
            nc.sync.dma_start(out=xbl[:], in_=xtbl[:])
            if HCN2 > 1:
                t = w1s.tile([P, 2, 4, KD, P], F8, name="wqt", tag="wqt")
                nc.sync.dma_start(out=t[:], in_=wq[E, 1])
                pre_w[1] = t
            nc.sync.dma_start(out=b2e8_sb[:], in_=b2e8[:])
            nc.sync.dma_start(out=sb2x_sb[:], in_=sb2x[:])

            # ---- gate + routing, per token tile (paired buckets) ----
            cntb = None

            def gate_tile(m):
                nonlocal cntb
                if m in pre_xc:
                    xchunk = pre_xc[m]
                else:
                    xchunk = gchunk.tile([P, KD, P], F32)
                    nc.sync.dma_start(out=xchunk[:],
                                      in_=xT[:, :, m * P:(m + 1) * P])

                pg = ps_y.tile([P, P], F32, space="PSUM", name="pg", tag="yp")
                for k in range(KD):
                    nc.tensor.matmul(out=pg[:, :E], lhsT=xchunk[:, k, :],
                                     rhs=gwt_sb[:, k, :],
                                     start=(k == 0), stop=False)
                nc.tensor.matmul(out=pg[:, :E], lhsT=onesf[:], rhs=gb_sb[:],
                                 start=False, stop=True)

                lg = gtmp.tile([P, E], F32)
                nc.scalar.activation(lg[:], pg[:, :E], AF.Copy)
                m8 = gtmp.tile([P, 8], F32)
                nc.vector.max(m8[:], lg[:])
                ex = gtmp.tile([P, E], F32)
                nc.vector.tensor_scalar(out=ex[:], in0=lg[:],
                                        scalar1=m8[:, 0:1], scalar2=None,
                                        op0=OP.subtract)
                nc.scalar.activation(ex[:], ex[:], AF.Exp, bias=zerob[:])
                mask = gtmp.tile([P, E], F32)
                nc.vector.tensor_scalar(out=mask[:], in0=lg[:],
                                        scalar1=m8[:, 1:2], scalar2=None,
                                        op0=OP.is_ge)
                e2 = gtmp.tile([P, 1], F32)
                nc.vector.tensor_tensor(out=e2[:], in0=m8[:, 1:2],
                                        in1=m8[:, 0:1], op=OP.subtract)
                nc.scalar.activation(e2[:], e2[:], AF.Exp, bias=zerob[:])
                den = gtmp.tile([P, 1], F32)
                nc.vector.tensor_scalar(out=den[:], in0=e2[:], scalar1=1.0,
                                        scalar2=None, op0=OP.add)
                rec = gtmp.tile([P, 1], F32)
                nc.vector.reciprocal(rec[:], den[:])
                cwm = gtmp.tile([P, E], F32)
                nc.vector.tensor_mul(cwm[:], ex[:], mask[:])
                nc.vector.tensor_scalar(out=cw[:, m, :], in0=cwm[:],
                                        scalar1=rec[:, 0:1], scalar2=None,
                                        op0=OP.mult)
                # bf16 transpose of the combine weights for the b2 matmul
                cwb = gtmp.tile([P, E], BF, name="cwb")
                nc.vector.tensor_copy(cwb[:], cw[:, m, :])
                ptp = ps_sm.tile([P, P], BF, space="PSUM", name="ptp", tag="sm")
                nc.tensor.transpose(out=ptp[:E, :], in_=cwb[:], identity=id_sb[:])
                nc.scalar.activation(cwT[:, m, :], ptp[:E, :], AF.Copy)

                # bucket-local slot: pair prefix(mask) - mask; OOB unrouted
                maskb = gtmp.tile([P, E], BF)
                nc.vector.tensor_copy(maskb[:], mask[:])
                pp = ps_y.tile([P, P], F32, space="PSUM", name="pp", tag="yp")
                if m % 2 == 0:
                    nc.tensor.matmul(out=pp[:, :E], lhsT=lt_sb[:],
                                     rhs=maskb[:], start=True, stop=True)
                    cnt_ps = ps_sm.tile([1, P], F32, space="PSUM",
                                        name="cntp", tag="sm")
                    nc.tensor.matmul(out=cnt_ps[0:1, :E], lhsT=onesc_sb[:],
                                     rhs=maskb[:], start=True, stop=True)
                    cntb = gtmp.tile([1, E], BF, name="cntb")
                    nc.scalar.activation(cntb[:], cnt_ps[0:1, :E], AF.Copy)
                else:
                    nc.tensor.matmul(out=pp[:, :E], lhsT=lt_sb[:],
                                     rhs=maskb[:], start=True, stop=False)
                    nc.tensor.matmul(out=pp[:, :E], lhsT=ones_sb[:],
                                     rhs=cntb[:], start=False, stop=True)
                t1m = gtmp.tile([P, E], F32)
                nc.vector.scalar_tensor_tensor(out=t1m[:], in0=mask[:],
                                               scalar=-1.0, in1=pp[:, :E],
                                               op0=OP.mult, op1=OP.add)
                notm = gtmp.tile([P, E], F32)
                nc.vector.tensor_scalar(out=notm[:], in0=mask[:],
                                        scalar1=-1.0, scalar2=1.0,
                                        op0=OP.mult, op1=OP.add)
                nc.vector.scalar_tensor_tensor(out=posb_all[:, m, :],
                                               in0=notm[:], scalar=OOB,
                                               in1=t1m[:],
                                               op0=OP.mult, op1=OP.add)

            def dr3(out_ap, lh, ll, rh, rl, kn, fsl, sel=None):
                """3-term compensated fp8 DoubleRow accumulation group.

                lh/ll: either plain [P, KD, P] tiles or a packed wqt tile
                indexed via sel=(i, jh, jl).  k2-major so gather evictions
                unblock the group incrementally.
                """
                for k2 in range(kn):
                    ksl2 = slice(2 * k2, 2 * k2 + 2)
                    if sel is None:
                        lhs_h, lhs_l = lh[:, ksl2, :], ll[:, ksl2, :]
                    else:
                        i, jh, jl = sel
                        lhs_h = lh[:, i, jh, ksl2, :]
                        lhs_l = lh[:, i, jl, ksl2, :]
                    rhs_h = rh[:, ksl2, fsl] if fsl is not None else rh[:, ksl2]
                    rhs_l = rl[:, ksl2, fsl] if fsl is not None else rl[:, ksl2]
                    for ti, (lt_, rt_) in enumerate(
                            ((lhs_h, rhs_h), (lhs_h, rhs_l), (lhs_l, rhs_h))):
                        nc.tensor.matmul(
                            out=out_ap, lhsT=lt_, rhs=rt_,
                            start=(k2 == 0 and ti == 0),
                            stop=(k2 == kn - 1 and ti == 2),
                            perf_mode=DR)

            def l1_evict(o1, o3, b1c, b3c, hh, hl, hc, fsl, fw):
                """h = silu(o1/WS + b1) * (o3/WS + b3) -> fp8 hi/lo pair."""
                s = s1p.tile([P, 512], F32, name="s")
                nc.scalar.activation(s[:, :fw], o1[:, :fw], AF.Silu,
                                     bias=b1c, scale=WSI)
                v = s1p.tile([P, 512], F32, name="v")
                nc.vector.tensor_scalar(out=v[:, :fw], in0=o3[:, :fw],
                                        scalar1=WSI, scalar2=b3c,
                                        op0=OP.mult, op1=OP.add)
                hf = s1p.tile([P, 512], F32, name="hf")
                nc.vector.tensor_mul(hf[:, :fw], s[:, :fw], v[:, :fw])
                nc.scalar.activation(hh[:, hc, fsl], hf[:, :fw], AF.Copy)
                nc.gpsimd.tensor_sub(hl[:, hc, fsl], hf[:, :fw],
                                     hh[:, hc, fsl])

            def gather(e, xeh, xel):
                """One-hot dispatch + feature-major token gather, hi+lo."""
                pes = []
                for pr in range(NP):
                    pe2 = pep.tile([P, 2, capm], F8, name="pe2", tag="pe2")
                    for half in range(2):
                        nc.vector.tensor_scalar(
                            out=pe2[:, half, :], in0=iota_sb[:, :capm],
                            scalar1=posb_all[:, 2 * pr + half, e:e + 1],
                            scalar2=None, op0=OP.is_equal)
                    pes.append(pe2)
                for k in range(KD):
                    ksl = slice(k * P, (k + 1) * P)
                    for src, dst, nm in ((xth, xeh, "gxh"), (xtl, xel, "gxl")):
                        gx = ps_sm.tile([P, NP * capm], F32,
                                        space="PSUM", name=nm, tag="sm")
                        for pr in range(NP):
                            nc.tensor.matmul(
                                out=gx[:, pr * capm:(pr + 1) * capm],
                                lhsT=src[:, 2 * pr:2 * pr + 2, ksl],
                                rhs=pes[pr][:],
                                start=True, stop=True, perf_mode=DR)
                        nc.scalar.activation(dst[:, k, :], gx[:], AF.Copy)

            def l1_phase(e, is_shared, rh_h, rl_h, pre=None, interleave=None):
                bqt = b13.tile([P, 2, HCN], F32, name="bqt")
                nc.sync.dma_start(out=bqt[:], in_=bq[e])
                hh = hpool.tile([P, HCN, T], F8, name="hh", tag="hh")
                hl = hpool.tile([P, HCN, T], F8, name="hl", tag="hl")
                for hcp in range(HCN2):
                    if interleave is not None and hcp in interleave:
                        interleave[hcp]()
                    if pre is not None and hcp in pre:
                        wqt = pre[hcp]
                    else:
                        wqt = w1s.tile([P, 2, 4, KD, P], F8, name="wqt",
                                       tag="wqt")
                        nc.sync.dma_start(out=wqt[:], in_=wq[e, hcp])
                    for i in range(2):
                        hc = 2 * hcp + i
                        b1c = bqt[:, 0, hc:hc + 1]
                        b3c = bqt[:, 1, hc:hc + 1]
                        if is_shared:
                            for ft in range(FT):
                                fsl = slice(ft * 512, min((ft + 1) * 512, T))
                                fw = fsl.stop - fsl.start
                                o1 = ps_l1.tile([P, 512], F32, space="PSUM",
                                                name="o1")
                                dr3(o1[:, :fw], wqt, None, rh_h, rl_h,
                                    KD2, fsl, sel=(i, 0, 1))
                                o3 = ps_l1.tile([P, 512], F32, space="PSUM",
                                                name="o3")
                                dr3(o3[:, :fw], wqt, None, rh_h, rl_h,
                                    KD2, fsl, sel=(i, 2, 3))
                                l1_evict(o1, o3, b1c, b3c, hh, hl, hc, fsl, fw)
                        else:
                            fsl = slice(0, CAPE)
                            o1 = ps_l1.tile([P, 512], F32, space="PSUM",
                                            name="o1")
                            dr3(o1[:, :CAPE], wqt, None, rh_h, rl_h,
                                KD2, None, sel=(i, 0, 1))
                            o3 = ps_l1.tile([P, 512], F32, space="PSUM",
                                            name="o3")
                            dr3(o3[:, :CAPE], wqt, None, rh_h, rl_h,
                                KD2, None, sel=(i, 2, 3))
                            l1_evict(o1, o3, b1c, b3c, hh, hl, hc, fsl, CAPE)
                w2qt = w2s.tile([P, 2, HCN, D], F8, name="w2qt")
                nc.sync.dma_start(out=w2qt[:], in_=w2q[e])
                return hh, hl, w2qt

            def l2_matmuls(yp, hh, hl, w2qt, tsl, dsl, dw, sw, first_start):
                for h2 in range(HCN2):
                    hsl = slice(2 * h2, 2 * h2 + 2)
                    for ti, (lt_, rt_) in enumerate((
                            (hh[:, hsl, tsl], w2qt[:, 0, hsl, dsl]),
                            (hl[:, hsl, tsl], w2qt[:, 0, hsl, dsl]),
                            (hh[:, hsl, tsl], w2qt[:, 1, hsl, dsl]))):
                        nc.tensor.matmul(
                            out=yp[:sw, :dw], lhsT=lt_, rhs=rt_,
                            start=(first_start and h2 == 0 and ti == 0),
                            stop=(h2 == HCN2 - 1 and ti == 2),
                            perf_mode=DR)

            # ---- gate (first two tiles), then shared-half L1 with the
            # remaining gate tiles interleaved so the PE never waits on the
            # fp32 x stream ----
            def _gate_then_xtok():
                gate_tile(6)
                gate_tile(7)
                nc.sync.dma_start(out=xth[:], in_=xtokh[:])
                nc.sync.dma_start(out=xtl[:], in_=xtokl[:])

            gate_tile(0)
            gate_tile(1)
            hh, hl, w2qt = l1_phase(
                E, True, xbh, xbl, pre=pre_w,
                interleave={1: lambda: [gate_tile(m) for m in (2, 3)],
                            2: lambda: [gate_tile(m) for m in (4, 5)],
                            3: _gate_then_xtok})
            # expert 0's gather hides under the shared L2 matmuls
            xeh = xep.tile([P, KD, CAPE], F8, name="xeh", tag="xeh")
            xel = xep.tile([P, KD, CAPE], F8, name="xel", tag="xel")
            gather(0, xeh, xel)
            for tt in range(TT):
                tsl = slice(tt * P, (tt + 1) * P)
                for dt in range(DT):
                    dsl = slice(dt * 512, min((dt + 1) * 512, D))
                    dw = dsl.stop - dsl.start
                    yp = ps_y.tile([P, 512], F32, space="PSUM", name="yp")
                    nc.tensor.matmul(out=yp[:, :dw], lhsT=ones_sb[:],
                                     rhs=sb2x_sb[0:1, dsl],
                                     start=True, stop=False)
                    # routed-expert bias term: 64 * sum_e cw[t,e] b2[e]
                    nc.tensor.matmul(out=yp[:, :dw], lhsT=cwT[:, tt, :],
                                     rhs=b2e8_sb[:, dsl],
                                     start=False, stop=False)
                    l2_matmuls(yp, hh, hl, w2qt, tsl, dsl, dw, P, False)
                    nc.scalar.activation(yshared[:, tt, dsl], yp[:, :dw],
                                         AF.Copy, scale=WSI)

            def combine_slice(e, yebA):
                """yshared[:, m, :] += cw[:, m, e] * yebA[slot(t, e)]."""
                for m in range(TT):
                    pr = m // 2
                    sts = sorted({(pr * capm) // P, (pr * capm + capm - 1) // P})
                    p2ss = []
                    for st0 in sts:
                        c = st0 * P - pr * capm
                        pes = gtmp.tile([P, P], BF, name="pcs")
                        nc.vector.tensor_scalar(
                            out=pes[:], in0=iota_sb[:], scalar1=float(c),
                            scalar2=posb_all[:, m, e:e + 1],
                            op0=OP.add, op1=OP.is_equal)
                        pew = gtmp.tile([P, P], BF, name="pws")
                        nc.vector.tensor_scalar(out=pew[:], in0=pes[:],
                                                scalar1=cw[:, m, e:e + 1],
                                                scalar2=None, op0=OP.mult)
                        p2 = ps_sm.tile([P, P], BF, space="PSUM",
                                        name="p2", tag="sm")
                        nc.tensor.transpose(out=p2[:], in_=pew[:],
                                            identity=id_sb[:])
                        p2s = gtmp.tile([P, P], BF, name="p2s")
                        nc.scalar.activation(p2s[:], p2[:], AF.Copy)
                        p2ss.append(p2s)
                    for dt in range(DT):
                        dsl = slice(dt * 512, min((dt + 1) * 512, D))
                        dw = dsl.stop - dsl.start
                        tmp = ps_y.tile([P, 512], F32, space="PSUM",
                                        name="ypc", tag="yp")
                        for pi, st0 in enumerate(sts):
                            nc.tensor.matmul(
                                out=tmp[:, :dw], lhsT=p2ss[pi][:],
                                rhs=yebA[:, st0, dsl],
                                start=(pi == 0), stop=(pi == len(sts) - 1))
                        nc.vector.tensor_add(yshared[:, m, dsl],
                                             yshared[:, m, dsl],
                                             tmp[:, :dw])

            # ---- routed experts over dispatched slots ----
            for e in range(E):
                hh, hl, w2qt = l1_phase(e, False, xeh, xel)
                if e < E - 1:
                    xeh = xep.tile([P, KD, CAPE], F8, name="xeh", tag="xeh")
                    xel = xep.tile([P, KD, CAPE], F8, name="xel", tag="xel")
                    gather(e + 1, xeh, xel)
                yebA = yea.tile([P, ST, D], BF, name="yebA")
                for st in range(ST):
                    sw = min(P, CAPE - st * P)
                    ssl = slice(st * P, st * P + sw)
                    for dt in range(DT):
                        dsl = slice(dt * 512, min((dt + 1) * 512, D))
                        dw = dsl.stop - dsl.start
                        yp = ps_y.tile([P, 512], F32, space="PSUM", name="yp")
                        l2_matmuls(yp, hh, hl, w2qt, ssl, dsl, dw, sw, True)
                        nc.scalar.activation(yebA[:sw, st, dsl], yp[:sw, :dw],
                                             AF.Copy, scale=WSI)
                combine_slice(e, yebA)

            # prefetch the second shared-half's first L1 chunks so its
            # matmuls start right after the last combine slice
            pre_w2 = {}
            for hcp in range(min(2, HCN2)):
                t = w1s.tile([P, 2, 4, KD, P], F8, name="wqt", tag="wqt")
                nc.sync.dma_start(out=t[:], in_=wq[E + 1, hcp])
                pre_w2[hcp] = t

            # ---- shared expert, second half; emits final y ----
            hh, hl, w2qt = l1_phase(E + 1, True, xbh, xbl, pre=pre_w2)
            for tt in range(TT):
                tsl = slice(tt * P, (tt + 1) * P)
                for dt in range(DT):
                    dsl = slice(dt * 512, min((dt + 1) * 512, D))
                    dw = dsl.stop - dsl.start
                    yp = ps_y.tile([P, 512], F32, space="PSUM", name="yp")
                    l2_matmuls(yp, hh, hl, w2qt, tsl, dsl, dw, P, True)
                    if dt == 0:
                        yt = comb.tile([P, 1024], F32, name="yt")
                    nc.vector.scalar_tensor_tensor(
                        out=yt[:, dsl], in0=yp[:, :dw], scalar=WSI,
                        in1=yshared[:, tt, dsl], op0=OP.mult, op1=OP.add)
                if D <= 1024:
                    nc.scalar.dma_start(out=y[:, tt, :], in_=yt[:, :D])

    nc.compile()
    return nc


# ---------------- host-side packing ----------------

def _split_fp8(a, scale=1.0):
    a = np.asarray(a, np.float32) * scale
    hi = a.astype(FP8)
    lo = (a - hi.astype(np.float32)).astype(FP8)
    return hi, lo


def pack_static(cfg: Cfg, gate_w, gate_b, w1, b1, w2, b2, w3, b3,
                sw1, sb1, sw2, sb2, sw3, sb3):
    D, H, E, NV, n_sh = cfg.D, cfg.H, cfg.E, cfg.NV, cfg.n_sh
    KD, HCN = cfg.KD, cfg.HCN
    HCN2 = HCN // 2

    w1T = np.transpose(w1, (0, 2, 1))                      # [E, D, H]
    w3T = np.transpose(w3, (0, 2, 1))
    w2T = np.transpose(w2, (0, 2, 1))                      # [E, H, D]
    s1T = sw1.T.reshape(D, n_sh, H).transpose(1, 0, 2)     # [n_sh, D, H]
    s3T = sw3.T.reshape(D, n_sh, H).transpose(1, 0, 2)
    s2T = sw2.T.reshape(n_sh, H, D)                        # [n_sh, H, D]
    w1T_all = np.concatenate([w1T, s1T], 0)                # [NV, D, H]
    w3T_all = np.concatenate([w3T, s3T], 0)
    w2T_all = np.concatenate([w2T, s2T], 0)                # [NV, H, D]

    w1t = np.ascontiguousarray(
        w1T_all.reshape(NV, KD, P, HCN, P).transpose(0, 3, 2, 1, 4))
    w3t = np.ascontiguousarray(
        w3T_all.reshape(NV, KD, P, HCN, P).transpose(0, 3, 2, 1, 4))
    w2t = np.ascontiguousarray(
        w2T_all.reshape(NV, HCN, P, D).transpose(0, 2, 1, 3))
    w1h_, w1l_ = _split_fp8(w1t, WS)
    w3h_, w3l_ = _split_fp8(w3t, WS)
    w2h_, w2l_ = _split_fp8(w2t, WS)

    # wq: [NV, HCN2, P, 2, 4, KD, P]
    wq_ = np.stack([w1h_, w1l_, w3h_, w3l_], axis=2)   # [NV, HCN, 4, P, KD, P]
    wq_ = wq_.reshape(NV, HCN2, 2, 4, P, KD, P).transpose(0, 1, 4, 2, 3, 5, 6)
    wq_ = np.ascontiguousarray(wq_)

    # w2q: [NV, P, 2, HCN, D]
    w2q_ = np.ascontiguousarray(
        np.stack([w2h_, w2l_], axis=1).transpose(0, 2, 1, 3, 4))

    b1_all = np.concatenate([b1, sb1.reshape(n_sh, H)], 0)  # [NV, H]
    b3_all = np.concatenate([b3, sb3.reshape(n_sh, H)], 0)
    b1a = b1_all.reshape(NV, HCN, P).transpose(0, 2, 1)     # [NV, P, HCN]
    b3a = b3_all.reshape(NV, HCN, P).transpose(0, 2, 1)
    bq_ = np.ascontiguousarray(
        np.stack([b1a, b3a], axis=2)).astype(np.float32)    # [NV, P, 2, HCN]

    gwt = np.ascontiguousarray(
        gate_w.T.reshape(KD, P, E).transpose(1, 0, 2)).astype(np.float32)

    return dict(
        wq=wq_, w2q=w2q_, bq=bq_,
        b2e8=(WS * b2).astype(BF16),                        # [E, D] x WS
        sb2x=(WS * sb2[None]).astype(BF16),                 # [1, D]
        gwt=gwt, gb=gate_b[None].astype(np.float32),
        ones1=np.ones((1, P), BF16),
        onesc=np.ones((P, 1), BF16),
        lt=np.triu(np.ones((P, P))).astype(BF16),
        ident=np.eye(P).astype(BF16),
        iota=np.tile(np.arange(P, dtype=np.float32), (P, 1)),
    )


def pack_x(cfg: Cfg, x_tokens):
    """x_tokens [T, D] fp32 -> device layouts (gate fp32 + fp8 hi/lo)."""
    T, D = x_tokens.shape
    xT = np.ascontiguousarray(
        x_tokens.T.reshape(cfg.KD, P, T).transpose(1, 0, 2)).astype(np.float32)
    xh, xl = _split_fp8(xT)
    xtok = np.ascontiguousarray(
        x_tokens.reshape(cfg.TT, P, D).transpose(1, 0, 2))
    xth, xtl = _split_fp8(xtok)
    return dict(xT=xT, xtbh=xh, xtbl=xl, xtokh=xth, xtokl=xtl)


def unpack_y(cfg: Cfg, y_dev):
    """y device layout [P, TT, D] -> [T, D]."""
    return np.ascontiguousarray(y_dev.transpose(1, 0, 2).reshape(cfg.T, cfg.D))


_CACHE = {}


def _get_nc(cfg: Cfg):
    key = (cfg.D, cfg.H, cfg.E, cfg.n_sh, cfg.T, cfg.capm)
    if key not in _CACHE:
        _CACHE[key] = build_nc_fp8(cfg)
    return _CACHE[key]


def make_in_maps(cfg: Cfg, inputs):
    static = pack_static(
        cfg,
        np.asarray(inputs["gate_w"], np.float32), np.asarray(inputs["gate_b"], np.float32),
        np.asarray(inputs["w1"], np.float32), np.asarray(inputs["b1"], np.float32),
        np.asarray(inputs["w2"], np.float32), np.asarray(inputs["b2"], np.float32),
        np.asarray(inputs["w3"], np.float32), np.asarray(inputs["b3"], np.float32),
        np.asarray(inputs["sw1"], np.float32), np.asarray(inputs["sb1"], np.float32),
        np.asarray(inputs["sw2"], np.float32), np.asarray(inputs["sb2"], np.float32),
        np.asarray(inputs["sw3"], np.float32), np.asarray(inputs["sb3"], np.float32),
    )
    x = np.asarray(inputs["x"], np.float32)
    B, S, D = x.shape
    xf = x.reshape(-1, D)
    in_maps = []
    for c in range(cfg.n_cores):
        m = dict(static)
        m.update(pack_x(cfg, xf[c * cfg.T:(c + 1) * cfg.T]))
        in_maps.append(m)
    return in_maps


def kernel(**inputs) -> np.ndarray:
    x = np.asarray(inputs["x"], np.float32)
    B, S, D = x.shape
    N = B * S
    cfg = Cfg(D=D, T=N // 8, n_cores=8)
    nc = _get_nc(cfg)
    in_maps = make_in_maps(cfg, inputs)
    res = run_bass_kernel_spmd(nc, in_maps, list(range(cfg.n_cores)))
    outs = [unpack_y(cfg, res.results[c]["y"]) for c in range(cfg.n_cores)]
    return np.concatenate(outs, 0).reshape(B, S, D)
